# revision 17
# baseline (speedup 1.0000x reference)
"""Trainium2 Bass kernel for nn_FEM_35072702939287 (attention + BN + channel gate).

Math (validated in numpy vs reference):
  A^T[s,t] = X_s^T G^T X_t + rX[s] (+ t-only/const terms that drop under
  softmax over s), G = Wk^T Wq, rX = (Wq^T bk)^T X.  The rX term rides as
  a per-partition bias into exp (ACT bias operand / tensor_scalar scalar2),
  so the A matmul contracts over exactly K=64 -> two s-blocks run
  CONCURRENTLY on the PE via row tiling (tile_position (0,0) | (64,0)).
  V^T blocks [V | 1 | rX] come from one matmul per block; PV accumulates
  [V|1]^T exp(A^T) with even blocks on PSUM rows 0:33 and odd blocks on
  rows 64:97 via col tiling (tile_position (0,0) | (0,64)) -> concurrent.
  A combining matmul with lhsT = [Wt^T; 1-row; Wt^T; 1-row] reduces both
  partials and applies the Wt conv in one shot; /D folds in after, bt
  cancels under BN.  BN batch stats all-reduced across 8 cores; a dummy
  all-reduce at kernel start warms the CC engine and absorbs launch skew.
  exp is split across 3 engines: ScalarE exact exp; DVE+GpSimd compute a
  Schraudolph fast exp (i16 = rne(1477.32*(A+bias) + 15316), bitcast f16,
  max rel err ~3%; end-to-end <1e-2 validated vs reference).

Sharding: data-parallel over batch N=16 -> 2 batches per core x 8 cores.
"""

import numpy as np

N_CORES = 8
N, C, T, V = 16, 64, 64, 25
TV = T * V            # 1600
IC = 32
NB = N // N_CORES     # batches per core
EPS = 1e-5
NSB = 13              # 12 full 128-row s-blocks + one 64-row tail
SB = [(j * 128, 128) for j in range(12)] + [(1536, 64)]
PAIRS = [(0, 1), (2, 3), (4, 5), (6, 7), (8, 9), (10, 11), (12, None)]
HALVES = [(0, 800), (800, 1600)]
CH = [(0, 512), (512, 800)]       # psum-bank chunks inside an 800-half tile
# bank-aligned chunks for the 1600-wide PV accumulator (matmul output
# must not cross a 2KB PSUM bank boundary)
CH_PV = [(0, 512), (512, 1024), (1024, 1536), (1536, 1600)]
A2 = 1024.0 * 1.4426950408889634  # fast-exp scale
B2 = 15.0 * 1024.0 - 44.0         # fast-exp shift (rne-optimal C=-44)

# exp engine per tile: 0=ACT exact exp, 1=DVE fast exp (GPSIMD can't read
# PSUM, so it gets the SBUF-only elementwise work instead).
# kinds order per pair-slot: [(j,h0), (j,h1), (j2,h0), (j2,h1)]
KINDS_EVEN = [0, 1, 1, 0]   # most slots: 2 ACT / 2 DVE
KINDS_LIGHT = [0, 1, 1, 1]  # every 3rd slot: 1 ACT / 3 DVE

ROW_TILE = True   # concurrent A-matmul pairs via PE row tiling
COL_TILE = True   # concurrent PV pairs via PE col tiling
N_WARM_MM = 22    # PE warmup matmuls (HAM)
CC_WARM = True    # dummy collective at start


def _build(nc, debug=False):
    import concourse.tile as tile
    from concourse import mybir
    from contextlib import ExitStack

    f32 = mybir.dt.float32
    f16 = mybir.dt.float16
    i16 = mybir.dt.int16
    AF = mybir.ActivationFunctionType
    ALU = mybir.AluOpType
    AX = mybir.AxisListType
    R = C // 16  # 4

    # ---------------- DRAM I/O ----------------
    x_in = nc.dram_tensor("x_in", [NB, C, TV], f32, kind="ExternalInput").ap()
    wq_d = nc.dram_tensor("wq", [IC, C], f32, kind="ExternalInput").ap()
    wk_d = nc.dram_tensor("wk", [IC, C], f32, kind="ExternalInput").ap()
    bk_d = nc.dram_tensor("bk", [IC, 1], f32, kind="ExternalInput").ap()
    wv_d = nc.dram_tensor("wv", [IC, C], f32, kind="ExternalInput").ap()
    bv_d = nc.dram_tensor("bv", [1, IC], f32, kind="ExternalInput").ap()
    wt_d = nc.dram_tensor("wt", [C, IC], f32, kind="ExternalInput").ap()
    gm_d = nc.dram_tensor("gamma", [C, 1], f32, kind="ExternalInput").ap()
    bt2_d = nc.dram_tensor("beta", [C, 1], f32, kind="ExternalInput").ap()
    w1_d = nc.dram_tensor("w1", [R, C], f32, kind="ExternalInput").ap()
    b1_d = nc.dram_tensor("b1", [R, 1], f32, kind="ExternalInput").ap()
    w2_d = nc.dram_tensor("w2", [C, R], f32, kind="ExternalInput").ap()
    b2_d = nc.dram_tensor("b2", [C, 1], f32, kind="ExternalInput").ap()
    out_d = nc.dram_tensor("out", [NB, C, TV], f32, kind="ExternalOutput").ap()
    if debug:
        dbg_eb = nc.dram_tensor("dbg_eb", [128, NSB, TV], f16, kind="ExternalOutput").ap()
        dbg_p2 = nc.dram_tensor("dbg_p2", [NB, C, TV], f32, kind="ExternalOutput").ap()
        dbg_gate = nc.dram_tensor("dbg_gate", [C, NB], f32, kind="ExternalOutput").ap()
        dbg_eba = nc.dram_tensor("dbg_eba", [128, NSB], f32, kind="ExternalOutput").ap()
        dbg_pd = nc.dram_tensor("dbg_pd", [128, TV], f16, kind="ExternalOutput").ap()

    with tile.TileContext(nc) as tc, ExitStack() as ctx:
        consts = ctx.enter_context(tc.tile_pool(name="consts", bufs=1))
        xpool = ctx.enter_context(tc.tile_pool(name="xpool", bufs=2))
        workp = ctx.enter_context(tc.tile_pool(name="workp", bufs=2))
        statp = ctx.enter_context(tc.tile_pool(name="statp", bufs=1))
        psA = ctx.enter_context(tc.tile_pool(name="psA", bufs=1, space="PSUM"))
        psP = ctx.enter_context(tc.tile_pool(name="psP", bufs=1, space="PSUM"))
        dramp = ctx.enter_context(tc.tile_pool(name="dramp", bufs=1, space="DRAM"))

        # ------------- warmup collective: absorbs CC cold start + launch skew
        if CC_WARM:
            ccw_in = dramp.tile([C, 2], f32, name="ccw_in")
            ccw_out = dramp.tile([C, 2], f32, name="ccw_out", addr_space="Shared")
            nc.gpsimd.collective_compute(
                "AllReduce", ALU.add, ins=[ccw_in.opt()], outs=[ccw_out.opt()],
                replica_groups=[list(range(N_CORES))])

        # ---------------- input DMAs first (sync queue) ------------------------
        xa32 = [None] * NB
        for b in range(NB):
            t = xpool.tile([C, TV], f32, name="xa32", tag="xa32")
            xa32[b] = t
            nc.sync.dma_start(out=t, in_=x_in[b])

        # ---------------- PE warmup (HAM): dummy matmuls -----------------------
        wuw = consts.tile([C, 128], f16)
        nc.vector.memset(wuw, 0.0)
        wur = consts.tile([C, 512], f16)
        nc.vector.memset(wur, 0.0)
        for i in range(N_WARM_MM):
            wups = psA.tile([128, 512], f32, name="wups", tag="a1")
            nc.tensor.matmul(wups, lhsT=wuw, rhs=wur, start=True, stop=True)

        # ---------------- ACT table warmup -------------------------------------
        warmz = consts.tile([1, 1], f32)
        nc.vector.memset(warmz, 1.0)
        warmo = consts.tile([1, 1], f32)
        nc.scalar.activation(warmo, warmz, AF.Exp)

        # ---------------- weights ----------------------------------------------
        wq_sb = consts.tile([IC, C], f32)
        nc.gpsimd.dma_start(out=wq_sb, in_=wq_d)
        wkbk = consts.tile([IC, C + 1], f32)
        nc.gpsimd.dma_start(out=wkbk[:, 0:C], in_=wk_d)
        nc.gpsimd.dma_start(out=wkbk[:, C:C + 1], in_=bk_d)

        # wvr [65, 36]: cols 0:32 = [Wv^T; bv], col 32 = ones-row marker,
        # col 33 = [r; 0], cols 34:36 pad
        wvr32 = consts.tile([C + 1, 36], f32)
        nc.vector.memset(wvr32, 0.0)
        nc.gpsimd.dma_start(out=wvr32[0:C, 0:IC], in_=wv_d.rearrange("i c -> c i"))
        nc.gpsimd.dma_start(out=wvr32[C:C + 1, 0:IC], in_=bv_d)
        nc.vector.memset(wvr32[C:C + 1, IC:IC + 1], 1.0)

        # G^T | r = Wq^T @ [Wk | bk]
        psg = psA.tile([C, C + 2], f32, name="psg", tag="a2")
        nc.tensor.matmul(psg[:, 0:C + 1], lhsT=wq_sb, rhs=wkbk, start=True, stop=True)
        gt16 = consts.tile([C, C], f16)
        nc.vector.tensor_copy(gt16, psg[:, 0:C])
        nc.vector.tensor_copy(wvr32[0:C, 33:34], psg[:, C:C + 1])
        wvr = consts.tile([C + 1, 36], f16)
        nc.vector.tensor_copy(wvr, wvr32)

        # wtc [128, 66]: rows 0:32 & 64:96 = Wt^T into cols 0:64; rows 32/96
        # put 1 in col 64 (D pass-through)
        wtT2 = consts.tile([128, C], f32)
        nc.gpsimd.dma_start(out=wtT2[0:IC, :], in_=wt_d.rearrange("c i -> i c"))
        nc.gpsimd.dma_start(out=wtT2[64:64 + IC, :], in_=wt_d.rearrange("c i -> i c"))
        wtc = consts.tile([128, 66], f16)
        nc.vector.memset(wtc, 0.0)
        nc.vector.tensor_copy(wtc[0:IC, 0:C], wtT2[0:IC, :])
        nc.vector.tensor_copy(wtc[64:64 + IC, 0:C], wtT2[64:64 + IC, :])
        nc.vector.memset(wtc[IC:IC + 1, 64:65], 1.0)
        nc.vector.memset(wtc[96:97, 64:65], 1.0)

        ones64 = consts.tile([C + 1, C], f16)
        nc.vector.memset(ones64, 0.0)
        nc.vector.memset(ones64[C:C + 1, :], 1.0)

        w1t = consts.tile([C, R], f32)
        nc.gpsimd.dma_start(out=w1t, in_=w1_d.rearrange("j c -> c j"))
        w2t = consts.tile([R, C], f32)
        nc.gpsimd.dma_start(out=w2t, in_=w2_d.rearrange("c j -> j c"))
        b1_sb = consts.tile([R, 1], f32)
        nc.gpsimd.dma_start(out=b1_sb, in_=b1_d)
        b2_sb = consts.tile([C, 1], f32)
        nc.gpsimd.dma_start(out=b2_sb, in_=b2_d)
        b2n = consts.tile([C, 1], f32)
        nc.vector.tensor_scalar_mul(b2n, b2_sb, -1.0)
        gamma_sb = consts.tile([C, 1], f32)
        nc.gpsimd.dma_start(out=gamma_sb, in_=gm_d)
        beta_sb = consts.tile([C, 1], f32)
        nc.gpsimd.dma_start(out=beta_sb, in_=bt2_d)

        # pd: combined PV partials (f16).  Rows 33:64 / 97:128 are never
        # written and multiply zero weight rows; zero them once so the f16
        # cast garbage can't inject NaN*0.
        pd = statp.tile([128, TV], f16)
        nc.vector.memset(pd[32:64, :], 0.0)
        nc.vector.memset(pd[96:128, :], 0.0)

        # ---------------- per-batch state ----------------
        xa16 = [None] * NB    # [65, TV] f16 : [X; 1]
        xhi = [None] * NB     # [128, TV] f16 : rows 64:128 = X replica
        ha = [None] * NB      # [128, TV] f16 : G X replicated in both halves
        vt1 = [None] * NB     # [128, NSB, 33] f16 : [V | 1] per s-block
        eba = [None] * NB     # [128, NSB] f32 : rX bias per s-block
        ebb = [None] * NB     # [128, NSB] f32 : scaled fast-exp bias
        eb = [None] * NB      # [128, NSB, TV] f16 : exp(A^T)
        p2 = [None] * NB      # [64, TV] f32
        wts = [None] * NB     # [64, TV] f32 : gate * p2
        avgs = statp.tile([C, NB], f32)
        stats = statp.tile([C, NB * 4, 6], f32)

        def prologue(b):
            x16 = xpool.tile([C + 1, TV], f16, name="xa16", tag="xa16")
            xa16[b] = x16
            if b == 0:
                nc.vector.tensor_copy(x16[0:C, :], xa32[b])
            else:
                nc.gpsimd.tensor_copy(x16[0:C, :], xa32[b])
            nc.gpsimd.memset(x16[C:C + 1, :], 1.0)
            xh = xpool.tile([128, TV], f16, name="xhi", tag="xhi")
            xhi[b] = xh
            nc.sync.dma_start(out=xh[64:128, :], in_=x16[0:C, :])
            nc.vector.reduce_sum(avgs[:, b:b + 1], xa32[b], axis=AX.X)

            # V^T | 1 | rX for all 13 blocks into one psum bank
            vps = psA.tile([128, NSB, 36], f32, name="vps", tag="a1")
            for j, (off, p) in enumerate(SB):
                nc.tensor.matmul(vps[0:p, j, :], lhsT=x16[:, off:off + p],
                                 rhs=wvr, start=True, stop=True)
            v = xpool.tile([128, NSB, 33], f16, name="vt1", tag="vt1")
            vt1[b] = v
            nc.vector.tensor_copy(v, vps[:, :, 0:33])
            ea = xpool.tile([128, NSB], f32, name="eba", tag="eba")
            eba[b] = ea
            nc.vector.tensor_copy(ea, vps[:, :, 33])
            bbt = xpool.tile([128, NSB], f32, name="ebb", tag="ebb")
            ebb[b] = bbt
            nc.vector.tensor_scalar(bbt, ea, A2, B2, op0=ALU.mult, op1=ALU.add)

            # H = G X, replicated into partition halves via col tiling
            h = xpool.tile([128, TV], f16, name="ha", tag="ha")
            ha[b] = h
            for hi, (h0, h1) in enumerate(HALVES):
                hps = psA.tile([128, 800], f32, name="hps", tag="a2")
                for (c0, c1) in CH:
                    nc.tensor.matmul(hps[0:C, c0:c1], lhsT=gt16,
                                     rhs=x16[0:C, h0 + c0:h0 + c1],
                                     start=True, stop=True, tile_position=(0, 0))
                    nc.tensor.matmul(hps[64:128, c0:c1], lhsT=gt16,
                                     rhs=x16[0:C, h0 + c0:h0 + c1],
                                     start=True, stop=True, tile_position=(0, 64))
                nc.vector.tensor_copy(h[:, h0:h1], hps)

            eb[b] = xpool.tile([128, NSB, TV], f16, name="eb", tag="eb")
            p2[b] = xpool.tile([C, TV], f32, name="p2", tag="p2")
            wts[b] = xpool.tile([C, TV], f32, name="wts", tag="wts")

        def emit_exp(b, j, p, h0, aps, kind):
            dst = eb[b][0:p, j, h0:h0 + 800]
            if kind == 0:
                nc.scalar.activation(dst, aps[0:p, :], AF.Exp,
                                     bias=eba[b][0:p, j:j + 1])
            else:
                nc.vector.tensor_scalar(dst.bitcast(i16), aps[0:p, :], A2,
                                        ebb[b][0:p, j:j + 1],
                                        op0=ALU.mult, op1=ALU.add)

        def emit_pv(b, jj, pacc):
            if jj is None:
                return
            j, j2 = jj
            p = SB[j][1]
            for (c0, c1) in CH_PV:
                nc.tensor.matmul(pacc[0:33, c0:c1],
                                 lhsT=vt1[b][0:p, j, :],
                                 rhs=eb[b][0:p, j, c0:c1],
                                 start=(j == 0), stop=(j == 12),
                                 tile_position=(0, 0),
                                 skip_group_check=True)
                if j2 is not None:
                    nc.tensor.matmul(pacc[64:97, c0:c1],
                                     lhsT=vt1[b][:, j2, :],
                                     rhs=eb[b][:, j2, c0:c1],
                                     start=(j2 == 1), stop=(j2 == 11),
                                     tile_position=(0, 64),
                                     skip_group_check=True)

        def phase1(b):
            pacc = psP.tile([128, TV], f32, name="pacc", tag="pacc")
            prev = None
            for si, (j, j2) in enumerate(PAIRS):
                off, p = SB[j]
                kinds = KINDS_LIGHT if si % 3 == 2 else KINDS_EVEN
                tiles = []
                for hi, (h0, h1) in enumerate(HALVES):
                    aA = psA.tile([128, 800], f32, name="apsA", tag="a1")
                    for (c0, c1) in CH:
                        nc.tensor.matmul(aA[0:p, c0:c1],
                                         lhsT=ha[b][0:C, off:off + p],
                                         rhs=xa16[b][0:C, h0 + c0:h0 + c1],
                                         start=True, stop=True,
                                         tile_position=(0, 0) if ROW_TILE else None)
                    tiles.append((j, p, h0, aA, kinds[hi]))
                    if j2 is not None:
                        off2, p2_ = SB[j2]
                        aB = psA.tile([128, 800], f32, name="apsB", tag="a2")
                        for (c0, c1) in CH:
                            if ROW_TILE:
                                nc.tensor.matmul(aB[:, c0:c1],
                                                 lhsT=ha[b][64:128, off2:off2 + p2_],
                                                 rhs=xhi[b][64:128, h0 + c0:h0 + c1],
                                                 start=True, stop=True,
                                                 tile_position=(64, 0))
                            else:
                                nc.tensor.matmul(aB[:, c0:c1],
                                                 lhsT=ha[b][0:C, off2:off2 + p2_],
                                                 rhs=xa16[b][0:C, h0 + c0:h0 + c1],
                                                 start=True, stop=True)
                        tiles.append((j2, p2_, h0, aB, kinds[2 + hi]))
                emit_pv(b, prev, pacc)
                for (tj, tp, th, taps, tk) in tiles:
                    emit_exp(b, tj, tp, th, taps, tk)
                prev = (j, j2)
            emit_pv(b, prev, pacc)
            return pacc

        def remainder(b, pacc):
            nc.vector.tensor_copy(pd[0:33, :], pacc[0:33, :])
            nc.vector.tensor_copy(pd[64:97, :], pacc[64:97, :])
            if debug and b == 0:
                nc.sync.dma_start(out=dbg_pd, in_=pd)
            for ti, (h0, h1) in enumerate(HALVES):
                p2ps = psA.tile([66, 800], f32, name="p2ps", tag="a1")
                for (c0, c1) in CH:
                    nc.tensor.matmul(p2ps[:, c0:c1], lhsT=wtc,
                                     rhs=pd[:, h0 + c0:h0 + c1],
                                     start=True, stop=True)
                rdt = workp.tile([C + 1, 800], f16, name="rdt", tag="rdt")
                with nc.allow_low_precision(reason="1/D in f16 validated vs reference"):
                    nc.vector.reciprocal(rdt[C:C + 1, :], p2ps[64:65, :])
                drep = psA.tile([C, 800], f32, name="drep", tag="a2")
                for (c0, c1) in CH:
                    nc.tensor.matmul(drep[:, c0:c1], lhsT=ones64[C:C + 1, :],
                                     rhs=rdt[C:C + 1, c0:c1], start=True, stop=True)
                rrep = workp.tile([C, 800], f32, name="rrep", tag="rrep")
                nc.vector.tensor_copy(rrep, drep)
                nc.vector.tensor_tensor(out=p2[b][:, h0:h1], in0=p2ps[0:C, :],
                                        in1=rrep, op=ALU.mult)
                nc.vector.bn_stats(stats[:, 4 * b + 2 * ti, :],
                                   p2[b][:, h0:h0 + 512])
                nc.vector.bn_stats(stats[:, 4 * b + 2 * ti + 1, :],
                                   p2[b][:, h0 + 512:h1])

        # ---------------- main schedule ----------------
        prologue(0)
        pa0 = phase1(0)
        remainder(0, pa0)
        prologue(1)
        pa1 = phase1(1)

        # channel gate (overlaps phase1(1) tail)
        hps2 = psA.tile([R, NB], f32, name="hps2", tag="a2")
        nc.tensor.matmul(hps2, lhsT=w1t, rhs=avgs, start=True, stop=True)
        h_pre = statp.tile([R, NB], f32)
        nc.vector.tensor_scalar(h_pre, hps2, 1.0 / TV, b1_sb,
                                op0=ALU.mult, op1=ALU.add)
        h_sb = statp.tile([R, NB], f32)
        nc.vector.tensor_scalar_max(h_sb, h_pre, 0.0)
        zps = psA.tile([C, NB], f32, name="zps", tag="a2")
        nc.tensor.matmul(zps, lhsT=w2t, rhs=h_sb, start=True, stop=True)
        eg = statp.tile([C, NB], f32)
        nc.scalar.activation(eg, zps, AF.Exp, bias=b2n, scale=-1.0)
        gp1 = statp.tile([C, NB], f32)
        nc.vector.tensor_scalar_add(gp1, eg, 1.0)
        gate = statp.tile([C, NB], f32)
        nc.vector.reciprocal(gate, gp1)

        remainder(1, pa1)
        nc.gpsimd.tensor_scalar_mul(wts[0], p2[0], gate[:, 0:1])
        nc.gpsimd.tensor_scalar_mul(wts[1], p2[1], gate[:, 1:2])

        # ---------------- BN stats -> allreduce -> global ----------------
        mv = statp.tile([C, 2], f32)
        nc.vector.bn_aggr(out=mv, in_=stats)
        m2 = statp.tile([C, 1], f32)
        nc.vector.tensor_mul(m2, mv[:, 0:1], mv[:, 0:1])
        ex2 = statp.tile([C, 1], f32)
        nc.vector.tensor_add(ex2, mv[:, 1:2], m2)
        sums = statp.tile([C, 2], f32)
        cnt_local = float(NB * TV)
        nc.vector.tensor_scalar_mul(sums[:, 0:1], mv[:, 0:1], cnt_local)
        nc.vector.tensor_scalar_mul(sums[:, 1:2], ex2, cnt_local)

        cc_in = dramp.tile([C, 2], f32, name="cc_in")
        cc_out = dramp.tile([C, 2], f32, name="cc_out", addr_space="Shared")
        nc.sync.dma_start(out=cc_in, in_=sums)
        nc.gpsimd.collective_compute(
            "AllReduce", ALU.add, ins=[cc_in.opt()], outs=[cc_out.opt()],
            replica_groups=[list(range(N_CORES))])
        gs = statp.tile([C, 2], f32)
        nc.sync.dma_start(out=gs, in_=cc_out)

        # mean/var -> sc, nsh (short chain)
        inv_cnt = 1.0 / (N * TV)
        mv2 = statp.tile([C, 2], f32)
        nc.vector.tensor_scalar_mul(mv2, gs, inv_cnt)
        nve = statp.tile([C, 1], f32)
        nc.vector.scalar_tensor_tensor(out=nve, in0=mv2[:, 0:1],
                                       scalar=mv2[:, 0:1], in1=mv2[:, 1:2],
                                       op0=ALU.mult, op1=ALU.subtract)
        ve = statp.tile([C, 1], f32)
        nc.vector.tensor_scalar(ve, nve, -1.0, EPS, op0=ALU.mult, op1=ALU.add)
        sq = statp.tile([C, 1], f32)
        nc.scalar.activation(sq, ve, AF.Sqrt)
        rstd = statp.tile([C, 1], f32)
        nc.vector.reciprocal(rstd, sq)
        sc = statp.tile([C, 1], f32)
        nc.vector.tensor_mul(sc, gamma_sb, rstd)
        nsh = statp.tile([C, 1], f32)
        nc.vector.scalar_tensor_tensor(out=nsh, in0=mv2[:, 0:1], scalar=sc,
                                       in1=beta_sb, op0=ALU.mult, op1=ALU.subtract)

        if debug:
            nc.sync.dma_start(out=dbg_eb, in_=eb[0])
            for _b in range(NB):
                nc.sync.dma_start(out=dbg_p2[_b], in_=p2[_b])
            nc.sync.dma_start(out=dbg_gate, in_=gate)
            nc.sync.dma_start(out=dbg_eba, in_=eba[0])

        # ------------- finalize: out = sc*(gate*p2) + (x - gate*nsh) -----------
        for b in range(NB):
            d_b = statp.tile([C, 1], f32, name=f"d_{b}")
            nc.vector.tensor_mul(d_b, gate[:, b:b + 1], nsh)
            x3 = workp.tile([C, TV], f32, name="x3", tag="x3")
            nc.gpsimd.tensor_scalar(x3, xa32[b], d_b, None, op0=ALU.subtract)
            for (h0, h1) in HALVES:
                osb = workp.tile([C, 800], f32, name="osb", tag="osb")
                nc.vector.scalar_tensor_tensor(out=osb, in0=wts[b][:, h0:h1],
                                               scalar=sc, in1=x3[:, h0:h1],
                                               op0=ALU.mult, op1=ALU.add)
                nc.sync.dma_start(out=out_d[b][:, h0:h1], in_=osb)


_CACHE = {}


def _get_compiled(debug=False):
    key = ("nc", debug)
    if key in _CACHE:
        return _CACHE[key]
    import concourse.bacc as bacc

    nc = bacc.Bacc("TRN2", target_bir_lowering=False, debug=False,
                   enable_asserts=False, num_devices=N_CORES)
    _build(nc, debug=debug)
    nc.compile()
    _CACHE[key] = nc
    return nc


def _run(inputs, trace=False, debug=False, **kw):
    from concourse import bass_utils

    nc = _get_compiled(debug=debug)
    x = np.ascontiguousarray(np.asarray(inputs["x"], dtype=np.float32))
    x = x.reshape(N, C, TV)
    f = lambda a: np.ascontiguousarray(np.asarray(a, dtype=np.float32))
    common = {
        "wq": f(inputs["Wq"]),
        "wk": f(inputs["Wk"]),
        "bk": f(inputs["bk"]).reshape(IC, 1),
        "wv": f(inputs["Wv"]),
        "bv": f(inputs["bv"]).reshape(1, IC),
        "wt": f(inputs["Wt"]),
        "gamma": f(inputs["gamma"]).reshape(C, 1),
        "beta": f(inputs["beta"]).reshape(C, 1),
        "w1": f(inputs["W1"]),
        "b1": f(inputs["b1"]).reshape(C // 16, 1),
        "w2": f(inputs["W2"]),
        "b2": f(inputs["b2"]).reshape(C, 1),
    }
    in_maps = []
    for c in range(N_CORES):
        m = dict(common)
        m["x_in"] = np.ascontiguousarray(x[c * NB:(c + 1) * NB])
        in_maps.append(m)
    try:
        res = bass_utils.run_bass_kernel_spmd(
            nc, in_maps, core_ids=list(range(N_CORES)), trace=trace, **kw)
    except Exception:
        import time as _time
        _time.sleep(5)
        res = bass_utils.run_bass_kernel_spmd(
            nc, in_maps, core_ids=list(range(N_CORES)), trace=False, **kw)
    out = np.concatenate([res.results[c]["out"] for c in range(N_CORES)], axis=0)
    return out.reshape(N, C, T, V).astype(np.float32), res


def kernel(**inputs):
    return _run(inputs, trace=False)[0]


# revision 25
# speedup vs baseline: 1.4715x; 1.4715x over previous
"""Trainium2 Bass kernel for nn_FEM_35072702939287 (attention + BN + channel gate).

Math (validated in numpy vs reference):
  A^T[s,t] = X_s^T G^T X_t + rX[s] (+ t-only/const terms that drop under
  softmax over s), G = Wk^T Wq, rX = (Wq^T bk)^T X.  The rX term rides as
  a per-partition bias into exp (ACT bias operand / tensor_scalar scalar2),
  so the A matmul contracts over exactly K=64 -> two s-blocks run
  CONCURRENTLY on the PE via row tiling (tile_position (0,0) | (64,0)).
  V^T blocks [V | 1 | rX] come from one matmul per block; PV accumulates
  [V|1]^T exp(A^T) with even blocks on PSUM rows 0:33 and odd blocks on
  rows 64:97 via col tiling (tile_position (0,0) | (0,64)) -> concurrent.
  A combining matmul with lhsT = [Wt^T; 1-row; Wt^T; 1-row] reduces both
  partials and applies the Wt conv in one shot; /D folds in after, bt
  cancels under BN.  BN batch stats all-reduced across 8 cores; a dummy
  all-reduce at kernel start warms the CC engine and absorbs launch skew.
  exp is split across 3 engines: ScalarE exact exp; DVE+GpSimd compute a
  Schraudolph fast exp (i16 = rne(1477.32*(A+bias) + 15316), bitcast f16,
  max rel err ~3%; end-to-end <1e-2 validated vs reference).

Sharding: data-parallel over batch N=16 -> 2 batches per core x 8 cores.
"""

import numpy as np

N_CORES = 8
N, C, T, V = 16, 64, 64, 25
TV = T * V            # 1600
IC = 32
NB = N // N_CORES     # batches per core
EPS = 1e-5
NSB = 13              # 12 full 128-row s-blocks + one 64-row tail
SB = [(j * 128, 128) for j in range(12)] + [(1536, 64)]
PAIRS = [(0, 1), (2, 3), (4, 5), (6, 7), (8, 9), (10, 11), (12, None)]
HALVES = [(0, 800), (800, 1600)]
CH = [(0, 512), (512, 800)]       # psum-bank chunks inside an 800-half tile
# bank-aligned chunks for the 1600-wide PV accumulator (matmul output
# must not cross a 2KB PSUM bank boundary)
CH_PV = [(0, 512), (512, 1024), (1024, 1536), (1536, 1600)]
A2 = 1024.0 * 1.4426950408889634  # fast-exp scale
B2 = 15.0 * 1024.0 - 44.0         # fast-exp shift (rne-optimal C=-44)

# exp engine per tile: 0=ACT exact exp, 1=DVE fast exp (GPSIMD can't read
# PSUM, so it gets the SBUF-only elementwise work instead).
# kinds order per pair-slot: [(j,h0), (j,h1), (j2,h0), (j2,h1)]
KINDS_EVEN = [0, 1, 0, 1]   # 2 ACT / 2 DVE
KINDS_LIGHT = [0, 1, 0, 1]

ROW_TILE = True   # concurrent A-matmul pairs via PE row tiling
COL_TILE = True   # concurrent PV pairs via PE col tiling
N_WARM_MM = 22    # PE warmup matmuls (HAM)
CC_WARM = True    # dummy collective at start


def _build(nc, debug=False):
    import concourse.tile as tile
    from concourse import mybir
    from contextlib import ExitStack

    f32 = mybir.dt.float32
    f16 = mybir.dt.float16
    i16 = mybir.dt.int16
    AF = mybir.ActivationFunctionType
    ALU = mybir.AluOpType
    AX = mybir.AxisListType
    R = C // 16  # 4

    # ---------------- DRAM I/O ----------------
    x_in = nc.dram_tensor("x_in", [NB, C, TV], f32, kind="ExternalInput").ap()
    wq_d = nc.dram_tensor("wq", [IC, C], f32, kind="ExternalInput").ap()
    wk_d = nc.dram_tensor("wk", [IC, C], f32, kind="ExternalInput").ap()
    bk_d = nc.dram_tensor("bk", [IC, 1], f32, kind="ExternalInput").ap()
    wv_d = nc.dram_tensor("wv", [IC, C], f32, kind="ExternalInput").ap()
    bv_d = nc.dram_tensor("bv", [1, IC], f32, kind="ExternalInput").ap()
    wt_d = nc.dram_tensor("wt", [C, IC], f32, kind="ExternalInput").ap()
    gm_d = nc.dram_tensor("gamma", [C, 1], f32, kind="ExternalInput").ap()
    bt2_d = nc.dram_tensor("beta", [C, 1], f32, kind="ExternalInput").ap()
    w1_d = nc.dram_tensor("w1", [R, C], f32, kind="ExternalInput").ap()
    b1_d = nc.dram_tensor("b1", [R, 1], f32, kind="ExternalInput").ap()
    w2_d = nc.dram_tensor("w2", [C, R], f32, kind="ExternalInput").ap()
    b2_d = nc.dram_tensor("b2", [C, 1], f32, kind="ExternalInput").ap()
    out_d = nc.dram_tensor("out", [NB, C, TV], f32, kind="ExternalOutput").ap()
    if debug:
        dbg_eb = nc.dram_tensor("dbg_eb", [128, NSB, TV], f16, kind="ExternalOutput").ap()
        dbg_p2 = nc.dram_tensor("dbg_p2", [NB, C, TV], f32, kind="ExternalOutput").ap()
        dbg_gate = nc.dram_tensor("dbg_gate", [C, NB], f32, kind="ExternalOutput").ap()
        dbg_eba = nc.dram_tensor("dbg_eba", [128, NSB], f32, kind="ExternalOutput").ap()
        dbg_pd = nc.dram_tensor("dbg_pd", [128, TV], f16, kind="ExternalOutput").ap()

    with tile.TileContext(nc) as tc, ExitStack() as ctx:
        consts = ctx.enter_context(tc.tile_pool(name="consts", bufs=1))
        xpool = ctx.enter_context(tc.tile_pool(name="xpool", bufs=2))
        workp = ctx.enter_context(tc.tile_pool(name="workp", bufs=2))
        statp = ctx.enter_context(tc.tile_pool(name="statp", bufs=1))
        psA = ctx.enter_context(tc.tile_pool(name="psA", bufs=1, space="PSUM"))
        psP = ctx.enter_context(tc.tile_pool(name="psP", bufs=1, space="PSUM"))
        dramp = ctx.enter_context(tc.tile_pool(name="dramp", bufs=1, space="DRAM"))

        # ------------- warmup collective: absorbs CC cold start + launch skew
        if CC_WARM:
            ccw_in = dramp.tile([C, 2], f32, name="ccw_in")
            ccw_out = dramp.tile([C, 2], f32, name="ccw_out", addr_space="Shared")
            nc.gpsimd.collective_compute(
                "AllReduce", ALU.add, ins=[ccw_in.opt()], outs=[ccw_out.opt()],
                replica_groups=[list(range(N_CORES))])

        # ---------------- input DMAs first (sync queue) ------------------------
        xa32 = [None] * NB
        for b in range(NB):
            t = xpool.tile([C, TV], f32, name="xa32", tag="xa32")
            xa32[b] = t
            nc.sync.dma_start(out=t, in_=x_in[b])

        # ---------------- PE warmup (HAM): dummy matmuls -----------------------
        wuw = consts.tile([C, 128], f16)
        nc.vector.memset(wuw, 0.0)
        wur = consts.tile([C, 512], f16)
        nc.vector.memset(wur, 0.0)
        for i in range(N_WARM_MM):
            wups = psA.tile([128, 512], f32, name="wups", tag="a1")
            nc.tensor.matmul(wups, lhsT=wuw, rhs=wur, start=True, stop=True)

        # ---------------- ACT table warmup -------------------------------------
        warmz = consts.tile([1, 1], f32)
        nc.vector.memset(warmz, 1.0)
        warmo = consts.tile([1, 1], f32)
        nc.scalar.activation(warmo, warmz, AF.Exp)

        # ---------------- weights ----------------------------------------------
        wq_sb = consts.tile([IC, C], f32)
        nc.gpsimd.dma_start(out=wq_sb, in_=wq_d)
        wkbk = consts.tile([IC, C + 1], f32)
        nc.gpsimd.dma_start(out=wkbk[:, 0:C], in_=wk_d)
        nc.gpsimd.dma_start(out=wkbk[:, C:C + 1], in_=bk_d)

        # wvr [65, 36]: cols 0:32 = [Wv^T; bv], col 32 = ones-row marker,
        # col 33 = [r; 0], cols 34:36 pad
        wvr32 = consts.tile([C + 1, 36], f32)
        nc.vector.memset(wvr32, 0.0)
        nc.gpsimd.dma_start(out=wvr32[0:C, 0:IC], in_=wv_d.rearrange("i c -> c i"))
        nc.gpsimd.dma_start(out=wvr32[C:C + 1, 0:IC], in_=bv_d)
        nc.vector.memset(wvr32[C:C + 1, IC:IC + 1], 1.0)

        # G^T | r = Wq^T @ [Wk | bk]
        psg = psA.tile([C, C + 2], f32, name="psg", tag="a2")
        nc.tensor.matmul(psg[:, 0:C + 1], lhsT=wq_sb, rhs=wkbk, start=True, stop=True)
        gt16 = consts.tile([C, C], f16)
        nc.vector.tensor_copy(gt16, psg[:, 0:C])
        nc.vector.tensor_copy(wvr32[0:C, 33:34], psg[:, C:C + 1])
        wvr = consts.tile([C + 1, 36], f16)
        nc.vector.tensor_copy(wvr, wvr32)

        # wtc [128, 66]: rows 0:32 & 64:96 = Wt^T into cols 0:64; rows 32/96
        # put 1 in col 64 (D pass-through)
        wtT2 = consts.tile([128, C], f32)
        nc.gpsimd.dma_start(out=wtT2[0:IC, :], in_=wt_d.rearrange("c i -> i c"))
        nc.gpsimd.dma_start(out=wtT2[64:64 + IC, :], in_=wt_d.rearrange("c i -> i c"))
        wtc = consts.tile([128, 66], f32)
        nc.vector.memset(wtc, 0.0)
        nc.vector.tensor_copy(wtc[0:IC, 0:C], wtT2[0:IC, :])
        nc.vector.tensor_copy(wtc[64:64 + IC, 0:C], wtT2[64:64 + IC, :])
        nc.vector.memset(wtc[IC:IC + 1, 64:65], 1.0)
        nc.vector.memset(wtc[96:97, 64:65], 1.0)
        f32r = mybir.dt.float32r
        wtc_r = consts.tile([128, 66], f32r)
        nc.vector.tensor_copy(wtc_r, wtc)

        ones64 = consts.tile([C + 1, C], f16)
        nc.vector.memset(ones64, 0.0)
        nc.vector.memset(ones64[C:C + 1, :], 1.0)

        w1t = consts.tile([C, R], f32)
        nc.gpsimd.dma_start(out=w1t, in_=w1_d.rearrange("j c -> c j"))
        w2t = consts.tile([R, C], f32)
        nc.gpsimd.dma_start(out=w2t, in_=w2_d.rearrange("c j -> j c"))
        b1_sb = consts.tile([R, 1], f32)
        nc.gpsimd.dma_start(out=b1_sb, in_=b1_d)
        b2_sb = consts.tile([C, 1], f32)
        nc.gpsimd.dma_start(out=b2_sb, in_=b2_d)
        b2n = consts.tile([C, 1], f32)
        nc.vector.tensor_scalar_mul(b2n, b2_sb, -1.0)
        gamma_sb = consts.tile([C, 1], f32)
        nc.gpsimd.dma_start(out=gamma_sb, in_=gm_d)
        beta_sb = consts.tile([C, 1], f32)
        nc.gpsimd.dma_start(out=beta_sb, in_=bt2_d)

        # pd: combined PV partials (f16).  Rows 33:64 / 97:128 are never
        # written and multiply zero weight rows; zero them once so the f16
        # cast garbage can't inject NaN*0.
        pd = statp.tile([128, TV], f32r)
        nc.vector.memset(pd.bitcast(f32)[32:64, :], 0.0)
        nc.vector.memset(pd.bitcast(f32)[96:128, :], 0.0)

        # ---------------- per-batch state ----------------
        xa16 = [None] * NB    # [65, TV] f16 : [X; 1]
        xhi = [None] * NB     # [128, TV] f16 : rows 64:128 = X replica
        ha = [None] * NB      # [128, TV] f16 : G X replicated in both halves
        vt1 = [None] * NB     # [128, NSB, 33] f16 : [V | 1] per s-block
        eba = [None] * NB     # [128, NSB] f32 : rX bias per s-block
        ebb = [None] * NB     # [128, NSB] f32 : scaled fast-exp bias
        eb = [None] * NB      # [128, NSB, TV] f16 : exp(A^T)
        p2 = [None] * NB      # [64, TV] f32
        wts = [None] * NB     # [64, TV] f32 : gate * p2
        avgs = statp.tile([C, NB], f32)
        stats = statp.tile([C, NB * 4, 6], f32)

        def prologue(b):
            x16 = xpool.tile([C + 1, TV], f16, name="xa16", tag="xa16")
            xa16[b] = x16
            # cast + row-sum (for the gate) in one ACT pass
            nc.scalar.activation(x16[0:C, :], xa32[b], AF.Copy,
                                 accum_out=avgs[:, b:b + 1])
            nc.gpsimd.memset(x16[C:C + 1, :], 1.0)
            xh = xpool.tile([128, TV], f16, name="xhi", tag="xhi")
            xhi[b] = xh
            nc.sync.dma_start(out=xh[64:128, :], in_=x16[0:C, :])

            # V^T | 1 | rX for all 13 blocks into one psum bank
            vps = psA.tile([128, NSB, 36], f32, name="vps", tag="a1")
            for j, (off, p) in enumerate(SB):
                nc.tensor.matmul(vps[0:p, j, :], lhsT=x16[:, off:off + p],
                                 rhs=wvr, start=True, stop=True)
            v = xpool.tile([128, NSB, 33], f16, name="vt1", tag="vt1")
            vt1[b] = v
            nc.vector.tensor_copy(v, vps[:, :, 0:33])
            ea = xpool.tile([128, NSB], f32, name="eba", tag="eba")
            eba[b] = ea
            nc.vector.tensor_copy(ea, vps[:, :, 33])
            bbt = xpool.tile([128, NSB], f32, name="ebb", tag="ebb")
            ebb[b] = bbt
            nc.vector.tensor_scalar(bbt, ea, A2, B2, op0=ALU.mult, op1=ALU.add)

            # H = G X, replicated into partition halves via col tiling
            h = xpool.tile([128, TV], f16, name="ha", tag="ha")
            ha[b] = h
            for hi, (h0, h1) in enumerate(HALVES):
                hps = psA.tile([128, 800], f32, name="hps", tag="a2")
                for (c0, c1) in CH:
                    nc.tensor.matmul(hps[0:C, c0:c1], lhsT=gt16,
                                     rhs=x16[0:C, h0 + c0:h0 + c1],
                                     start=True, stop=True, tile_position=(0, 0))
                    nc.tensor.matmul(hps[64:128, c0:c1], lhsT=gt16,
                                     rhs=x16[0:C, h0 + c0:h0 + c1],
                                     start=True, stop=True, tile_position=(0, 64))
                nc.vector.tensor_copy(h[:, h0:h1], hps)

            eb[b] = xpool.tile([128, NSB, TV], f16, name="eb", tag="eb")
            p2[b] = xpool.tile([C, TV], f32, name="p2", tag="p2")
            wts[b] = xpool.tile([C, TV], f32, name="wts", tag="wts")

        def emit_exp(b, j, p, h0, aps, kind):
            dst = eb[b][0:p, j, h0:h0 + 800]
            if kind == 0:
                nc.scalar.activation(dst, aps[0:p, :], AF.Exp,
                                     bias=eba[b][0:p, j:j + 1])
            else:
                nc.vector.tensor_scalar(dst.bitcast(i16), aps[0:p, :], A2,
                                        ebb[b][0:p, j:j + 1],
                                        op0=ALU.mult, op1=ALU.add)

        def emit_pv(b, jj, pacc):
            if jj is None:
                return
            j, j2 = jj
            p = SB[j][1]
            for (c0, c1) in CH_PV:
                nc.tensor.matmul(pacc[0:33, c0:c1],
                                 lhsT=vt1[b][0:p, j, :],
                                 rhs=eb[b][0:p, j, c0:c1],
                                 start=(j == 0), stop=(j == 12),
                                 tile_position=(0, 0),
                                 skip_group_check=True)
                if j2 is not None:
                    nc.tensor.matmul(pacc[64:97, c0:c1],
                                     lhsT=vt1[b][:, j2, :],
                                     rhs=eb[b][:, j2, c0:c1],
                                     start=(j2 == 1), stop=(j2 == 11),
                                     tile_position=(0, 64),
                                     skip_group_check=True)

        def phase1(b):
            pacc = psP.tile([128, TV], f32, name="pacc", tag="pacc")
            prev = None
            for si, (j, j2) in enumerate(PAIRS):
                off, p = SB[j]
                kinds = KINDS_LIGHT if si % 3 == 2 else KINDS_EVEN
                tiles = []
                for hi, (h0, h1) in enumerate(HALVES):
                    aA = psA.tile([128, 800], f32, name="apsA", tag="a1")
                    for (c0, c1) in CH:
                        nc.tensor.matmul(aA[0:p, c0:c1],
                                         lhsT=ha[b][0:C, off:off + p],
                                         rhs=xa16[b][0:C, h0 + c0:h0 + c1],
                                         start=True, stop=True,
                                         tile_position=(0, 0) if ROW_TILE else None)
                    tiles.append((j, p, h0, aA, kinds[hi]))
                    if j2 is not None:
                        off2, p2_ = SB[j2]
                        aB = psA.tile([128, 800], f32, name="apsB", tag="a2")
                        for (c0, c1) in CH:
                            if ROW_TILE:
                                nc.tensor.matmul(aB[:, c0:c1],
                                                 lhsT=ha[b][64:128, off2:off2 + p2_],
                                                 rhs=xhi[b][64:128, h0 + c0:h0 + c1],
                                                 start=True, stop=True,
                                                 tile_position=(64, 0))
                            else:
                                nc.tensor.matmul(aB[:, c0:c1],
                                                 lhsT=ha[b][0:C, off2:off2 + p2_],
                                                 rhs=xa16[b][0:C, h0 + c0:h0 + c1],
                                                 start=True, stop=True)
                        tiles.append((j2, p2_, h0, aB, kinds[2 + hi]))
                emit_pv(b, prev, pacc)
                for (tj, tp, th, taps, tk) in tiles:
                    emit_exp(b, tj, tp, th, taps, tk)
                prev = (j, j2)
            emit_pv(b, prev, pacc)
            return pacc

        def remainder(b, pacc):
            nc.vector.tensor_copy(pd[0:33, :], pacc[0:33, :])
            nc.vector.tensor_copy(pd[64:97, :], pacc[64:97, :])
            if debug and b == 0:
                nc.sync.dma_start(out=dbg_pd, in_=pd)
            for ti, (h0, h1) in enumerate(HALVES):
                p2ps = psA.tile([66, 800], f32, name="p2ps", tag="a1")
                for (c0, c1) in CH:
                    nc.tensor.matmul(p2ps[:, c0:c1], lhsT=wtc_r,
                                     rhs=pd[:, h0 + c0:h0 + c1],
                                     start=True, stop=True)
                rdt = workp.tile([C + 1, 800], f16, name="rdt", tag="rdt")
                nc.vector.tensor_copy(rdt[C:C + 1, :], p2ps[64:65, :])
                drep = psA.tile([C, 800], f32, name="drep", tag="a2")
                for (c0, c1) in CH:
                    nc.tensor.matmul(drep[:, c0:c1], lhsT=ones64[C:C + 1, :],
                                     rhs=rdt[C:C + 1, c0:c1], start=True, stop=True)
                rrep = workp.tile([C, 800], f32, name="rrep", tag="rrep")
                nc.vector.reciprocal_approx_fast(out=rrep, in_=drep)
                nc.vector.tensor_tensor(out=p2[b][:, h0:h1], in0=p2ps[0:C, :],
                                        in1=rrep, op=ALU.mult)
                nc.vector.bn_stats(stats[:, 4 * b + 2 * ti, :],
                                   p2[b][:, h0:h0 + 512])
                nc.vector.bn_stats(stats[:, 4 * b + 2 * ti + 1, :],
                                   p2[b][:, h0 + 512:h1])

        # ---------------- main schedule ----------------
        prologue(0)
        pa0 = phase1(0)
        remainder(0, pa0)
        prologue(1)
        pa1 = phase1(1)

        # channel gate (overlaps phase1(1) tail)
        hps2 = psA.tile([R, NB], f32, name="hps2", tag="a2")
        nc.tensor.matmul(hps2, lhsT=w1t, rhs=avgs, start=True, stop=True)
        h_pre = statp.tile([R, NB], f32)
        nc.vector.tensor_scalar(h_pre, hps2, 1.0 / TV, b1_sb,
                                op0=ALU.mult, op1=ALU.add)
        h_sb = statp.tile([R, NB], f32)
        nc.vector.tensor_scalar_max(h_sb, h_pre, 0.0)
        zps = psA.tile([C, NB], f32, name="zps", tag="a2")
        nc.tensor.matmul(zps, lhsT=w2t, rhs=h_sb, start=True, stop=True)
        eg = statp.tile([C, NB], f32)
        nc.scalar.activation(eg, zps, AF.Exp, bias=b2n, scale=-1.0)
        gp1 = statp.tile([C, NB], f32)
        nc.vector.tensor_scalar_add(gp1, eg, 1.0)
        gate = statp.tile([C, NB], f32)
        nc.vector.reciprocal(gate, gp1)

        remainder(1, pa1)
        nc.vector.tensor_scalar_mul(wts[0], p2[0], gate[:, 0:1])
        nc.vector.tensor_scalar_mul(wts[1], p2[1], gate[:, 1:2])

        # ---------------- BN stats -> allreduce -> global ----------------
        mv = statp.tile([C, 2], f32)
        nc.vector.bn_aggr(out=mv, in_=stats)
        m2 = statp.tile([C, 1], f32)
        nc.vector.tensor_mul(m2, mv[:, 0:1], mv[:, 0:1])
        ex2 = statp.tile([C, 1], f32)
        nc.vector.tensor_add(ex2, mv[:, 1:2], m2)
        sums = statp.tile([C, 2], f32)
        cnt_local = float(NB * TV)
        nc.vector.tensor_scalar_mul(sums[:, 0:1], mv[:, 0:1], cnt_local)
        nc.vector.tensor_scalar_mul(sums[:, 1:2], ex2, cnt_local)

        cc_in = dramp.tile([C, 2], f32, name="cc_in")
        cc_out = dramp.tile([C, 2], f32, name="cc_out", addr_space="Shared")
        nc.sync.dma_start(out=cc_in, in_=sums)
        nc.gpsimd.collective_compute(
            "AllReduce", ALU.add, ins=[cc_in.opt()], outs=[cc_out.opt()],
            replica_groups=[list(range(N_CORES))])
        gs = statp.tile([C, 2], f32)
        nc.sync.dma_start(out=gs, in_=cc_out)

        # mean/var -> sc, nsh (short chain)
        inv_cnt = 1.0 / (N * TV)
        mv2 = statp.tile([C, 2], f32)
        nc.vector.tensor_scalar_mul(mv2, gs, inv_cnt)
        nve = statp.tile([C, 1], f32)
        nc.vector.scalar_tensor_tensor(out=nve, in0=mv2[:, 0:1],
                                       scalar=mv2[:, 0:1], in1=mv2[:, 1:2],
                                       op0=ALU.mult, op1=ALU.subtract)
        ve = statp.tile([C, 1], f32)
        nc.vector.tensor_scalar(ve, nve, -1.0, EPS, op0=ALU.mult, op1=ALU.add)
        sq = statp.tile([C, 1], f32)
        nc.scalar.activation(sq, ve, AF.Sqrt)
        rstd = statp.tile([C, 1], f32)
        nc.vector.reciprocal(rstd, sq)
        sc = statp.tile([C, 1], f32)
        nc.vector.tensor_mul(sc, gamma_sb, rstd)
        nsh = statp.tile([C, 1], f32)
        nc.vector.scalar_tensor_tensor(out=nsh, in0=mv2[:, 0:1], scalar=sc,
                                       in1=beta_sb, op0=ALU.mult, op1=ALU.subtract)

        if debug:
            nc.sync.dma_start(out=dbg_eb, in_=eb[0])
            for _b in range(NB):
                nc.sync.dma_start(out=dbg_p2[_b], in_=p2[_b])
            nc.sync.dma_start(out=dbg_gate, in_=gate)
            nc.sync.dma_start(out=dbg_eba, in_=eba[0])

        # ------- finalize: out = (sc*(gate*p2) - gate*nsh) + x -----------------
        for b in range(NB):
            d_b = statp.tile([C, 1], f32, name=f"d_{b}")
            nc.vector.tensor_mul(d_b, gate[:, b:b + 1], nsh)
            for (h0, h1) in HALVES:
                ot = workp.tile([C, 800], f32, name="ot", tag="ot")
                nc.vector.tensor_scalar(ot, wts[b][:, h0:h1], sc, d_b,
                                        op0=ALU.mult, op1=ALU.subtract)
                osb = workp.tile([C, 800], f32, name="osb", tag="osb")
                nc.vector.tensor_tensor(out=osb, in0=ot, in1=xa32[b][:, h0:h1],
                                        op=ALU.add)
                nc.sync.dma_start(out=out_d[b][:, h0:h1], in_=osb)


_CACHE = {}


def _get_compiled(debug=False):
    key = ("nc", debug)
    if key in _CACHE:
        return _CACHE[key]
    import concourse.bacc as bacc

    nc = bacc.Bacc("TRN2", target_bir_lowering=False, debug=False,
                   enable_asserts=False, num_devices=N_CORES)
    _build(nc, debug=debug)
    nc.compile()
    _CACHE[key] = nc
    return nc


def _run(inputs, trace=False, debug=False, **kw):
    from concourse import bass_utils

    nc = _get_compiled(debug=debug)
    x = np.ascontiguousarray(np.asarray(inputs["x"], dtype=np.float32))
    x = x.reshape(N, C, TV)
    f = lambda a: np.ascontiguousarray(np.asarray(a, dtype=np.float32))
    common = {
        "wq": f(inputs["Wq"]),
        "wk": f(inputs["Wk"]),
        "bk": f(inputs["bk"]).reshape(IC, 1),
        "wv": f(inputs["Wv"]),
        "bv": f(inputs["bv"]).reshape(1, IC),
        "wt": f(inputs["Wt"]),
        "gamma": f(inputs["gamma"]).reshape(C, 1),
        "beta": f(inputs["beta"]).reshape(C, 1),
        "w1": f(inputs["W1"]),
        "b1": f(inputs["b1"]).reshape(C // 16, 1),
        "w2": f(inputs["W2"]),
        "b2": f(inputs["b2"]).reshape(C, 1),
    }
    in_maps = []
    for c in range(N_CORES):
        m = dict(common)
        m["x_in"] = np.ascontiguousarray(x[c * NB:(c + 1) * NB])
        in_maps.append(m)
    try:
        res = bass_utils.run_bass_kernel_spmd(
            nc, in_maps, core_ids=list(range(N_CORES)), trace=trace, **kw)
    except Exception:
        import time as _time
        _time.sleep(5)
        res = bass_utils.run_bass_kernel_spmd(
            nc, in_maps, core_ids=list(range(N_CORES)), trace=False, **kw)
    out = np.concatenate([res.results[c]["out"] for c in range(N_CORES)], axis=0)
    return out.reshape(N, C, T, V).astype(np.float32), res


def kernel(**inputs):
    return _run(inputs, trace=False)[0]


# revision 26
# speedup vs baseline: 1.5525x; 1.0550x over previous
"""Trainium2 Bass kernel for nn_FEM_35072702939287 (attention + BN + channel gate).

Math (validated in numpy vs reference):
  A^T[s,t] = X_s^T G^T X_t + rX[s] (+ t-only/const terms that drop under
  softmax over s), G = Wk^T Wq, rX = (Wq^T bk)^T X.  The rX term rides as
  a per-partition bias into exp (ACT bias operand / tensor_scalar scalar2),
  so the A matmul contracts over exactly K=64 -> two s-blocks run
  CONCURRENTLY on the PE via row tiling (tile_position (0,0) | (64,0)).
  V^T blocks [V | 1 | rX] come from one matmul per block; PV accumulates
  [V|1]^T exp(A^T) with even blocks on PSUM rows 0:33 and odd blocks on
  rows 64:97 via col tiling (tile_position (0,0) | (0,64)) -> concurrent.
  A combining matmul with lhsT = [Wt^T; 1-row; Wt^T; 1-row] reduces both
  partials and applies the Wt conv in one shot; /D folds in after, bt
  cancels under BN.  BN batch stats all-reduced across 8 cores; a dummy
  all-reduce at kernel start warms the CC engine and absorbs launch skew.
  exp is split across 3 engines: ScalarE exact exp; DVE+GpSimd compute a
  Schraudolph fast exp (i16 = rne(1477.32*(A+bias) + 15316), bitcast f16,
  max rel err ~3%; end-to-end <1e-2 validated vs reference).

Sharding: data-parallel over batch N=16 -> 2 batches per core x 8 cores.
"""

import numpy as np

N_CORES = 8
N, C, T, V = 16, 64, 64, 25
TV = T * V            # 1600
IC = 32
NB = N // N_CORES     # batches per core
EPS = 1e-5
NSB = 13              # 12 full 128-row s-blocks + one 64-row tail
SB = [(j * 128, 128) for j in range(12)] + [(1536, 64)]
PAIRS = [(0, 1), (2, 3), (4, 5), (6, 7), (8, 9), (10, 11), (12, None)]
HALVES = [(0, 800), (800, 1600)]
CH = [(0, 512), (512, 800)]       # psum-bank chunks inside an 800-half tile
# bank-aligned chunks for the 1600-wide PV accumulator (matmul output
# must not cross a 2KB PSUM bank boundary)
CH_PV = [(0, 512), (512, 1024), (1024, 1536), (1536, 1600)]
A2 = 1024.0 * 1.4426950408889634  # fast-exp scale
B2 = 15.0 * 1024.0 - 44.0         # fast-exp shift (rne-optimal C=-44)

# exp engine per tile: 0=ACT exact exp, 1=DVE fast exp (GPSIMD can't read
# PSUM, so it gets the SBUF-only elementwise work instead).
# kinds order per pair-slot: [(j,h0), (j,h1), (j2,h0), (j2,h1)]
KINDS_EVEN = [0, 1, 0, 1]   # 2 ACT / 2 DVE
KINDS_LIGHT = [0, 1, 0, 1]

ROW_TILE = True   # concurrent A-matmul pairs via PE row tiling
COL_TILE = True   # concurrent PV pairs via PE col tiling
N_WARM_MM = 22    # PE warmup matmuls (HAM)
CC_WARM = True    # dummy collective at start


def _build(nc, debug=False):
    import concourse.tile as tile
    from concourse import mybir
    from contextlib import ExitStack

    f32 = mybir.dt.float32
    f16 = mybir.dt.float16
    i16 = mybir.dt.int16
    AF = mybir.ActivationFunctionType
    ALU = mybir.AluOpType
    AX = mybir.AxisListType
    R = C // 16  # 4

    # ---------------- DRAM I/O ----------------
    x_in = nc.dram_tensor("x_in", [NB, C, TV], f32, kind="ExternalInput").ap()
    wq_d = nc.dram_tensor("wq", [IC, C], f32, kind="ExternalInput").ap()
    wk_d = nc.dram_tensor("wk", [IC, C], f32, kind="ExternalInput").ap()
    bk_d = nc.dram_tensor("bk", [IC, 1], f32, kind="ExternalInput").ap()
    wv_d = nc.dram_tensor("wv", [IC, C], f32, kind="ExternalInput").ap()
    bv_d = nc.dram_tensor("bv", [1, IC], f32, kind="ExternalInput").ap()
    wt_d = nc.dram_tensor("wt", [C, IC], f32, kind="ExternalInput").ap()
    gm_d = nc.dram_tensor("gamma", [C, 1], f32, kind="ExternalInput").ap()
    bt2_d = nc.dram_tensor("beta", [C, 1], f32, kind="ExternalInput").ap()
    w1_d = nc.dram_tensor("w1", [R, C], f32, kind="ExternalInput").ap()
    b1_d = nc.dram_tensor("b1", [R, 1], f32, kind="ExternalInput").ap()
    w2_d = nc.dram_tensor("w2", [C, R], f32, kind="ExternalInput").ap()
    b2_d = nc.dram_tensor("b2", [C, 1], f32, kind="ExternalInput").ap()
    out_d = nc.dram_tensor("out", [NB, C, TV], f32, kind="ExternalOutput").ap()
    if debug:
        dbg_eb = nc.dram_tensor("dbg_eb", [128, NSB, TV], f16, kind="ExternalOutput").ap()
        dbg_p2 = nc.dram_tensor("dbg_p2", [NB, C, TV], f32, kind="ExternalOutput").ap()
        dbg_gate = nc.dram_tensor("dbg_gate", [C, NB], f32, kind="ExternalOutput").ap()
        dbg_eba = nc.dram_tensor("dbg_eba", [128, NSB], f32, kind="ExternalOutput").ap()
        dbg_pd = nc.dram_tensor("dbg_pd", [128, TV], f16, kind="ExternalOutput").ap()

    with tile.TileContext(nc) as tc, ExitStack() as ctx:
        consts = ctx.enter_context(tc.tile_pool(name="consts", bufs=1))
        xpool = ctx.enter_context(tc.tile_pool(name="xpool", bufs=2))
        workp = ctx.enter_context(tc.tile_pool(name="workp", bufs=2))
        statp = ctx.enter_context(tc.tile_pool(name="statp", bufs=1))
        psA = ctx.enter_context(tc.tile_pool(name="psA", bufs=1, space="PSUM"))
        psP = ctx.enter_context(tc.tile_pool(name="psP", bufs=1, space="PSUM"))
        dramp = ctx.enter_context(tc.tile_pool(name="dramp", bufs=1, space="DRAM"))

        # ------------- warmup collective: absorbs CC cold start + launch skew
        if CC_WARM:
            ccw_in = dramp.tile([C, 2], f32, name="ccw_in")
            ccw_out = dramp.tile([C, 2], f32, name="ccw_out", addr_space="Shared")
            nc.gpsimd.collective_compute(
                "AllReduce", ALU.add, ins=[ccw_in.opt()], outs=[ccw_out.opt()],
                replica_groups=[list(range(N_CORES))])

        # ---------------- input DMAs first (sync queue) ------------------------
        xa32 = [None] * NB
        for b in range(NB):
            t = xpool.tile([C, TV], f32, name="xa32", tag="xa32")
            xa32[b] = t
            nc.sync.dma_start(out=t, in_=x_in[b])

        # ---------------- PE warmup (HAM): dummy matmuls -----------------------
        wuw = consts.tile([C, 128], f16)
        nc.vector.memset(wuw, 0.0)
        wur = consts.tile([C, 512], f16)
        nc.vector.memset(wur, 0.0)
        for i in range(N_WARM_MM):
            wups = psA.tile([128, 512], f32, name="wups", tag="a1")
            nc.tensor.matmul(wups, lhsT=wuw, rhs=wur, start=True, stop=True)

        # ---------------- ACT table warmup -------------------------------------
        warmz = consts.tile([1, 1], f32)
        nc.vector.memset(warmz, 1.0)
        warmo = consts.tile([1, 1], f32)
        nc.scalar.activation(warmo, warmz, AF.Exp)

        # ---------------- weights ----------------------------------------------
        wq_sb = consts.tile([IC, C], f32)
        nc.gpsimd.dma_start(out=wq_sb, in_=wq_d)
        wkbk = consts.tile([IC, C + 1], f32)
        nc.gpsimd.dma_start(out=wkbk[:, 0:C], in_=wk_d)
        nc.gpsimd.dma_start(out=wkbk[:, C:C + 1], in_=bk_d)

        # wvr [65, 36]: cols 0:32 = [Wv^T; bv], col 32 = ones-row marker,
        # col 33 = [r; 0], cols 34:36 pad
        wvr32 = consts.tile([C + 1, 36], f32)
        nc.vector.memset(wvr32, 0.0)
        nc.gpsimd.dma_start(out=wvr32[0:C, 0:IC], in_=wv_d.rearrange("i c -> c i"))
        nc.gpsimd.dma_start(out=wvr32[C:C + 1, 0:IC], in_=bv_d)
        nc.vector.memset(wvr32[C:C + 1, IC:IC + 1], 1.0)

        # G^T | r = Wq^T @ [Wk | bk]
        psg = psA.tile([C, C + 2], f32, name="psg", tag="a2")
        nc.tensor.matmul(psg[:, 0:C + 1], lhsT=wq_sb, rhs=wkbk, start=True, stop=True)
        gt16 = consts.tile([C, C], f16)
        nc.vector.tensor_copy(gt16, psg[:, 0:C])
        nc.vector.tensor_copy(wvr32[0:C, 33:34], psg[:, C:C + 1])
        wvr = consts.tile([C + 1, 36], f16)
        nc.vector.tensor_copy(wvr, wvr32)

        # wtc [128, 66]: rows 0:32 & 64:96 = Wt^T into cols 0:64; rows 32/96
        # put 1 in col 64 (D pass-through)
        wtT2 = consts.tile([128, C], f32)
        nc.gpsimd.dma_start(out=wtT2[0:IC, :], in_=wt_d.rearrange("c i -> i c"))
        nc.gpsimd.dma_start(out=wtT2[64:64 + IC, :], in_=wt_d.rearrange("c i -> i c"))
        wtc = consts.tile([128, 66], f32)
        nc.vector.memset(wtc, 0.0)
        nc.vector.tensor_copy(wtc[0:IC, 0:C], wtT2[0:IC, :])
        nc.vector.tensor_copy(wtc[64:64 + IC, 0:C], wtT2[64:64 + IC, :])
        nc.vector.memset(wtc[IC:IC + 1, 64:65], 1.0)
        nc.vector.memset(wtc[96:97, 64:65], 1.0)
        f32r = mybir.dt.float32r
        wtc_r = consts.tile([128, 66], f32r)
        nc.vector.tensor_copy(wtc_r, wtc)

        ones64 = consts.tile([C + 1, C], f16)
        nc.vector.memset(ones64, 0.0)
        nc.vector.memset(ones64[C:C + 1, :], 1.0)

        w1t = consts.tile([C, R], f32)
        nc.gpsimd.dma_start(out=w1t, in_=w1_d.rearrange("j c -> c j"))
        w2t = consts.tile([R, C], f32)
        nc.gpsimd.dma_start(out=w2t, in_=w2_d.rearrange("c j -> j c"))
        b1_sb = consts.tile([R, 1], f32)
        nc.gpsimd.dma_start(out=b1_sb, in_=b1_d)
        b2_sb = consts.tile([C, 1], f32)
        nc.gpsimd.dma_start(out=b2_sb, in_=b2_d)
        b2n = consts.tile([C, 1], f32)
        nc.vector.tensor_scalar_mul(b2n, b2_sb, -1.0)
        gamma_sb = consts.tile([C, 1], f32)
        nc.gpsimd.dma_start(out=gamma_sb, in_=gm_d)
        beta_sb = consts.tile([C, 1], f32)
        nc.gpsimd.dma_start(out=beta_sb, in_=bt2_d)

        # pd: combined PV partials (f16).  Rows 33:64 / 97:128 are never
        # written and multiply zero weight rows; zero them once so the f16
        # cast garbage can't inject NaN*0.
        pd = statp.tile([128, TV], f32r)
        nc.vector.memset(pd.bitcast(f32)[32:64, :], 0.0)
        nc.vector.memset(pd.bitcast(f32)[96:128, :], 0.0)

        # ---------------- per-batch state ----------------
        xa16 = [None] * NB    # [65, TV] f16 : [X; 1]
        xhi = [None] * NB     # [128, TV] f16 : rows 64:128 = X replica
        ha = [None] * NB      # [128, TV] f16 : G X replicated in both halves
        vt1 = [None] * NB     # [128, NSB, 33] f16 : [V | 1] per s-block
        eba = [None] * NB     # [128, NSB] f32 : rX bias per s-block
        ebb = [None] * NB     # [128, NSB] f32 : scaled fast-exp bias
        eb = [None] * NB      # [128, NSB, TV] f16 : exp(A^T)
        p2 = [None] * NB      # [64, TV] f32
        wts = [None] * NB     # [64, TV] f32 : gate * p2
        avgs = statp.tile([C, NB], f32)
        stats = statp.tile([C, NB * 4, 6], f32)

        def prologue(b):
            x16 = xpool.tile([C + 1, TV], f16, name="xa16", tag="xa16")
            xa16[b] = x16
            # cast + row-sum (for the gate) in one ACT pass
            nc.scalar.activation(x16[0:C, :], xa32[b], AF.Copy,
                                 accum_out=avgs[:, b:b + 1])
            nc.gpsimd.memset(x16[C:C + 1, :], 1.0)
            xh = xpool.tile([128, TV], f16, name="xhi", tag="xhi")
            xhi[b] = xh
            nc.sync.dma_start(out=xh[64:128, :], in_=x16[0:C, :])

            # V^T | 1 | rX for all 13 blocks into one psum bank
            vps = psA.tile([128, NSB, 36], f32, name="vps", tag="a1")
            for j, (off, p) in enumerate(SB):
                nc.tensor.matmul(vps[0:p, j, :], lhsT=x16[:, off:off + p],
                                 rhs=wvr, start=True, stop=True)
            v = xpool.tile([128, NSB, 33], f16, name="vt1", tag="vt1")
            vt1[b] = v
            nc.vector.tensor_copy(v, vps[:, :, 0:33])
            ea = xpool.tile([128, NSB], f32, name="eba", tag="eba")
            eba[b] = ea
            nc.vector.tensor_copy(ea, vps[:, :, 33])
            bbt = xpool.tile([128, NSB], f32, name="ebb", tag="ebb")
            ebb[b] = bbt
            nc.vector.tensor_scalar(bbt, ea, A2, B2, op0=ALU.mult, op1=ALU.add)

            # H = G X, replicated into partition halves via col tiling
            h = xpool.tile([128, TV], f16, name="ha", tag="ha")
            ha[b] = h
            for hi, (h0, h1) in enumerate(HALVES):
                hps = psA.tile([128, 800], f32, name="hps", tag="a2")
                for (c0, c1) in CH:
                    nc.tensor.matmul(hps[0:C, c0:c1], lhsT=gt16,
                                     rhs=x16[0:C, h0 + c0:h0 + c1],
                                     start=True, stop=True, tile_position=(0, 0))
                    nc.tensor.matmul(hps[64:128, c0:c1], lhsT=gt16,
                                     rhs=x16[0:C, h0 + c0:h0 + c1],
                                     start=True, stop=True, tile_position=(0, 64))
                nc.vector.tensor_copy(h[:, h0:h1], hps)

            eb[b] = xpool.tile([128, NSB, TV], f16, name="eb", tag="eb")
            p2[b] = xpool.tile([C, TV], f32, name="p2", tag="p2")
            wts[b] = xpool.tile([C, TV], f32, name="wts", tag="wts")

        def emit_exp(b, j, p, h0, aps, kind):
            dst = eb[b][0:p, j, h0:h0 + 800]
            if kind == 0:
                nc.scalar.activation(dst, aps[0:p, :], AF.Exp,
                                     bias=eba[b][0:p, j:j + 1])
            else:
                nc.vector.tensor_scalar(dst.bitcast(i16), aps[0:p, :], A2,
                                        ebb[b][0:p, j:j + 1],
                                        op0=ALU.mult, op1=ALU.add)

        def emit_pv(b, jj, hb, pacc):
            if jj is None:
                return
            j, j2 = jj
            p = SB[j][1]
            for (c0, c1) in CH:
                nc.tensor.matmul(pacc[0:33, c0:c1],
                                 lhsT=vt1[b][0:p, j, :],
                                 rhs=eb[b][0:p, j, hb + c0:hb + c1],
                                 start=(j == 0), stop=(j == 12),
                                 tile_position=(0, 0),
                                 skip_group_check=True)
                if j2 is not None:
                    nc.tensor.matmul(pacc[64:97, c0:c1],
                                     lhsT=vt1[b][:, j2, :],
                                     rhs=eb[b][:, j2, hb + c0:hb + c1],
                                     start=(j2 == 1), stop=(j2 == 11),
                                     tile_position=(0, 64),
                                     skip_group_check=True)

        def remainder_half(b, hi, pacc):
            hb = HALVES[hi][0]
            nc.vector.tensor_copy(pd[0:33, hb:hb + 800], pacc[0:33, :])
            nc.vector.tensor_copy(pd[64:97, hb:hb + 800], pacc[64:97, :])
            p2ps = psA.tile([66, 800], f32, name="p2ps", tag="a1")
            for (c0, c1) in CH:
                nc.tensor.matmul(p2ps[:, c0:c1], lhsT=wtc_r,
                                 rhs=pd[:, hb + c0:hb + c1],
                                 start=True, stop=True)
            rdt = workp.tile([C + 1, 800], f16, name="rdt", tag="rdt")
            nc.vector.tensor_copy(rdt[C:C + 1, :], p2ps[64:65, :])
            drep = psA.tile([C, 800], f32, name="drep", tag="a2")
            for (c0, c1) in CH:
                nc.tensor.matmul(drep[:, c0:c1], lhsT=ones64[C:C + 1, :],
                                 rhs=rdt[C:C + 1, c0:c1], start=True, stop=True)
            rrep = workp.tile([C, 800], f32, name="rrep", tag="rrep")
            nc.vector.reciprocal_approx_fast(out=rrep, in_=drep)
            nc.vector.tensor_tensor(out=p2[b][:, hb:hb + 800], in0=p2ps[0:C, :],
                                    in1=rrep, op=ALU.mult)
            nc.vector.bn_stats(stats[:, 4 * b + 2 * hi, :],
                               p2[b][:, hb:hb + 512])
            nc.vector.bn_stats(stats[:, 4 * b + 2 * hi + 1, :],
                               p2[b][:, hb + 512:hb + 800])

        def phase1(b):
            """t-halves outermost: 2-bank PV accumulator per half, three
            rotating A-psum tags so the PE never waits on the exp it just
            fed; pair chunks interleaved for row/col-tile concurrency."""
            for hi, (h0, h1) in enumerate(HALVES):
                pacc = psP.tile([128, 800], f32, name="pacc", tag="pacc")
                prev = None
                for si, (j, j2) in enumerate(PAIRS):
                    off, p = SB[j]
                    tagA = "a1" if si % 2 == 0 else "a3"
                    aA = psA.tile([128, 800], f32, name="apsA", tag=tagA)
                    aB = None
                    if j2 is not None:
                        off2, p2_ = SB[j2]
                        aB = psA.tile([128, 800], f32, name="apsB", tag="a2")
                    for (c0, c1) in CH:
                        nc.tensor.matmul(aA[0:p, c0:c1],
                                         lhsT=ha[b][0:C, off:off + p],
                                         rhs=xa16[b][0:C, h0 + c0:h0 + c1],
                                         start=True, stop=True,
                                         tile_position=(0, 0))
                        if aB is not None:
                            nc.tensor.matmul(aB[:, c0:c1],
                                             lhsT=ha[b][64:128, off2:off2 + p2_],
                                             rhs=xhi[b][64:128, h0 + c0:h0 + c1],
                                             start=True, stop=True,
                                             tile_position=(64, 0))
                    emit_pv(b, prev, h0, pacc)
                    ka, kb = (0, 1) if (si + hi) % 2 == 0 else (1, 0)
                    emit_exp(b, j, p, h0, aA, ka)
                    if aB is not None:
                        emit_exp(b, j2, p2_, h0, aB, kb)
                    prev = (j, j2)
                emit_pv(b, prev, h0, pacc)
                remainder_half(b, hi, pacc)

        # ---------------- main schedule ----------------
        prologue(0)
        phase1(0)
        prologue(1)
        phase1(1)

        # channel gate (overlaps phase1(1) tail)
        hps2 = psA.tile([R, NB], f32, name="hps2", tag="a2")
        nc.tensor.matmul(hps2, lhsT=w1t, rhs=avgs, start=True, stop=True)
        h_pre = statp.tile([R, NB], f32)
        nc.vector.tensor_scalar(h_pre, hps2, 1.0 / TV, b1_sb,
                                op0=ALU.mult, op1=ALU.add)
        h_sb = statp.tile([R, NB], f32)
        nc.vector.tensor_scalar_max(h_sb, h_pre, 0.0)
        zps = psA.tile([C, NB], f32, name="zps", tag="a2")
        nc.tensor.matmul(zps, lhsT=w2t, rhs=h_sb, start=True, stop=True)
        eg = statp.tile([C, NB], f32)
        nc.scalar.activation(eg, zps, AF.Exp, bias=b2n, scale=-1.0)
        gp1 = statp.tile([C, NB], f32)
        nc.vector.tensor_scalar_add(gp1, eg, 1.0)
        gate = statp.tile([C, NB], f32)
        nc.vector.reciprocal(gate, gp1)

        nc.vector.tensor_scalar_mul(wts[0], p2[0], gate[:, 0:1])
        nc.vector.tensor_scalar_mul(wts[1], p2[1], gate[:, 1:2])

        # ---------------- BN stats -> allreduce -> global ----------------
        mv = statp.tile([C, 2], f32)
        nc.vector.bn_aggr(out=mv, in_=stats)
        m2 = statp.tile([C, 1], f32)
        nc.vector.tensor_mul(m2, mv[:, 0:1], mv[:, 0:1])
        ex2 = statp.tile([C, 1], f32)
        nc.vector.tensor_add(ex2, mv[:, 1:2], m2)
        sums = statp.tile([C, 2], f32)
        cnt_local = float(NB * TV)
        nc.vector.tensor_scalar_mul(sums[:, 0:1], mv[:, 0:1], cnt_local)
        nc.vector.tensor_scalar_mul(sums[:, 1:2], ex2, cnt_local)

        cc_in = dramp.tile([C, 2], f32, name="cc_in")
        cc_out = dramp.tile([C, 2], f32, name="cc_out", addr_space="Shared")
        nc.sync.dma_start(out=cc_in, in_=sums)
        nc.gpsimd.collective_compute(
            "AllReduce", ALU.add, ins=[cc_in.opt()], outs=[cc_out.opt()],
            replica_groups=[list(range(N_CORES))])
        gs = statp.tile([C, 2], f32)
        nc.sync.dma_start(out=gs, in_=cc_out)

        # mean/var -> sc, nsh (short chain)
        inv_cnt = 1.0 / (N * TV)
        mv2 = statp.tile([C, 2], f32)
        nc.vector.tensor_scalar_mul(mv2, gs, inv_cnt)
        nve = statp.tile([C, 1], f32)
        nc.vector.scalar_tensor_tensor(out=nve, in0=mv2[:, 0:1],
                                       scalar=mv2[:, 0:1], in1=mv2[:, 1:2],
                                       op0=ALU.mult, op1=ALU.subtract)
        ve = statp.tile([C, 1], f32)
        nc.vector.tensor_scalar(ve, nve, -1.0, EPS, op0=ALU.mult, op1=ALU.add)
        sq = statp.tile([C, 1], f32)
        nc.scalar.activation(sq, ve, AF.Sqrt)
        rstd = statp.tile([C, 1], f32)
        nc.vector.reciprocal(rstd, sq)
        sc = statp.tile([C, 1], f32)
        nc.vector.tensor_mul(sc, gamma_sb, rstd)
        nsh = statp.tile([C, 1], f32)
        nc.vector.scalar_tensor_tensor(out=nsh, in0=mv2[:, 0:1], scalar=sc,
                                       in1=beta_sb, op0=ALU.mult, op1=ALU.subtract)

        if debug:
            nc.sync.dma_start(out=dbg_eb, in_=eb[0])
            for _b in range(NB):
                nc.sync.dma_start(out=dbg_p2[_b], in_=p2[_b])
            nc.sync.dma_start(out=dbg_gate, in_=gate)
            nc.sync.dma_start(out=dbg_eba, in_=eba[0])

        # ------- finalize: out = (sc*(gate*p2) - gate*nsh) + x -----------------
        for b in range(NB):
            d_b = statp.tile([C, 1], f32, name=f"d_{b}")
            nc.vector.tensor_mul(d_b, gate[:, b:b + 1], nsh)
            for (h0, h1) in HALVES:
                ot = workp.tile([C, 800], f32, name="ot", tag="ot")
                nc.vector.tensor_scalar(ot, wts[b][:, h0:h1], sc, d_b,
                                        op0=ALU.mult, op1=ALU.subtract)
                osb = workp.tile([C, 800], f32, name="osb", tag="osb")
                nc.vector.tensor_tensor(out=osb, in0=ot, in1=xa32[b][:, h0:h1],
                                        op=ALU.add)
                nc.sync.dma_start(out=out_d[b][:, h0:h1], in_=osb)


_CACHE = {}


def _get_compiled(debug=False):
    key = ("nc", debug)
    if key in _CACHE:
        return _CACHE[key]
    import concourse.bacc as bacc

    nc = bacc.Bacc("TRN2", target_bir_lowering=False, debug=False,
                   enable_asserts=False, num_devices=N_CORES)
    _build(nc, debug=debug)
    nc.compile()
    _CACHE[key] = nc
    return nc


def _run(inputs, trace=False, debug=False, **kw):
    from concourse import bass_utils

    nc = _get_compiled(debug=debug)
    x = np.ascontiguousarray(np.asarray(inputs["x"], dtype=np.float32))
    x = x.reshape(N, C, TV)
    f = lambda a: np.ascontiguousarray(np.asarray(a, dtype=np.float32))
    common = {
        "wq": f(inputs["Wq"]),
        "wk": f(inputs["Wk"]),
        "bk": f(inputs["bk"]).reshape(IC, 1),
        "wv": f(inputs["Wv"]),
        "bv": f(inputs["bv"]).reshape(1, IC),
        "wt": f(inputs["Wt"]),
        "gamma": f(inputs["gamma"]).reshape(C, 1),
        "beta": f(inputs["beta"]).reshape(C, 1),
        "w1": f(inputs["W1"]),
        "b1": f(inputs["b1"]).reshape(C // 16, 1),
        "w2": f(inputs["W2"]),
        "b2": f(inputs["b2"]).reshape(C, 1),
    }
    in_maps = []
    for c in range(N_CORES):
        m = dict(common)
        m["x_in"] = np.ascontiguousarray(x[c * NB:(c + 1) * NB])
        in_maps.append(m)
    try:
        res = bass_utils.run_bass_kernel_spmd(
            nc, in_maps, core_ids=list(range(N_CORES)), trace=trace, **kw)
    except Exception:
        import time as _time
        _time.sleep(5)
        res = bass_utils.run_bass_kernel_spmd(
            nc, in_maps, core_ids=list(range(N_CORES)), trace=False, **kw)
    out = np.concatenate([res.results[c]["out"] for c in range(N_CORES)], axis=0)
    return out.reshape(N, C, T, V).astype(np.float32), res


def kernel(**inputs):
    return _run(inputs, trace=False)[0]


# revision 27
# speedup vs baseline: 1.5786x; 1.0168x over previous
"""Trainium2 Bass kernel for nn_FEM_35072702939287 (attention + BN + channel gate).

Math (validated in numpy vs reference):
  A^T[s,t] = X_s^T G^T X_t + rX[s] (+ t-only/const terms that drop under
  softmax over s), G = Wk^T Wq, rX = (Wq^T bk)^T X.  The rX term rides as
  a per-partition bias into exp (ACT bias operand / tensor_scalar scalar2),
  so the A matmul contracts over exactly K=64 -> two s-blocks run
  CONCURRENTLY on the PE via row tiling (tile_position (0,0) | (64,0)).
  V^T blocks [V | 1 | rX] come from one matmul per block; PV accumulates
  [V|1]^T exp(A^T) with even blocks on PSUM rows 0:33 and odd blocks on
  rows 64:97 via col tiling (tile_position (0,0) | (0,64)) -> concurrent.
  A combining matmul with lhsT = [Wt^T; 1-row; Wt^T; 1-row] reduces both
  partials and applies the Wt conv in one shot; /D folds in after, bt
  cancels under BN.  BN batch stats all-reduced across 8 cores; a dummy
  all-reduce at kernel start warms the CC engine and absorbs launch skew.
  exp is split across 3 engines: ScalarE exact exp; DVE+GpSimd compute a
  Schraudolph fast exp (i16 = rne(1477.32*(A+bias) + 15316), bitcast f16,
  max rel err ~3%; end-to-end <1e-2 validated vs reference).

Sharding: data-parallel over batch N=16 -> 2 batches per core x 8 cores.
"""

import numpy as np

N_CORES = 8
N, C, T, V = 16, 64, 64, 25
TV = T * V            # 1600
IC = 32
NB = N // N_CORES     # batches per core
EPS = 1e-5
NSB = 13              # 12 full 128-row s-blocks + one 64-row tail
SB = [(j * 128, 128) for j in range(12)] + [(1536, 64)]
PAIRS = [(0, 1), (2, 3), (4, 5), (6, 7), (8, 9), (10, 11), (12, None)]
HALVES = [(0, 800), (800, 1600)]
CH = [(0, 512), (512, 800)]       # psum-bank chunks inside an 800-half tile
# bank-aligned chunks for the 1600-wide PV accumulator (matmul output
# must not cross a 2KB PSUM bank boundary)
CH_PV = [(0, 512), (512, 1024), (1024, 1536), (1536, 1600)]
A2 = 1024.0 * 1.4426950408889634  # fast-exp scale
B2 = 15.0 * 1024.0 - 44.0         # fast-exp shift (rne-optimal C=-44)

# exp engine per tile: 0=ACT exact exp, 1=DVE fast exp (GPSIMD can't read
# PSUM, so it gets the SBUF-only elementwise work instead).
# kinds order per pair-slot: [(j,h0), (j,h1), (j2,h0), (j2,h1)]
KINDS_EVEN = [0, 1, 0, 1]   # 2 ACT / 2 DVE
KINDS_LIGHT = [0, 1, 0, 1]

ROW_TILE = True   # concurrent A-matmul pairs via PE row tiling
COL_TILE = True   # concurrent PV pairs via PE col tiling
N_WARM_MM = 22    # PE warmup matmuls (HAM)
CC_WARM = True    # dummy collective at start


def _build(nc, debug=False):
    import concourse.tile as tile
    from concourse import mybir
    from contextlib import ExitStack

    f32 = mybir.dt.float32
    f16 = mybir.dt.float16
    i16 = mybir.dt.int16
    AF = mybir.ActivationFunctionType
    ALU = mybir.AluOpType
    AX = mybir.AxisListType
    R = C // 16  # 4

    # ---------------- DRAM I/O ----------------
    x_in = nc.dram_tensor("x_in", [NB, C, TV], f32, kind="ExternalInput").ap()
    wq_d = nc.dram_tensor("wq", [IC, C], f32, kind="ExternalInput").ap()
    wk_d = nc.dram_tensor("wk", [IC, C], f32, kind="ExternalInput").ap()
    bk_d = nc.dram_tensor("bk", [IC, 1], f32, kind="ExternalInput").ap()
    wv_d = nc.dram_tensor("wv", [IC, C], f32, kind="ExternalInput").ap()
    bv_d = nc.dram_tensor("bv", [1, IC], f32, kind="ExternalInput").ap()
    wt_d = nc.dram_tensor("wt", [C, IC], f32, kind="ExternalInput").ap()
    gm_d = nc.dram_tensor("gamma", [C, 1], f32, kind="ExternalInput").ap()
    bt2_d = nc.dram_tensor("beta", [C, 1], f32, kind="ExternalInput").ap()
    w1_d = nc.dram_tensor("w1", [R, C], f32, kind="ExternalInput").ap()
    b1_d = nc.dram_tensor("b1", [R, 1], f32, kind="ExternalInput").ap()
    w2_d = nc.dram_tensor("w2", [C, R], f32, kind="ExternalInput").ap()
    b2_d = nc.dram_tensor("b2", [C, 1], f32, kind="ExternalInput").ap()
    out_d = nc.dram_tensor("out", [NB, C, TV], f32, kind="ExternalOutput").ap()
    if debug:
        dbg_eb = nc.dram_tensor("dbg_eb", [128, NSB, TV], f16, kind="ExternalOutput").ap()
        dbg_p2 = nc.dram_tensor("dbg_p2", [NB, C, TV], f32, kind="ExternalOutput").ap()
        dbg_gate = nc.dram_tensor("dbg_gate", [C, NB], f32, kind="ExternalOutput").ap()
        dbg_eba = nc.dram_tensor("dbg_eba", [128, NSB], f32, kind="ExternalOutput").ap()
        dbg_pd = nc.dram_tensor("dbg_pd", [128, TV], f16, kind="ExternalOutput").ap()

    with tile.TileContext(nc) as tc, ExitStack() as ctx:
        consts = ctx.enter_context(tc.tile_pool(name="consts", bufs=1))
        xpool = ctx.enter_context(tc.tile_pool(name="xpool", bufs=2))
        workp = ctx.enter_context(tc.tile_pool(name="workp", bufs=2))
        statp = ctx.enter_context(tc.tile_pool(name="statp", bufs=1))
        psA = ctx.enter_context(tc.tile_pool(name="psA", bufs=1, space="PSUM"))
        psP = ctx.enter_context(tc.tile_pool(name="psP", bufs=1, space="PSUM"))
        dramp = ctx.enter_context(tc.tile_pool(name="dramp", bufs=1, space="DRAM"))

        # ------------- warmup collective: absorbs CC cold start + launch skew
        if CC_WARM:
            ccw_in = dramp.tile([C, 2], f32, name="ccw_in")
            ccw_out = dramp.tile([C, 2], f32, name="ccw_out", addr_space="Shared")
            nc.gpsimd.collective_compute(
                "AllReduce", ALU.add, ins=[ccw_in.opt()], outs=[ccw_out.opt()],
                replica_groups=[list(range(N_CORES))])

        # ---------------- input DMAs first (sync queue) ------------------------
        xa32 = [None] * NB
        for b in range(NB):
            t = xpool.tile([C, TV], f32, name="xa32", tag="xa32")
            xa32[b] = t
            nc.sync.dma_start(out=t, in_=x_in[b])

        # ---------------- PE warmup (HAM): dummy matmuls -----------------------
        wuw = consts.tile([C, 128], f16)
        nc.vector.memset(wuw, 0.0)
        wur = consts.tile([C, 512], f16)
        nc.vector.memset(wur, 0.0)
        for i in range(N_WARM_MM):
            wups = psA.tile([128, 512], f32, name="wups", tag="a1")
            nc.tensor.matmul(wups, lhsT=wuw, rhs=wur, start=True, stop=True)

        # ---------------- ACT table warmup -------------------------------------
        warmz = consts.tile([1, 1], f32)
        nc.vector.memset(warmz, 1.0)
        warmo = consts.tile([1, 1], f32)
        nc.scalar.activation(warmo, warmz, AF.Exp)

        # ---------------- weights ----------------------------------------------
        wq_sb = consts.tile([IC, C], f32)
        nc.gpsimd.dma_start(out=wq_sb, in_=wq_d)
        wkbk = consts.tile([IC, C + 1], f32)
        nc.gpsimd.dma_start(out=wkbk[:, 0:C], in_=wk_d)
        nc.gpsimd.dma_start(out=wkbk[:, C:C + 1], in_=bk_d)

        # wvr [65, 36]: cols 0:32 = [Wv^T; bv], col 32 = ones-row marker,
        # col 33 = [r; 0], cols 34:36 pad
        wvr32 = consts.tile([C + 1, 36], f32)
        nc.vector.memset(wvr32, 0.0)
        nc.gpsimd.dma_start(out=wvr32[0:C, 0:IC], in_=wv_d.rearrange("i c -> c i"))
        nc.gpsimd.dma_start(out=wvr32[C:C + 1, 0:IC], in_=bv_d)
        nc.vector.memset(wvr32[C:C + 1, IC:IC + 1], 1.0)

        # G^T | r = Wq^T @ [Wk | bk]
        psg = psA.tile([C, C + 2], f32, name="psg", tag="a2")
        nc.tensor.matmul(psg[:, 0:C + 1], lhsT=wq_sb, rhs=wkbk, start=True, stop=True)
        gt16 = consts.tile([C, C], f16)
        nc.vector.tensor_copy(gt16, psg[:, 0:C])
        nc.vector.tensor_copy(wvr32[0:C, 33:34], psg[:, C:C + 1])
        wvr = consts.tile([C + 1, 36], f16)
        nc.vector.tensor_copy(wvr, wvr32)

        # wtc [128, 66]: rows 0:32 & 64:96 = Wt^T into cols 0:64; rows 32/96
        # put 1 in col 64 (D pass-through)
        wtT2 = consts.tile([128, C], f32)
        nc.gpsimd.dma_start(out=wtT2[0:IC, :], in_=wt_d.rearrange("c i -> i c"))
        nc.gpsimd.dma_start(out=wtT2[64:64 + IC, :], in_=wt_d.rearrange("c i -> i c"))
        wtc = consts.tile([128, 66], f32)
        nc.vector.memset(wtc, 0.0)
        nc.vector.tensor_copy(wtc[0:IC, 0:C], wtT2[0:IC, :])
        nc.vector.tensor_copy(wtc[64:64 + IC, 0:C], wtT2[64:64 + IC, :])
        nc.vector.memset(wtc[IC:IC + 1, 64:65], 1.0)
        nc.vector.memset(wtc[96:97, 64:65], 1.0)
        f32r = mybir.dt.float32r
        wtc_r = consts.tile([128, 66], f32r)
        nc.vector.tensor_copy(wtc_r, wtc)

        ones64 = consts.tile([C + 1, C], f16)
        nc.vector.memset(ones64, 0.0)
        nc.vector.memset(ones64[C:C + 1, :], 1.0)

        w1t = consts.tile([C, R], f32)
        nc.gpsimd.dma_start(out=w1t, in_=w1_d.rearrange("j c -> c j"))
        w2t = consts.tile([R, C], f32)
        nc.gpsimd.dma_start(out=w2t, in_=w2_d.rearrange("c j -> j c"))
        b1_sb = consts.tile([R, 1], f32)
        nc.gpsimd.dma_start(out=b1_sb, in_=b1_d)
        b2_sb = consts.tile([C, 1], f32)
        nc.gpsimd.dma_start(out=b2_sb, in_=b2_d)
        b2n = consts.tile([C, 1], f32)
        nc.vector.tensor_scalar_mul(b2n, b2_sb, -1.0)
        gamma_sb = consts.tile([C, 1], f32)
        nc.gpsimd.dma_start(out=gamma_sb, in_=gm_d)
        beta_sb = consts.tile([C, 1], f32)
        nc.gpsimd.dma_start(out=beta_sb, in_=bt2_d)

        # pd: combined PV partials (f16).  Rows 33:64 / 97:128 are never
        # written and multiply zero weight rows; zero them once so the f16
        # cast garbage can't inject NaN*0.
        pd = statp.tile([128, TV], f32r)
        nc.vector.memset(pd.bitcast(f32)[32:64, :], 0.0)
        nc.vector.memset(pd.bitcast(f32)[96:128, :], 0.0)

        # ---------------- per-batch state ----------------
        xa16 = [None] * NB    # [65, TV] f16 : [X; 1]
        xhi = [None] * NB     # [128, TV] f16 : rows 64:128 = X replica
        ha = [None] * NB      # [128, TV] f16 : G X replicated in both halves
        vt1 = [None] * NB     # [128, NSB, 33] f16 : [V | 1] per s-block
        eba = [None] * NB     # [128, NSB] f32 : rX bias per s-block
        ebb = [None] * NB     # [128, NSB] f32 : scaled fast-exp bias
        eb = [None] * NB      # [128, NSB, TV] f16 : exp(A^T)
        p2 = [None] * NB      # [64, TV] f32
        wts = [None] * NB     # [64, TV] f32 : gate * p2
        avgs = statp.tile([C, NB], f32)
        stats = statp.tile([C, NB * 4, 6], f32)

        def prologue(b):
            x16 = xpool.tile([C + 1, TV], f16, name="xa16", tag="xa16")
            xa16[b] = x16
            # cast + row-sum (for the gate) in one ACT pass
            nc.scalar.activation(x16[0:C, :], xa32[b], AF.Copy,
                                 accum_out=avgs[:, b:b + 1])
            nc.vector.memset(x16[C:C + 1, :], 1.0)
            xh = xpool.tile([128, TV], f16, name="xhi", tag="xhi")
            xhi[b] = xh
            nc.sync.dma_start(out=xh[64:128, :], in_=x16[0:C, :])

            # V^T | 1 | rX for all 13 blocks into one psum bank
            vps = psA.tile([128, NSB, 36], f32, name="vps", tag="a1")
            for j, (off, p) in enumerate(SB):
                nc.tensor.matmul(vps[0:p, j, :], lhsT=x16[:, off:off + p],
                                 rhs=wvr, start=True, stop=True)
            v = xpool.tile([128, NSB, 33], f16, name="vt1", tag="vt1")
            vt1[b] = v
            nc.vector.tensor_copy(v, vps[:, :, 0:33])
            ea = xpool.tile([128, NSB], f32, name="eba", tag="eba")
            eba[b] = ea
            nc.vector.tensor_copy(ea, vps[:, :, 33])
            bbt = xpool.tile([128, NSB], f32, name="ebb", tag="ebb")
            ebb[b] = bbt
            nc.vector.tensor_scalar(bbt, ea, A2, B2, op0=ALU.mult, op1=ALU.add)

            # H = G X, replicated into partition halves via col tiling
            h = xpool.tile([128, TV], f16, name="ha", tag="ha")
            ha[b] = h
            for hi, (h0, h1) in enumerate(HALVES):
                hps = psA.tile([128, 800], f32, name="hps", tag="a2")
                for (c0, c1) in CH:
                    nc.tensor.matmul(hps[0:C, c0:c1], lhsT=gt16,
                                     rhs=x16[0:C, h0 + c0:h0 + c1],
                                     start=True, stop=True, tile_position=(0, 0))
                    nc.tensor.matmul(hps[64:128, c0:c1], lhsT=gt16,
                                     rhs=x16[0:C, h0 + c0:h0 + c1],
                                     start=True, stop=True, tile_position=(0, 64))
                nc.vector.tensor_copy(h[:, h0:h1], hps)

            eb[b] = xpool.tile([128, NSB, TV], f16, name="eb", tag="eb")
            p2[b] = xpool.tile([C, TV], f32, name="p2", tag="p2")
            wts[b] = xpool.tile([C, TV], f32, name="wts", tag="wts")

        def emit_exp(b, j, p, h0, aps, kind):
            dst = eb[b][0:p, j, h0:h0 + 800]
            if kind == 0:
                nc.scalar.activation(dst, aps[0:p, :], AF.Exp,
                                     bias=eba[b][0:p, j:j + 1])
            else:
                nc.vector.tensor_scalar(dst.bitcast(i16), aps[0:p, :], A2,
                                        ebb[b][0:p, j:j + 1],
                                        op0=ALU.mult, op1=ALU.add)

        def emit_pv(b, jj, hb, pacc):
            if jj is None:
                return
            j, j2 = jj
            p = SB[j][1]
            for (c0, c1) in CH:
                nc.tensor.matmul(pacc[0:33, c0:c1],
                                 lhsT=vt1[b][0:p, j, :],
                                 rhs=eb[b][0:p, j, hb + c0:hb + c1],
                                 start=(j == 0), stop=(j == 12),
                                 tile_position=(0, 0),
                                 skip_group_check=True)
                if j2 is not None:
                    nc.tensor.matmul(pacc[64:97, c0:c1],
                                     lhsT=vt1[b][:, j2, :],
                                     rhs=eb[b][:, j2, hb + c0:hb + c1],
                                     start=(j2 == 1), stop=(j2 == 11),
                                     tile_position=(0, 64),
                                     skip_group_check=True)

        def remainder_half(b, hi, pacc):
            hb = HALVES[hi][0]
            nc.vector.tensor_copy(pd[0:33, hb:hb + 800], pacc[0:33, :])
            nc.vector.tensor_copy(pd[64:97, hb:hb + 800], pacc[64:97, :])
            p2ps = psA.tile([66, 800], f32, name="p2ps", tag="a1")
            for (c0, c1) in CH:
                nc.tensor.matmul(p2ps[:, c0:c1], lhsT=wtc_r,
                                 rhs=pd[:, hb + c0:hb + c1],
                                 start=True, stop=True)
            rdt = workp.tile([C + 1, 800], f16, name="rdt", tag="rdt")
            nc.vector.tensor_copy(rdt[C:C + 1, :], p2ps[64:65, :])
            drep = psA.tile([C, 800], f32, name="drep", tag="a2")
            for (c0, c1) in CH:
                nc.tensor.matmul(drep[:, c0:c1], lhsT=ones64[C:C + 1, :],
                                 rhs=rdt[C:C + 1, c0:c1], start=True, stop=True)
            rrep = workp.tile([C, 800], f32, name="rrep", tag="rrep")
            nc.vector.reciprocal_approx_fast(out=rrep, in_=drep)
            nc.vector.tensor_tensor(out=p2[b][:, hb:hb + 800], in0=p2ps[0:C, :],
                                    in1=rrep, op=ALU.mult)
            nc.vector.bn_stats(stats[:, 4 * b + 2 * hi, :],
                               p2[b][:, hb:hb + 512])
            nc.vector.bn_stats(stats[:, 4 * b + 2 * hi + 1, :],
                               p2[b][:, hb + 512:hb + 800])

        def phase1(b):
            """t-halves outermost: 2-bank PV accumulator per half, three
            rotating A-psum tags so the PE never waits on the exp it just
            fed; pair chunks interleaved for row/col-tile concurrency."""
            for hi, (h0, h1) in enumerate(HALVES):
                pacc = psP.tile([128, 800], f32, name="pacc", tag="pacc")
                prev = None
                for si, (j, j2) in enumerate(PAIRS):
                    off, p = SB[j]
                    tagA = "a1" if si % 2 == 0 else "a3"
                    aA = psA.tile([128, 800], f32, name="apsA", tag=tagA)
                    aB = None
                    if j2 is not None:
                        off2, p2_ = SB[j2]
                        aB = psA.tile([128, 800], f32, name="apsB", tag="a2")
                    for (c0, c1) in CH:
                        nc.tensor.matmul(aA[0:p, c0:c1],
                                         lhsT=ha[b][0:C, off:off + p],
                                         rhs=xa16[b][0:C, h0 + c0:h0 + c1],
                                         start=True, stop=True,
                                         tile_position=(0, 0))
                        if aB is not None:
                            nc.tensor.matmul(aB[:, c0:c1],
                                             lhsT=ha[b][64:128, off2:off2 + p2_],
                                             rhs=xhi[b][64:128, h0 + c0:h0 + c1],
                                             start=True, stop=True,
                                             tile_position=(64, 0))
                    emit_pv(b, prev, h0, pacc)
                    ka, kb = (0, 1) if (si + hi) % 2 == 0 else (1, 0)
                    emit_exp(b, j, p, h0, aA, ka)
                    if aB is not None:
                        emit_exp(b, j2, p2_, h0, aB, kb)
                    prev = (j, j2)
                emit_pv(b, prev, h0, pacc)
                remainder_half(b, hi, pacc)

        # ---------------- main schedule ----------------
        prologue(0)
        phase1(0)
        prologue(1)
        phase1(1)

        # channel gate (overlaps phase1(1) tail)
        hps2 = psA.tile([R, NB], f32, name="hps2", tag="a2")
        nc.tensor.matmul(hps2, lhsT=w1t, rhs=avgs, start=True, stop=True)
        h_pre = statp.tile([R, NB], f32)
        nc.vector.tensor_scalar(h_pre, hps2, 1.0 / TV, b1_sb,
                                op0=ALU.mult, op1=ALU.add)
        h_sb = statp.tile([R, NB], f32)
        nc.vector.tensor_scalar_max(h_sb, h_pre, 0.0)
        zps = psA.tile([C, NB], f32, name="zps", tag="a2")
        nc.tensor.matmul(zps, lhsT=w2t, rhs=h_sb, start=True, stop=True)
        eg = statp.tile([C, NB], f32)
        nc.scalar.activation(eg, zps, AF.Exp, bias=b2n, scale=-1.0)
        gp1 = statp.tile([C, NB], f32)
        nc.vector.tensor_scalar_add(gp1, eg, 1.0)
        gate = statp.tile([C, NB], f32)
        nc.vector.reciprocal(gate, gp1)

        nc.vector.tensor_scalar_mul(wts[0], p2[0], gate[:, 0:1])
        nc.vector.tensor_scalar_mul(wts[1], p2[1], gate[:, 1:2])

        # ---------------- BN stats -> allreduce -> global ----------------
        mv = statp.tile([C, 2], f32)
        nc.vector.bn_aggr(out=mv, in_=stats)
        m2 = statp.tile([C, 1], f32)
        nc.vector.tensor_mul(m2, mv[:, 0:1], mv[:, 0:1])
        ex2 = statp.tile([C, 1], f32)
        nc.vector.tensor_add(ex2, mv[:, 1:2], m2)
        sums = statp.tile([C, 2], f32)
        cnt_local = float(NB * TV)
        nc.vector.tensor_scalar_mul(sums[:, 0:1], mv[:, 0:1], cnt_local)
        nc.vector.tensor_scalar_mul(sums[:, 1:2], ex2, cnt_local)

        cc_in = dramp.tile([C, 2], f32, name="cc_in")
        cc_out = dramp.tile([C, 2], f32, name="cc_out", addr_space="Shared")
        nc.sync.dma_start(out=cc_in, in_=sums)
        nc.gpsimd.collective_compute(
            "AllReduce", ALU.add, ins=[cc_in.opt()], outs=[cc_out.opt()],
            replica_groups=[list(range(N_CORES))])
        gs = statp.tile([C, 2], f32)
        nc.sync.dma_start(out=gs, in_=cc_out)

        # mean/var -> sc, nsh (short chain)
        inv_cnt = 1.0 / (N * TV)
        mv2 = statp.tile([C, 2], f32)
        nc.vector.tensor_scalar_mul(mv2, gs, inv_cnt)
        nve = statp.tile([C, 1], f32)
        nc.vector.scalar_tensor_tensor(out=nve, in0=mv2[:, 0:1],
                                       scalar=mv2[:, 0:1], in1=mv2[:, 1:2],
                                       op0=ALU.mult, op1=ALU.subtract)
        ve = statp.tile([C, 1], f32)
        nc.vector.tensor_scalar(ve, nve, -1.0, EPS, op0=ALU.mult, op1=ALU.add)
        sq = statp.tile([C, 1], f32)
        nc.scalar.activation(sq, ve, AF.Sqrt)
        rstd = statp.tile([C, 1], f32)
        nc.vector.reciprocal(rstd, sq)
        sc = statp.tile([C, 1], f32)
        nc.vector.tensor_mul(sc, gamma_sb, rstd)
        nsh = statp.tile([C, 1], f32)
        nc.vector.scalar_tensor_tensor(out=nsh, in0=mv2[:, 0:1], scalar=sc,
                                       in1=beta_sb, op0=ALU.mult, op1=ALU.subtract)

        if debug:
            nc.sync.dma_start(out=dbg_eb, in_=eb[0])
            for _b in range(NB):
                nc.sync.dma_start(out=dbg_p2[_b], in_=p2[_b])
            nc.sync.dma_start(out=dbg_gate, in_=gate)
            nc.sync.dma_start(out=dbg_eba, in_=eba[0])

        # ------- finalize: out = (sc*(gate*p2) - gate*nsh) + x -----------------
        for b in range(NB):
            d_b = statp.tile([C, 1], f32, name=f"d_{b}")
            nc.vector.tensor_mul(d_b, gate[:, b:b + 1], nsh)
            osb = workp.tile([C, TV], f32, name="osb", tag="osb")
            for (h0, h1) in HALVES:
                ot = workp.tile([C, 800], f32, name="ot", tag="ot")
                nc.vector.tensor_scalar(ot, wts[b][:, h0:h1], sc, d_b,
                                        op0=ALU.mult, op1=ALU.subtract)
                nc.vector.tensor_tensor(out=osb[:, h0:h1], in0=ot,
                                        in1=xa32[b][:, h0:h1], op=ALU.add)
            nc.sync.dma_start(out=out_d[b], in_=osb)


_CACHE = {}


def _get_compiled(debug=False):
    key = ("nc", debug)
    if key in _CACHE:
        return _CACHE[key]
    import concourse.bacc as bacc

    nc = bacc.Bacc("TRN2", target_bir_lowering=False, debug=False,
                   enable_asserts=False, num_devices=N_CORES)
    _build(nc, debug=debug)
    nc.compile()
    _CACHE[key] = nc
    return nc


def _run(inputs, trace=False, debug=False, **kw):
    from concourse import bass_utils

    nc = _get_compiled(debug=debug)
    x = np.ascontiguousarray(np.asarray(inputs["x"], dtype=np.float32))
    x = x.reshape(N, C, TV)
    f = lambda a: np.ascontiguousarray(np.asarray(a, dtype=np.float32))
    common = {
        "wq": f(inputs["Wq"]),
        "wk": f(inputs["Wk"]),
        "bk": f(inputs["bk"]).reshape(IC, 1),
        "wv": f(inputs["Wv"]),
        "bv": f(inputs["bv"]).reshape(1, IC),
        "wt": f(inputs["Wt"]),
        "gamma": f(inputs["gamma"]).reshape(C, 1),
        "beta": f(inputs["beta"]).reshape(C, 1),
        "w1": f(inputs["W1"]),
        "b1": f(inputs["b1"]).reshape(C // 16, 1),
        "w2": f(inputs["W2"]),
        "b2": f(inputs["b2"]).reshape(C, 1),
    }
    in_maps = []
    for c in range(N_CORES):
        m = dict(common)
        m["x_in"] = np.ascontiguousarray(x[c * NB:(c + 1) * NB])
        in_maps.append(m)
    try:
        res = bass_utils.run_bass_kernel_spmd(
            nc, in_maps, core_ids=list(range(N_CORES)), trace=trace, **kw)
    except Exception:
        import time as _time
        _time.sleep(5)
        res = bass_utils.run_bass_kernel_spmd(
            nc, in_maps, core_ids=list(range(N_CORES)), trace=False, **kw)
    out = np.concatenate([res.results[c]["out"] for c in range(N_CORES)], axis=0)
    return out.reshape(N, C, T, V).astype(np.float32), res


def kernel(**inputs):
    return _run(inputs, trace=False)[0]


# revision 28
# speedup vs baseline: 1.6072x; 1.0182x over previous
"""Trainium2 Bass kernel for nn_FEM_35072702939287 (attention + BN + channel gate).

Math (validated in numpy vs reference):
  A^T[s,t] = X_s^T G^T X_t + rX[s] (+ t-only/const terms that drop under
  softmax over s), G = Wk^T Wq, rX = (Wq^T bk)^T X.  The rX term rides as
  a per-partition bias into exp (ACT bias operand / tensor_scalar scalar2),
  so the A matmul contracts over exactly K=64 -> two s-blocks run
  CONCURRENTLY on the PE via row tiling (tile_position (0,0) | (64,0)).
  V^T blocks [V | 1 | rX] come from one matmul per block; PV accumulates
  [V|1]^T exp(A^T) with even blocks on PSUM rows 0:33 and odd blocks on
  rows 64:97 via col tiling (tile_position (0,0) | (0,64)) -> concurrent.
  A combining matmul with lhsT = [Wt^T; 1-row; Wt^T; 1-row] reduces both
  partials and applies the Wt conv in one shot; /D folds in after, bt
  cancels under BN.  BN batch stats all-reduced across 8 cores; a dummy
  all-reduce at kernel start warms the CC engine and absorbs launch skew.
  exp is split across 3 engines: ScalarE exact exp; DVE+GpSimd compute a
  Schraudolph fast exp (i16 = rne(1477.32*(A+bias) + 15316), bitcast f16,
  max rel err ~3%; end-to-end <1e-2 validated vs reference).

Sharding: data-parallel over batch N=16 -> 2 batches per core x 8 cores.
"""

import numpy as np

N_CORES = 8
N, C, T, V = 16, 64, 64, 25
TV = T * V            # 1600
IC = 32
NB = N // N_CORES     # batches per core
EPS = 1e-5
NSB = 13              # 12 full 128-row s-blocks + one 64-row tail
SB = [(j * 128, 128) for j in range(12)] + [(1536, 64)]
PAIRS = [(0, 1), (2, 3), (4, 5), (6, 7), (8, 9), (10, 11), (12, None)]
HALVES = [(0, 800), (800, 1600)]
CH = [(0, 512), (512, 800)]       # psum-bank chunks inside an 800-half tile
# bank-aligned chunks for the 1600-wide PV accumulator (matmul output
# must not cross a 2KB PSUM bank boundary)
CH_PV = [(0, 512), (512, 1024), (1024, 1536), (1536, 1600)]
A2 = 1024.0 * 1.4426950408889634  # fast-exp scale
B2 = 15.0 * 1024.0 - 44.0         # fast-exp shift (rne-optimal C=-44)

# exp engine per tile: 0=ACT exact exp, 1=DVE fast exp (GPSIMD can't read
# PSUM, so it gets the SBUF-only elementwise work instead).
# kinds order per pair-slot: [(j,h0), (j,h1), (j2,h0), (j2,h1)]
KINDS_EVEN = [0, 1, 0, 1]   # 2 ACT / 2 DVE
KINDS_LIGHT = [0, 1, 0, 1]

ROW_TILE = True   # concurrent A-matmul pairs via PE row tiling
COL_TILE = True   # concurrent PV pairs via PE col tiling
N_WARM_MM = 22    # PE warmup matmuls (HAM)
CC_WARM = True    # dummy collective at start


def _build(nc, debug=False):
    import concourse.tile as tile
    from concourse import mybir
    from contextlib import ExitStack

    f32 = mybir.dt.float32
    f16 = mybir.dt.float16
    i16 = mybir.dt.int16
    AF = mybir.ActivationFunctionType
    ALU = mybir.AluOpType
    AX = mybir.AxisListType
    R = C // 16  # 4

    # ---------------- DRAM I/O ----------------
    x_in = nc.dram_tensor("x_in", [NB, C, TV], f32, kind="ExternalInput").ap()
    wq_d = nc.dram_tensor("wq", [IC, C], f32, kind="ExternalInput").ap()
    wk_d = nc.dram_tensor("wk", [IC, C], f32, kind="ExternalInput").ap()
    bk_d = nc.dram_tensor("bk", [IC, 1], f32, kind="ExternalInput").ap()
    wv_d = nc.dram_tensor("wv", [IC, C], f32, kind="ExternalInput").ap()
    bv_d = nc.dram_tensor("bv", [1, IC], f32, kind="ExternalInput").ap()
    wt_d = nc.dram_tensor("wt", [C, IC], f32, kind="ExternalInput").ap()
    gm_d = nc.dram_tensor("gamma", [C, 1], f32, kind="ExternalInput").ap()
    bt2_d = nc.dram_tensor("beta", [C, 1], f32, kind="ExternalInput").ap()
    w1_d = nc.dram_tensor("w1", [R, C], f32, kind="ExternalInput").ap()
    b1_d = nc.dram_tensor("b1", [R, 1], f32, kind="ExternalInput").ap()
    w2_d = nc.dram_tensor("w2", [C, R], f32, kind="ExternalInput").ap()
    b2_d = nc.dram_tensor("b2", [C, 1], f32, kind="ExternalInput").ap()
    out_d = nc.dram_tensor("out", [NB, C, TV], f32, kind="ExternalOutput").ap()
    if debug:
        dbg_eb = nc.dram_tensor("dbg_eb", [128, NSB, TV], f16, kind="ExternalOutput").ap()
        dbg_p2 = nc.dram_tensor("dbg_p2", [NB, C, TV], f32, kind="ExternalOutput").ap()
        dbg_gate = nc.dram_tensor("dbg_gate", [C, NB], f32, kind="ExternalOutput").ap()
        dbg_eba = nc.dram_tensor("dbg_eba", [128, NSB], f32, kind="ExternalOutput").ap()
        dbg_pd = nc.dram_tensor("dbg_pd", [128, TV], f16, kind="ExternalOutput").ap()

    with tile.TileContext(nc) as tc, ExitStack() as ctx:
        consts = ctx.enter_context(tc.tile_pool(name="consts", bufs=1))
        xpool = ctx.enter_context(tc.tile_pool(name="xpool", bufs=2))
        workp = ctx.enter_context(tc.tile_pool(name="workp", bufs=2))
        statp = ctx.enter_context(tc.tile_pool(name="statp", bufs=1))
        psA = ctx.enter_context(tc.tile_pool(name="psA", bufs=1, space="PSUM"))
        psP = ctx.enter_context(tc.tile_pool(name="psP", bufs=1, space="PSUM"))
        dramp = ctx.enter_context(tc.tile_pool(name="dramp", bufs=1, space="DRAM"))

        # ------------- warmup collective: absorbs CC cold start + launch skew
        if CC_WARM:
            ccw_in = dramp.tile([C, 2], f32, name="ccw_in")
            ccw_out = dramp.tile([C, 2], f32, name="ccw_out", addr_space="Shared")
            nc.gpsimd.collective_compute(
                "AllReduce", ALU.add, ins=[ccw_in.opt()], outs=[ccw_out.opt()],
                replica_groups=[list(range(N_CORES))])

        # ---------------- input DMAs first (sync queue) ------------------------
        xa32 = [None] * NB
        for b in range(NB):
            t = xpool.tile([C, TV], f32, name="xa32", tag="xa32")
            xa32[b] = t
            nc.sync.dma_start(out=t, in_=x_in[b])

        # ---------------- PE warmup (HAM): dummy matmuls -----------------------
        wuw = consts.tile([C, 128], f16)
        nc.vector.memset(wuw, 0.0)
        wur = consts.tile([C, 512], f16)
        nc.vector.memset(wur, 0.0)
        for i in range(N_WARM_MM):
            wups = psA.tile([128, 512], f32, name="wups", tag="a1")
            nc.tensor.matmul(wups, lhsT=wuw, rhs=wur, start=True, stop=True)

        # ---------------- ACT table warmup -------------------------------------
        warmz = consts.tile([1, 1], f32)
        nc.vector.memset(warmz, 1.0)
        warmo = consts.tile([1, 1], f32)
        nc.scalar.activation(warmo, warmz, AF.Exp)

        # ---------------- weights ----------------------------------------------
        wq_sb = consts.tile([IC, C], f32)
        nc.gpsimd.dma_start(out=wq_sb, in_=wq_d)
        wkbk = consts.tile([IC, C + 1], f32)
        nc.gpsimd.dma_start(out=wkbk[:, 0:C], in_=wk_d)
        nc.gpsimd.dma_start(out=wkbk[:, C:C + 1], in_=bk_d)

        # wvr [65, 36]: cols 0:32 = [Wv^T; bv], col 32 = ones-row marker,
        # col 33 = [r; 0], cols 34:36 pad
        wvr32 = consts.tile([C + 1, 36], f32)
        nc.vector.memset(wvr32, 0.0)
        nc.gpsimd.dma_start(out=wvr32[0:C, 0:IC], in_=wv_d.rearrange("i c -> c i"))
        nc.gpsimd.dma_start(out=wvr32[C:C + 1, 0:IC], in_=bv_d)
        nc.vector.memset(wvr32[C:C + 1, IC:IC + 1], 1.0)

        # G^T | r = Wq^T @ [Wk | bk]
        psg = psA.tile([C, C + 2], f32, name="psg", tag="a2")
        nc.tensor.matmul(psg[:, 0:C + 1], lhsT=wq_sb, rhs=wkbk, start=True, stop=True)
        gt16 = consts.tile([C, C], f16)
        nc.vector.tensor_copy(gt16, psg[:, 0:C])
        nc.vector.tensor_copy(wvr32[0:C, 33:34], psg[:, C:C + 1])
        wvr = consts.tile([C + 1, 36], f16)
        nc.vector.tensor_copy(wvr, wvr32)

        # wtc [128, 66]: rows 0:32 & 64:96 = Wt^T into cols 0:64; rows 32/96
        # put 1 in col 64 (D pass-through)
        wtT2 = consts.tile([128, C], f32)
        nc.gpsimd.dma_start(out=wtT2[0:IC, :], in_=wt_d.rearrange("c i -> i c"))
        nc.gpsimd.dma_start(out=wtT2[64:64 + IC, :], in_=wt_d.rearrange("c i -> i c"))
        wtc = consts.tile([128, 66], f32)
        nc.vector.memset(wtc, 0.0)
        nc.vector.tensor_copy(wtc[0:IC, 0:C], wtT2[0:IC, :])
        nc.vector.tensor_copy(wtc[64:64 + IC, 0:C], wtT2[64:64 + IC, :])
        nc.vector.memset(wtc[IC:IC + 1, 64:65], 1.0)
        nc.vector.memset(wtc[96:97, 64:65], 1.0)
        f32r = mybir.dt.float32r
        wtc_r = consts.tile([128, 66], f32r)
        nc.vector.tensor_copy(wtc_r, wtc)

        ones64 = consts.tile([C + 1, C], f16)
        nc.vector.memset(ones64, 0.0)
        nc.vector.memset(ones64[C:C + 1, :], 1.0)

        w1t = consts.tile([C, R], f32)
        nc.gpsimd.dma_start(out=w1t, in_=w1_d.rearrange("j c -> c j"))
        w2t = consts.tile([R, C], f32)
        nc.gpsimd.dma_start(out=w2t, in_=w2_d.rearrange("c j -> j c"))
        b1_sb = consts.tile([R, 1], f32)
        nc.gpsimd.dma_start(out=b1_sb, in_=b1_d)
        b2_sb = consts.tile([C, 1], f32)
        nc.gpsimd.dma_start(out=b2_sb, in_=b2_d)
        b2n = consts.tile([C, 1], f32)
        nc.vector.tensor_scalar_mul(b2n, b2_sb, -1.0)
        gamma_sb = consts.tile([C, 1], f32)
        nc.gpsimd.dma_start(out=gamma_sb, in_=gm_d)
        beta_sb = consts.tile([C, 1], f32)
        nc.gpsimd.dma_start(out=beta_sb, in_=bt2_d)

        # pd: combined PV partials (f16).  Rows 33:64 / 97:128 are never
        # written and multiply zero weight rows; zero them once so the f16
        # cast garbage can't inject NaN*0.
        pd = statp.tile([128, TV], f32r)
        nc.vector.memset(pd.bitcast(f32)[32:64, :], 0.0)
        nc.vector.memset(pd.bitcast(f32)[96:128, :], 0.0)

        # ---------------- per-batch state ----------------
        xa16 = [None] * NB    # [65, TV] f16 : [X; 1]
        xhi = [None] * NB     # [128, TV] f16 : rows 64:128 = X replica
        ha = [None] * NB      # [128, TV] f16 : G X replicated in both halves
        vt1 = [None] * NB     # [128, NSB, 33] f16 : [V | 1] per s-block
        eba = [None] * NB     # [128, NSB] f32 : rX bias per s-block
        ebb = [None] * NB     # [128, NSB] f32 : scaled fast-exp bias
        eb = [None] * NB      # [128, NSB, TV] f16 : exp(A^T)
        p2 = [None] * NB      # [64, TV] f32
        wts = [None] * NB     # [64, TV] f32 : gate * p2
        avgs = statp.tile([C, NB], f32)
        stats = statp.tile([C, NB * 4, 6], f32)

        def prologue(b):
            x16 = xpool.tile([C + 1, TV], f16, name="xa16", tag="xa16")
            xa16[b] = x16
            # cast + row-sum (for the gate) in one ACT pass
            nc.scalar.activation(x16[0:C, :], xa32[b], AF.Copy,
                                 accum_out=avgs[:, b:b + 1])
            nc.vector.memset(x16[C:C + 1, :], 1.0)
            xh = xpool.tile([128, TV], f16, name="xhi", tag="xhi")
            xhi[b] = xh
            nc.sync.dma_start(out=xh[64:128, :], in_=x16[0:C, :])

            # V^T | 1 | rX for all 13 blocks into one psum bank
            vps = psA.tile([128, NSB, 36], f32, name="vps", tag="a1")
            for j, (off, p) in enumerate(SB):
                nc.tensor.matmul(vps[0:p, j, :], lhsT=x16[:, off:off + p],
                                 rhs=wvr, start=True, stop=True)
            v = xpool.tile([128, NSB, 33], f16, name="vt1", tag="vt1")
            vt1[b] = v
            nc.vector.tensor_copy(v, vps[:, :, 0:33])
            ea = xpool.tile([128, NSB], f32, name="eba", tag="eba")
            eba[b] = ea
            nc.vector.tensor_copy(ea, vps[:, :, 33])
            bbt = xpool.tile([128, NSB], f32, name="ebb", tag="ebb")
            ebb[b] = bbt
            nc.vector.tensor_scalar(bbt, ea, A2, B2, op0=ALU.mult, op1=ALU.add)

            # H = G X, replicated into partition halves via col tiling
            h = xpool.tile([128, TV], f16, name="ha", tag="ha")
            ha[b] = h
            for hi, (h0, h1) in enumerate(HALVES):
                hps = psA.tile([128, 800], f32, name="hps", tag="a2")
                for (c0, c1) in CH:
                    nc.tensor.matmul(hps[0:C, c0:c1], lhsT=gt16,
                                     rhs=x16[0:C, h0 + c0:h0 + c1],
                                     start=True, stop=True, tile_position=(0, 0))
                    nc.tensor.matmul(hps[64:128, c0:c1], lhsT=gt16,
                                     rhs=x16[0:C, h0 + c0:h0 + c1],
                                     start=True, stop=True, tile_position=(0, 64))
                nc.vector.tensor_copy(h[:, h0:h1], hps)

            eb[b] = xpool.tile([128, NSB, TV], f16, name="eb", tag="eb")
            p2[b] = xpool.tile([C, TV], f32, name="p2", tag="p2")
            wts[b] = xpool.tile([C, TV], f32, name="wts", tag="wts")

        def emit_exp(b, j, p, h0, aps, kind):
            dst = eb[b][0:p, j, h0:h0 + 800]
            if kind == 0:
                nc.scalar.activation(dst, aps[0:p, :], AF.Exp,
                                     bias=eba[b][0:p, j:j + 1])
            else:
                nc.vector.tensor_scalar(dst.bitcast(i16), aps[0:p, :], A2,
                                        ebb[b][0:p, j:j + 1],
                                        op0=ALU.mult, op1=ALU.add)

        def emit_pv(b, jj, hb, pacc):
            if jj is None:
                return
            j, j2 = jj
            p = SB[j][1]
            for (c0, c1) in CH:
                nc.tensor.matmul(pacc[0:33, c0:c1],
                                 lhsT=vt1[b][0:p, j, :],
                                 rhs=eb[b][0:p, j, hb + c0:hb + c1],
                                 start=(j == 0), stop=(j == 12),
                                 tile_position=(0, 0),
                                 skip_group_check=True)
                if j2 is not None:
                    nc.tensor.matmul(pacc[64:97, c0:c1],
                                     lhsT=vt1[b][:, j2, :],
                                     rhs=eb[b][:, j2, hb + c0:hb + c1],
                                     start=(j2 == 1), stop=(j2 == 11),
                                     tile_position=(0, 64),
                                     skip_group_check=True)

        def remainder_half(b, hi, pacc):
            hb = HALVES[hi][0]
            nc.vector.tensor_copy(pd[0:33, hb:hb + 800], pacc[0:33, :])
            nc.vector.tensor_copy(pd[64:97, hb:hb + 800], pacc[64:97, :])
            p2ps = psA.tile([66, 800], f32, name="p2ps", tag="a1")
            for (c0, c1) in CH:
                nc.tensor.matmul(p2ps[:, c0:c1], lhsT=wtc_r,
                                 rhs=pd[:, hb + c0:hb + c1],
                                 start=True, stop=True)
            rdt = workp.tile([C + 1, 800], f16, name="rdt", tag="rdt")
            nc.vector.tensor_copy(rdt[C:C + 1, :], p2ps[64:65, :])
            drep = psA.tile([C, 800], f32, name="drep", tag="a2")
            for (c0, c1) in CH:
                nc.tensor.matmul(drep[:, c0:c1], lhsT=ones64[C:C + 1, :],
                                 rhs=rdt[C:C + 1, c0:c1], start=True, stop=True)
            rrep = workp.tile([C, 800], f32, name="rrep", tag="rrep")
            nc.vector.reciprocal_approx_fast(out=rrep, in_=drep)
            nc.vector.tensor_tensor(out=p2[b][:, hb:hb + 800], in0=p2ps[0:C, :],
                                    in1=rrep, op=ALU.mult)
            nc.vector.bn_stats(stats[:, 4 * b + 2 * hi, :],
                               p2[b][:, hb:hb + 512])
            nc.vector.bn_stats(stats[:, 4 * b + 2 * hi + 1, :],
                               p2[b][:, hb + 512:hb + 800])

        def phase1(b):
            """t-halves outermost: 2-bank PV accumulator per half, three
            rotating A-psum tags so the PE never waits on the exp it just
            fed; pair chunks interleaved for row/col-tile concurrency."""
            for hi, (h0, h1) in enumerate(HALVES):
                pacc = psP.tile([128, 800], f32, name="pacc", tag="pacc")
                prev = None
                for si, (j, j2) in enumerate(PAIRS):
                    off, p = SB[j]
                    tagA = "a1" if si % 2 == 0 else "a3"
                    aA = psA.tile([128, 800], f32, name="apsA", tag=tagA)
                    aB = None
                    if j2 is not None:
                        off2, p2_ = SB[j2]
                        aB = psA.tile([128, 800], f32, name="apsB", tag="a2")
                    for (c0, c1) in CH:
                        nc.tensor.matmul(aA[0:p, c0:c1],
                                         lhsT=ha[b][0:C, off:off + p],
                                         rhs=xa16[b][0:C, h0 + c0:h0 + c1],
                                         start=True, stop=True,
                                         tile_position=(0, 0))
                        if aB is not None:
                            nc.tensor.matmul(aB[:, c0:c1],
                                             lhsT=ha[b][64:128, off2:off2 + p2_],
                                             rhs=xhi[b][64:128, h0 + c0:h0 + c1],
                                             start=True, stop=True,
                                             tile_position=(64, 0))
                    emit_pv(b, prev, h0, pacc)
                    if si % 3 == 2:
                        ka, kb = 0, 0   # ACT has slack: double-ACT slot
                    else:
                        ka, kb = (0, 1) if (si + hi) % 2 == 0 else (1, 0)
                    emit_exp(b, j, p, h0, aA, ka)
                    if aB is not None:
                        emit_exp(b, j2, p2_, h0, aB, kb)
                    prev = (j, j2)
                emit_pv(b, prev, h0, pacc)
                remainder_half(b, hi, pacc)

        # ---------------- main schedule ----------------
        prologue(0)
        phase1(0)
        prologue(1)
        phase1(1)

        # channel gate (overlaps phase1(1) tail)
        hps2 = psA.tile([R, NB], f32, name="hps2", tag="a2")
        nc.tensor.matmul(hps2, lhsT=w1t, rhs=avgs, start=True, stop=True)
        h_pre = statp.tile([R, NB], f32)
        nc.vector.tensor_scalar(h_pre, hps2, 1.0 / TV, b1_sb,
                                op0=ALU.mult, op1=ALU.add)
        h_sb = statp.tile([R, NB], f32)
        nc.vector.tensor_scalar_max(h_sb, h_pre, 0.0)
        zps = psA.tile([C, NB], f32, name="zps", tag="a2")
        nc.tensor.matmul(zps, lhsT=w2t, rhs=h_sb, start=True, stop=True)
        eg = statp.tile([C, NB], f32)
        nc.scalar.activation(eg, zps, AF.Exp, bias=b2n, scale=-1.0)
        gp1 = statp.tile([C, NB], f32)
        nc.vector.tensor_scalar_add(gp1, eg, 1.0)
        gate = statp.tile([C, NB], f32)
        nc.vector.reciprocal(gate, gp1)

        nc.vector.tensor_scalar_mul(wts[0], p2[0], gate[:, 0:1])
        nc.vector.tensor_scalar_mul(wts[1], p2[1], gate[:, 1:2])

        # ---------------- BN stats -> allreduce -> global ----------------
        mv = statp.tile([C, 2], f32)
        nc.vector.bn_aggr(out=mv, in_=stats)
        m2 = statp.tile([C, 1], f32)
        nc.vector.tensor_mul(m2, mv[:, 0:1], mv[:, 0:1])
        ex2 = statp.tile([C, 1], f32)
        nc.vector.tensor_add(ex2, mv[:, 1:2], m2)
        sums = statp.tile([C, 2], f32)
        cnt_local = float(NB * TV)
        nc.vector.tensor_scalar_mul(sums[:, 0:1], mv[:, 0:1], cnt_local)
        nc.vector.tensor_scalar_mul(sums[:, 1:2], ex2, cnt_local)

        cc_in = dramp.tile([C, 2], f32, name="cc_in")
        cc_out = dramp.tile([C, 2], f32, name="cc_out", addr_space="Shared")
        nc.sync.dma_start(out=cc_in, in_=sums)
        nc.gpsimd.collective_compute(
            "AllReduce", ALU.add, ins=[cc_in.opt()], outs=[cc_out.opt()],
            replica_groups=[list(range(N_CORES))])
        gs = statp.tile([C, 2], f32)
        nc.sync.dma_start(out=gs, in_=cc_out)

        # mean/var -> sc, nsh (short chain)
        inv_cnt = 1.0 / (N * TV)
        mv2 = statp.tile([C, 2], f32)
        nc.vector.tensor_scalar_mul(mv2, gs, inv_cnt)
        nve = statp.tile([C, 1], f32)
        nc.vector.scalar_tensor_tensor(out=nve, in0=mv2[:, 0:1],
                                       scalar=mv2[:, 0:1], in1=mv2[:, 1:2],
                                       op0=ALU.mult, op1=ALU.subtract)
        ve = statp.tile([C, 1], f32)
        nc.vector.tensor_scalar(ve, nve, -1.0, EPS, op0=ALU.mult, op1=ALU.add)
        sq = statp.tile([C, 1], f32)
        nc.scalar.activation(sq, ve, AF.Sqrt)
        rstd = statp.tile([C, 1], f32)
        nc.vector.reciprocal(rstd, sq)
        sc = statp.tile([C, 1], f32)
        nc.vector.tensor_mul(sc, gamma_sb, rstd)
        nsh = statp.tile([C, 1], f32)
        nc.vector.scalar_tensor_tensor(out=nsh, in0=mv2[:, 0:1], scalar=sc,
                                       in1=beta_sb, op0=ALU.mult, op1=ALU.subtract)

        if debug:
            nc.sync.dma_start(out=dbg_eb, in_=eb[0])
            for _b in range(NB):
                nc.sync.dma_start(out=dbg_p2[_b], in_=p2[_b])
            nc.sync.dma_start(out=dbg_gate, in_=gate)
            nc.sync.dma_start(out=dbg_eba, in_=eba[0])

        # ------- finalize: out = (sc*(gate*p2) - gate*nsh) + x -----------------
        for b in range(NB):
            d_b = statp.tile([C, 1], f32, name=f"d_{b}")
            nc.vector.tensor_mul(d_b, gate[:, b:b + 1], nsh)
            osb = workp.tile([C, TV], f32, name="osb", tag="osb")
            for (h0, h1) in HALVES:
                ot = workp.tile([C, 800], f32, name="ot", tag="ot")
                nc.vector.tensor_scalar(ot, wts[b][:, h0:h1], sc, d_b,
                                        op0=ALU.mult, op1=ALU.subtract)
                nc.vector.tensor_tensor(out=osb[:, h0:h1], in0=ot,
                                        in1=xa32[b][:, h0:h1], op=ALU.add)
            nc.sync.dma_start(out=out_d[b], in_=osb)


_CACHE = {}


def _get_compiled(debug=False):
    key = ("nc", debug)
    if key in _CACHE:
        return _CACHE[key]
    import concourse.bacc as bacc

    nc = bacc.Bacc("TRN2", target_bir_lowering=False, debug=False,
                   enable_asserts=False, num_devices=N_CORES)
    _build(nc, debug=debug)
    nc.compile()
    _CACHE[key] = nc
    return nc


def _run(inputs, trace=False, debug=False, **kw):
    from concourse import bass_utils

    nc = _get_compiled(debug=debug)
    x = np.ascontiguousarray(np.asarray(inputs["x"], dtype=np.float32))
    x = x.reshape(N, C, TV)
    f = lambda a: np.ascontiguousarray(np.asarray(a, dtype=np.float32))
    common = {
        "wq": f(inputs["Wq"]),
        "wk": f(inputs["Wk"]),
        "bk": f(inputs["bk"]).reshape(IC, 1),
        "wv": f(inputs["Wv"]),
        "bv": f(inputs["bv"]).reshape(1, IC),
        "wt": f(inputs["Wt"]),
        "gamma": f(inputs["gamma"]).reshape(C, 1),
        "beta": f(inputs["beta"]).reshape(C, 1),
        "w1": f(inputs["W1"]),
        "b1": f(inputs["b1"]).reshape(C // 16, 1),
        "w2": f(inputs["W2"]),
        "b2": f(inputs["b2"]).reshape(C, 1),
    }
    in_maps = []
    for c in range(N_CORES):
        m = dict(common)
        m["x_in"] = np.ascontiguousarray(x[c * NB:(c + 1) * NB])
        in_maps.append(m)
    try:
        res = bass_utils.run_bass_kernel_spmd(
            nc, in_maps, core_ids=list(range(N_CORES)), trace=trace, **kw)
    except Exception:
        import time as _time
        _time.sleep(5)
        res = bass_utils.run_bass_kernel_spmd(
            nc, in_maps, core_ids=list(range(N_CORES)), trace=False, **kw)
    out = np.concatenate([res.results[c]["out"] for c in range(N_CORES)], axis=0)
    return out.reshape(N, C, T, V).astype(np.float32), res


def kernel(**inputs):
    return _run(inputs, trace=False)[0]


# revision 29
# speedup vs baseline: 1.6082x; 1.0006x over previous
"""Trainium2 Bass kernel for nn_FEM_35072702939287 (attention + BN + channel gate).

Math (validated in numpy vs reference):
  A^T[s,t] = X_s^T G^T X_t + rX[s] (+ t-only/const terms that drop under
  softmax over s), G = Wk^T Wq, rX = (Wq^T bk)^T X.  The rX term rides as
  a per-partition bias into exp (ACT bias operand / tensor_scalar scalar2),
  so the A matmul contracts over exactly K=64 -> two s-blocks run
  CONCURRENTLY on the PE via row tiling (tile_position (0,0) | (64,0)).
  V^T blocks [V | 1 | rX] come from one matmul per block; PV accumulates
  [V|1]^T exp(A^T) with even blocks on PSUM rows 0:33 and odd blocks on
  rows 64:97 via col tiling (tile_position (0,0) | (0,64)) -> concurrent.
  A combining matmul with lhsT = [Wt^T; 1-row; Wt^T; 1-row] reduces both
  partials and applies the Wt conv in one shot; /D folds in after, bt
  cancels under BN.  BN batch stats all-reduced across 8 cores; a dummy
  all-reduce at kernel start warms the CC engine and absorbs launch skew.
  exp is split across 3 engines: ScalarE exact exp; DVE+GpSimd compute a
  Schraudolph fast exp (i16 = rne(1477.32*(A+bias) + 15316), bitcast f16,
  max rel err ~3%; end-to-end <1e-2 validated vs reference).

Sharding: data-parallel over batch N=16 -> 2 batches per core x 8 cores.
"""

import numpy as np

N_CORES = 8
N, C, T, V = 16, 64, 64, 25
TV = T * V            # 1600
IC = 32
NB = N // N_CORES     # batches per core
EPS = 1e-5
NSB = 13              # 12 full 128-row s-blocks + one 64-row tail
SB = [(j * 128, 128) for j in range(12)] + [(1536, 64)]
PAIRS = [(0, 1), (2, 3), (4, 5), (6, 7), (8, 9), (10, 11), (12, None)]
HALVES = [(0, 800), (800, 1600)]
CH = [(0, 512), (512, 800)]       # psum-bank chunks inside an 800-half tile
# bank-aligned chunks for the 1600-wide PV accumulator (matmul output
# must not cross a 2KB PSUM bank boundary)
CH_PV = [(0, 512), (512, 1024), (1024, 1536), (1536, 1600)]
A2 = 1024.0 * 1.4426950408889634  # fast-exp scale
B2 = 15.0 * 1024.0 - 44.0         # fast-exp shift (rne-optimal C=-44)

# exp engine per tile: 0=ACT exact exp, 1=DVE fast exp (GPSIMD can't read
# PSUM, so it gets the SBUF-only elementwise work instead).
# kinds order per pair-slot: [(j,h0), (j,h1), (j2,h0), (j2,h1)]
KINDS_EVEN = [0, 1, 0, 1]   # 2 ACT / 2 DVE
KINDS_LIGHT = [0, 1, 0, 1]

ROW_TILE = True   # concurrent A-matmul pairs via PE row tiling
COL_TILE = True   # concurrent PV pairs via PE col tiling
N_WARM_MM = 22    # PE warmup matmuls (HAM)
CC_WARM = True    # dummy collective at start


def _build(nc, debug=False):
    import concourse.tile as tile
    from concourse import mybir
    from contextlib import ExitStack

    f32 = mybir.dt.float32
    f16 = mybir.dt.float16
    i16 = mybir.dt.int16
    AF = mybir.ActivationFunctionType
    ALU = mybir.AluOpType
    AX = mybir.AxisListType
    R = C // 16  # 4

    # ---------------- DRAM I/O ----------------
    x_in = nc.dram_tensor("x_in", [NB, C, TV], f32, kind="ExternalInput").ap()
    wq_d = nc.dram_tensor("wq", [IC, C], f32, kind="ExternalInput").ap()
    wk_d = nc.dram_tensor("wk", [IC, C], f32, kind="ExternalInput").ap()
    bk_d = nc.dram_tensor("bk", [IC, 1], f32, kind="ExternalInput").ap()
    wv_d = nc.dram_tensor("wv", [IC, C], f32, kind="ExternalInput").ap()
    bv_d = nc.dram_tensor("bv", [1, IC], f32, kind="ExternalInput").ap()
    wt_d = nc.dram_tensor("wt", [C, IC], f32, kind="ExternalInput").ap()
    gm_d = nc.dram_tensor("gamma", [C, 1], f32, kind="ExternalInput").ap()
    bt2_d = nc.dram_tensor("beta", [C, 1], f32, kind="ExternalInput").ap()
    w1_d = nc.dram_tensor("w1", [R, C], f32, kind="ExternalInput").ap()
    b1_d = nc.dram_tensor("b1", [R, 1], f32, kind="ExternalInput").ap()
    w2_d = nc.dram_tensor("w2", [C, R], f32, kind="ExternalInput").ap()
    b2_d = nc.dram_tensor("b2", [C, 1], f32, kind="ExternalInput").ap()
    out_d = nc.dram_tensor("out", [NB, C, TV], f32, kind="ExternalOutput").ap()
    if debug:
        dbg_eb = nc.dram_tensor("dbg_eb", [128, NSB, TV], f16, kind="ExternalOutput").ap()
        dbg_p2 = nc.dram_tensor("dbg_p2", [NB, C, TV], f32, kind="ExternalOutput").ap()
        dbg_gate = nc.dram_tensor("dbg_gate", [C, NB], f32, kind="ExternalOutput").ap()
        dbg_eba = nc.dram_tensor("dbg_eba", [128, NSB], f32, kind="ExternalOutput").ap()
        dbg_pd = nc.dram_tensor("dbg_pd", [128, TV], f16, kind="ExternalOutput").ap()

    with tile.TileContext(nc) as tc, ExitStack() as ctx:
        consts = ctx.enter_context(tc.tile_pool(name="consts", bufs=1))
        xpool = ctx.enter_context(tc.tile_pool(name="xpool", bufs=2))
        workp = ctx.enter_context(tc.tile_pool(name="workp", bufs=2))
        statp = ctx.enter_context(tc.tile_pool(name="statp", bufs=1))
        psA = ctx.enter_context(tc.tile_pool(name="psA", bufs=1, space="PSUM"))
        psP = ctx.enter_context(tc.tile_pool(name="psP", bufs=1, space="PSUM"))
        dramp = ctx.enter_context(tc.tile_pool(name="dramp", bufs=1, space="DRAM"))

        # ------------- warmup collective: absorbs CC cold start + launch skew
        if CC_WARM:
            ccw_in = dramp.tile([C, 2], f32, name="ccw_in")
            ccw_out = dramp.tile([C, 2], f32, name="ccw_out", addr_space="Shared")
            nc.gpsimd.collective_compute(
                "AllReduce", ALU.add, ins=[ccw_in.opt()], outs=[ccw_out.opt()],
                replica_groups=[list(range(N_CORES))])

        # ---------------- input DMAs first (sync queue) ------------------------
        xa32 = [None] * NB
        for b in range(NB):
            t = xpool.tile([C, TV], f32, name="xa32", tag="xa32")
            xa32[b] = t
            nc.sync.dma_start(out=t, in_=x_in[b])

        # ---------------- PE warmup (HAM): dummy matmuls -----------------------
        wuw = consts.tile([C, 128], f16)
        nc.vector.memset(wuw, 0.0)
        wur = consts.tile([C, 512], f16)
        nc.vector.memset(wur, 0.0)
        for i in range(N_WARM_MM):
            wups = psA.tile([128, 512], f32, name="wups", tag="a1")
            nc.tensor.matmul(wups, lhsT=wuw, rhs=wur, start=True, stop=True)

        # ---------------- ACT table warmup -------------------------------------
        warmz = consts.tile([1, 1], f32)
        nc.vector.memset(warmz, 1.0)
        warmo = consts.tile([1, 1], f32)
        nc.scalar.activation(warmo, warmz, AF.Exp)

        # ---------------- weights ----------------------------------------------
        wq_sb = consts.tile([IC, C], f32)
        nc.gpsimd.dma_start(out=wq_sb, in_=wq_d)
        wkbk = consts.tile([IC, C + 1], f32)
        nc.gpsimd.dma_start(out=wkbk[:, 0:C], in_=wk_d)
        nc.gpsimd.dma_start(out=wkbk[:, C:C + 1], in_=bk_d)

        # wvr [65, 36]: cols 0:32 = [Wv^T; bv], col 32 = ones-row marker,
        # col 33 = [r; 0], cols 34:36 pad
        wvr32 = consts.tile([C + 1, 36], f32)
        nc.vector.memset(wvr32, 0.0)
        nc.gpsimd.dma_start(out=wvr32[0:C, 0:IC], in_=wv_d.rearrange("i c -> c i"))
        nc.gpsimd.dma_start(out=wvr32[C:C + 1, 0:IC], in_=bv_d)
        nc.vector.memset(wvr32[C:C + 1, IC:IC + 1], 1.0)

        # G^T | r = Wq^T @ [Wk | bk]
        psg = psA.tile([C, C + 2], f32, name="psg", tag="a2")
        nc.tensor.matmul(psg[:, 0:C + 1], lhsT=wq_sb, rhs=wkbk, start=True, stop=True)
        gt16 = consts.tile([C, C], f16)
        nc.vector.tensor_copy(gt16, psg[:, 0:C])
        nc.vector.tensor_copy(wvr32[0:C, 33:34], psg[:, C:C + 1])
        wvr = consts.tile([C + 1, 36], f16)
        nc.vector.tensor_copy(wvr, wvr32)

        # wtc [128, 66]: rows 0:32 & 64:96 = Wt^T into cols 0:64; rows 32/96
        # put 1 in col 64 (D pass-through)
        wtT2 = consts.tile([128, C], f32)
        nc.gpsimd.dma_start(out=wtT2[0:IC, :], in_=wt_d.rearrange("c i -> i c"))
        nc.gpsimd.dma_start(out=wtT2[64:64 + IC, :], in_=wt_d.rearrange("c i -> i c"))
        wtc = consts.tile([128, 66], f32)
        nc.vector.memset(wtc, 0.0)
        nc.vector.tensor_copy(wtc[0:IC, 0:C], wtT2[0:IC, :])
        nc.vector.tensor_copy(wtc[64:64 + IC, 0:C], wtT2[64:64 + IC, :])
        nc.vector.memset(wtc[IC:IC + 1, 64:65], 1.0)
        nc.vector.memset(wtc[96:97, 64:65], 1.0)
        f32r = mybir.dt.float32r
        wtc_r = consts.tile([128, 66], f32r)
        nc.vector.tensor_copy(wtc_r, wtc)

        ones64 = consts.tile([C + 1, C], f16)
        nc.vector.memset(ones64, 0.0)
        nc.vector.memset(ones64[C:C + 1, :], 1.0)

        w1t = consts.tile([C, R], f32)
        nc.gpsimd.dma_start(out=w1t, in_=w1_d.rearrange("j c -> c j"))
        w2t = consts.tile([R, C], f32)
        nc.gpsimd.dma_start(out=w2t, in_=w2_d.rearrange("c j -> j c"))
        b1_sb = consts.tile([R, 1], f32)
        nc.gpsimd.dma_start(out=b1_sb, in_=b1_d)
        b2_sb = consts.tile([C, 1], f32)
        nc.gpsimd.dma_start(out=b2_sb, in_=b2_d)
        b2n = consts.tile([C, 1], f32)
        nc.vector.tensor_scalar_mul(b2n, b2_sb, -1.0)
        gamma_sb = consts.tile([C, 1], f32)
        nc.gpsimd.dma_start(out=gamma_sb, in_=gm_d)
        beta_sb = consts.tile([C, 1], f32)
        nc.gpsimd.dma_start(out=beta_sb, in_=bt2_d)

        # pd: combined PV partials (f16).  Rows 33:64 / 97:128 are never
        # written and multiply zero weight rows; zero them once so the f16
        # cast garbage can't inject NaN*0.
        pd = statp.tile([128, TV], f32r)
        nc.vector.memset(pd.bitcast(f32)[32:64, :], 0.0)
        nc.vector.memset(pd.bitcast(f32)[96:128, :], 0.0)

        # ---------------- per-batch state ----------------
        xa16 = [None] * NB    # [65, TV] f16 : [X; 1]
        xhi = [None] * NB     # [128, TV] f16 : rows 64:128 = X replica
        ha = [None] * NB      # [128, TV] f16 : G X replicated in both halves
        vt1 = [None] * NB     # [128, NSB, 33] f16 : [V | 1] per s-block
        eba = [None] * NB     # [128, NSB] f32 : rX bias per s-block
        ebb = [None] * NB     # [128, NSB] f32 : scaled fast-exp bias
        eb = [None] * NB      # [128, NSB, TV] f16 : exp(A^T)
        p2 = [None] * NB      # [64, TV] f32
        wts = [None] * NB     # [64, TV] f32 : gate * p2
        avgs = statp.tile([C, NB], f32)
        stats = statp.tile([C, NB * 4, 6], f32)

        def prologue(b):
            x16 = xpool.tile([C + 1, TV], f16, name="xa16", tag="xa16")
            xa16[b] = x16
            # cast + row-sum (for the gate) in one ACT pass
            nc.scalar.activation(x16[0:C, :], xa32[b], AF.Copy,
                                 accum_out=avgs[:, b:b + 1])
            nc.vector.memset(x16[C:C + 1, :], 1.0)
            xh = xpool.tile([128, TV], f16, name="xhi", tag="xhi")
            xhi[b] = xh
            nc.sync.dma_start(out=xh[64:128, :], in_=x16[0:C, :])

            # V^T | 1 | rX for all 13 blocks into one psum bank
            vps = psA.tile([128, NSB, 36], f32, name="vps", tag="a1")
            for j, (off, p) in enumerate(SB):
                nc.tensor.matmul(vps[0:p, j, :], lhsT=x16[:, off:off + p],
                                 rhs=wvr, start=True, stop=True)
            v = xpool.tile([128, NSB, 33], f16, name="vt1", tag="vt1")
            vt1[b] = v
            nc.vector.tensor_copy(v, vps[:, :, 0:33])
            ea = xpool.tile([128, NSB], f32, name="eba", tag="eba")
            eba[b] = ea
            nc.vector.tensor_copy(ea, vps[:, :, 33])
            bbt = xpool.tile([128, NSB], f32, name="ebb", tag="ebb")
            ebb[b] = bbt
            nc.vector.tensor_scalar(bbt, ea, A2, B2, op0=ALU.mult, op1=ALU.add)

            # H = G X, replicated into partition halves via col tiling
            h = xpool.tile([128, TV], f16, name="ha", tag="ha")
            ha[b] = h
            for hi, (h0, h1) in enumerate(HALVES):
                hps = psA.tile([128, 800], f32, name="hps", tag="a2")
                for (c0, c1) in CH:
                    nc.tensor.matmul(hps[0:C, c0:c1], lhsT=gt16,
                                     rhs=x16[0:C, h0 + c0:h0 + c1],
                                     start=True, stop=True, tile_position=(0, 0))
                    nc.tensor.matmul(hps[64:128, c0:c1], lhsT=gt16,
                                     rhs=x16[0:C, h0 + c0:h0 + c1],
                                     start=True, stop=True, tile_position=(0, 64))
                nc.vector.tensor_copy(h[:, h0:h1], hps)

            eb[b] = xpool.tile([128, NSB, TV], f16, name="eb", tag="eb")
            p2[b] = xpool.tile([C, TV], f32, name="p2", tag="p2")
            wts[b] = xpool.tile([C, TV], f32, name="wts", tag="wts")

        def emit_exp(b, j, p, h0, aps, kind):
            dst = eb[b][0:p, j, h0:h0 + 800]
            if kind == 0:
                nc.scalar.activation(dst, aps[0:p, :], AF.Exp,
                                     bias=eba[b][0:p, j:j + 1])
            else:
                nc.vector.tensor_scalar(dst.bitcast(i16), aps[0:p, :], A2,
                                        ebb[b][0:p, j:j + 1],
                                        op0=ALU.mult, op1=ALU.add)

        def emit_pv(b, jj, hb, pacc):
            if jj is None:
                return
            j, j2 = jj
            p = SB[j][1]
            for (c0, c1) in CH:
                nc.tensor.matmul(pacc[0:33, c0:c1],
                                 lhsT=vt1[b][0:p, j, :],
                                 rhs=eb[b][0:p, j, hb + c0:hb + c1],
                                 start=(j == 0), stop=(j == 12),
                                 tile_position=(0, 0),
                                 skip_group_check=True)
                if j2 is not None:
                    nc.tensor.matmul(pacc[64:97, c0:c1],
                                     lhsT=vt1[b][:, j2, :],
                                     rhs=eb[b][:, j2, hb + c0:hb + c1],
                                     start=(j2 == 1), stop=(j2 == 11),
                                     tile_position=(0, 64),
                                     skip_group_check=True)

        def remainder_half(b, hi, pacc):
            hb = HALVES[hi][0]
            nc.vector.tensor_copy(pd[0:33, hb:hb + 800], pacc[0:33, :])
            nc.vector.tensor_copy(pd[64:97, hb:hb + 800], pacc[64:97, :])
            p2ps = psA.tile([66, 800], f32, name="p2ps", tag="a1")
            for (c0, c1) in CH:
                nc.tensor.matmul(p2ps[:, c0:c1], lhsT=wtc_r,
                                 rhs=pd[:, hb + c0:hb + c1],
                                 start=True, stop=True)
            rdt = workp.tile([C + 1, 800], f16, name="rdt", tag="rdt")
            nc.vector.tensor_copy(rdt[C:C + 1, :], p2ps[64:65, :])
            drep = psA.tile([C, 800], f32, name="drep", tag="a2")
            for (c0, c1) in CH:
                nc.tensor.matmul(drep[:, c0:c1], lhsT=ones64[C:C + 1, :],
                                 rhs=rdt[C:C + 1, c0:c1], start=True, stop=True)
            rrep = workp.tile([C, 800], f32, name="rrep", tag="rrep")
            nc.vector.reciprocal_approx_fast(out=rrep, in_=drep)
            nc.vector.tensor_tensor(out=p2[b][:, hb:hb + 800], in0=p2ps[0:C, :],
                                    in1=rrep, op=ALU.mult)
            nc.vector.bn_stats(stats[:, 4 * b + 2 * hi, :],
                               p2[b][:, hb:hb + 512])
            nc.vector.bn_stats(stats[:, 4 * b + 2 * hi + 1, :],
                               p2[b][:, hb + 512:hb + 800])

        def phase1(b):
            """t-halves outermost: 2-bank PV accumulator per half, three
            rotating A-psum tags so the PE never waits on the exp it just
            fed; pair chunks interleaved for row/col-tile concurrency.
            Each half's remainder is deferred into the next half's slot
            stream so its DVE-dependent matmuls don't stall the PE queue."""
            deferred = [None]
            for hi, (h0, h1) in enumerate(HALVES):
                pacc = psP.tile([128, 800], f32, name="pacc", tag="pacc")
                prev = None
                for si, (j, j2) in enumerate(PAIRS):
                    if si == 2 and deferred[0] is not None:
                        remainder_half(b, hi - 1, deferred[0])
                        deferred[0] = None
                    off, p = SB[j]
                    tagA = "a1" if si % 2 == 0 else "a3"
                    aA = psA.tile([128, 800], f32, name="apsA", tag=tagA)
                    aB = None
                    if j2 is not None:
                        off2, p2_ = SB[j2]
                        aB = psA.tile([128, 800], f32, name="apsB", tag="a2")
                    for (c0, c1) in CH:
                        nc.tensor.matmul(aA[0:p, c0:c1],
                                         lhsT=ha[b][0:C, off:off + p],
                                         rhs=xa16[b][0:C, h0 + c0:h0 + c1],
                                         start=True, stop=True,
                                         tile_position=(0, 0))
                        if aB is not None:
                            nc.tensor.matmul(aB[:, c0:c1],
                                             lhsT=ha[b][64:128, off2:off2 + p2_],
                                             rhs=xhi[b][64:128, h0 + c0:h0 + c1],
                                             start=True, stop=True,
                                             tile_position=(64, 0))
                    emit_pv(b, prev, h0, pacc)
                    if si % 3 == 2:
                        ka, kb = 0, 0   # ACT has slack: double-ACT slot
                    else:
                        ka, kb = (0, 1) if (si + hi) % 2 == 0 else (1, 0)
                    emit_exp(b, j, p, h0, aA, ka)
                    if aB is not None:
                        emit_exp(b, j2, p2_, h0, aB, kb)
                    prev = (j, j2)
                emit_pv(b, prev, h0, pacc)
                if hi == 0:
                    deferred[0] = pacc
                else:
                    remainder_half(b, hi, pacc)

        # ---------------- main schedule ----------------
        prologue(0)
        phase1(0)
        prologue(1)
        phase1(1)

        # channel gate (overlaps phase1(1) tail)
        hps2 = psA.tile([R, NB], f32, name="hps2", tag="a2")
        nc.tensor.matmul(hps2, lhsT=w1t, rhs=avgs, start=True, stop=True)
        h_pre = statp.tile([R, NB], f32)
        nc.vector.tensor_scalar(h_pre, hps2, 1.0 / TV, b1_sb,
                                op0=ALU.mult, op1=ALU.add)
        h_sb = statp.tile([R, NB], f32)
        nc.vector.tensor_scalar_max(h_sb, h_pre, 0.0)
        zps = psA.tile([C, NB], f32, name="zps", tag="a2")
        nc.tensor.matmul(zps, lhsT=w2t, rhs=h_sb, start=True, stop=True)
        eg = statp.tile([C, NB], f32)
        nc.scalar.activation(eg, zps, AF.Exp, bias=b2n, scale=-1.0)
        gp1 = statp.tile([C, NB], f32)
        nc.vector.tensor_scalar_add(gp1, eg, 1.0)
        gate = statp.tile([C, NB], f32)
        nc.vector.reciprocal(gate, gp1)

        nc.vector.tensor_scalar_mul(wts[0], p2[0], gate[:, 0:1])
        nc.vector.tensor_scalar_mul(wts[1], p2[1], gate[:, 1:2])

        # ---------------- BN stats -> allreduce -> global ----------------
        mv = statp.tile([C, 2], f32)
        nc.vector.bn_aggr(out=mv, in_=stats)
        m2 = statp.tile([C, 1], f32)
        nc.vector.tensor_mul(m2, mv[:, 0:1], mv[:, 0:1])
        ex2 = statp.tile([C, 1], f32)
        nc.vector.tensor_add(ex2, mv[:, 1:2], m2)
        sums = statp.tile([C, 2], f32)
        cnt_local = float(NB * TV)
        nc.vector.tensor_scalar_mul(sums[:, 0:1], mv[:, 0:1], cnt_local)
        nc.vector.tensor_scalar_mul(sums[:, 1:2], ex2, cnt_local)

        cc_in = dramp.tile([C, 2], f32, name="cc_in")
        cc_out = dramp.tile([C, 2], f32, name="cc_out", addr_space="Shared")
        nc.sync.dma_start(out=cc_in, in_=sums)
        nc.gpsimd.collective_compute(
            "AllReduce", ALU.add, ins=[cc_in.opt()], outs=[cc_out.opt()],
            replica_groups=[list(range(N_CORES))])
        gs = statp.tile([C, 2], f32)
        nc.sync.dma_start(out=gs, in_=cc_out)

        # mean/var -> sc, nsh (short chain)
        inv_cnt = 1.0 / (N * TV)
        mv2 = statp.tile([C, 2], f32)
        nc.vector.tensor_scalar_mul(mv2, gs, inv_cnt)
        nve = statp.tile([C, 1], f32)
        nc.vector.scalar_tensor_tensor(out=nve, in0=mv2[:, 0:1],
                                       scalar=mv2[:, 0:1], in1=mv2[:, 1:2],
                                       op0=ALU.mult, op1=ALU.subtract)
        ve = statp.tile([C, 1], f32)
        nc.vector.tensor_scalar(ve, nve, -1.0, EPS, op0=ALU.mult, op1=ALU.add)
        sq = statp.tile([C, 1], f32)
        nc.scalar.activation(sq, ve, AF.Sqrt)
        rstd = statp.tile([C, 1], f32)
        nc.vector.reciprocal(rstd, sq)
        sc = statp.tile([C, 1], f32)
        nc.vector.tensor_mul(sc, gamma_sb, rstd)
        nsh = statp.tile([C, 1], f32)
        nc.vector.scalar_tensor_tensor(out=nsh, in0=mv2[:, 0:1], scalar=sc,
                                       in1=beta_sb, op0=ALU.mult, op1=ALU.subtract)

        if debug:
            nc.sync.dma_start(out=dbg_eb, in_=eb[0])
            for _b in range(NB):
                nc.sync.dma_start(out=dbg_p2[_b], in_=p2[_b])
            nc.sync.dma_start(out=dbg_gate, in_=gate)
            nc.sync.dma_start(out=dbg_eba, in_=eba[0])

        # ------- finalize: out = (sc*(gate*p2) - gate*nsh) + x -----------------
        for b in range(NB):
            d_b = statp.tile([C, 1], f32, name=f"d_{b}")
            nc.vector.tensor_mul(d_b, gate[:, b:b + 1], nsh)
            osb = workp.tile([C, TV], f32, name="osb", tag="osb")
            for (h0, h1) in HALVES:
                ot = workp.tile([C, 800], f32, name="ot", tag="ot")
                nc.vector.tensor_scalar(ot, wts[b][:, h0:h1], sc, d_b,
                                        op0=ALU.mult, op1=ALU.subtract)
                nc.vector.tensor_tensor(out=osb[:, h0:h1], in0=ot,
                                        in1=xa32[b][:, h0:h1], op=ALU.add)
            nc.sync.dma_start(out=out_d[b], in_=osb)


_CACHE = {}


def _get_compiled(debug=False):
    key = ("nc", debug)
    if key in _CACHE:
        return _CACHE[key]
    import concourse.bacc as bacc

    nc = bacc.Bacc("TRN2", target_bir_lowering=False, debug=False,
                   enable_asserts=False, num_devices=N_CORES)
    _build(nc, debug=debug)
    nc.compile()
    _CACHE[key] = nc
    return nc


def _run(inputs, trace=False, debug=False, **kw):
    from concourse import bass_utils

    nc = _get_compiled(debug=debug)
    x = np.ascontiguousarray(np.asarray(inputs["x"], dtype=np.float32))
    x = x.reshape(N, C, TV)
    f = lambda a: np.ascontiguousarray(np.asarray(a, dtype=np.float32))
    common = {
        "wq": f(inputs["Wq"]),
        "wk": f(inputs["Wk"]),
        "bk": f(inputs["bk"]).reshape(IC, 1),
        "wv": f(inputs["Wv"]),
        "bv": f(inputs["bv"]).reshape(1, IC),
        "wt": f(inputs["Wt"]),
        "gamma": f(inputs["gamma"]).reshape(C, 1),
        "beta": f(inputs["beta"]).reshape(C, 1),
        "w1": f(inputs["W1"]),
        "b1": f(inputs["b1"]).reshape(C // 16, 1),
        "w2": f(inputs["W2"]),
        "b2": f(inputs["b2"]).reshape(C, 1),
    }
    in_maps = []
    for c in range(N_CORES):
        m = dict(common)
        m["x_in"] = np.ascontiguousarray(x[c * NB:(c + 1) * NB])
        in_maps.append(m)
    try:
        res = bass_utils.run_bass_kernel_spmd(
            nc, in_maps, core_ids=list(range(N_CORES)), trace=trace, **kw)
    except Exception:
        import time as _time
        _time.sleep(5)
        res = bass_utils.run_bass_kernel_spmd(
            nc, in_maps, core_ids=list(range(N_CORES)), trace=False, **kw)
    out = np.concatenate([res.results[c]["out"] for c in range(N_CORES)], axis=0)
    return out.reshape(N, C, T, V).astype(np.float32), res


def kernel(**inputs):
    return _run(inputs, trace=False)[0]


# revision 30
# speedup vs baseline: 1.7550x; 1.0913x over previous
"""Trainium2 Bass kernel for nn_FEM_35072702939287 (attention + BN + channel gate).

Math restructuring (validated vs reference to ~1e-6):
  A[t,s] = (Wk x + bk)[:,t] . (Wq x + bq)[:,s]
         = [X_aug^T @ H_aug](t,s) + row-const(t) + const
  where X_aug = [X; 1] (65 x TV), H_aug = [G X ; r^T X], G = Wk^T Wq,
  r = Wq^T bk.  Row-constant terms drop under softmax over s.
  We compute A^T tiles [s_block=128, t] so softmax's denominator
  D[t] = sum_s exp(A^T[s,t]) falls out of the PV matmul by augmenting
  V^T with a ones column.  The division by D is folded past the Wt conv:
  P2 = (Wt^T @ P~) * (1/D broadcast); the conv bias bt cancels under BN.
  BatchNorm batch stats are all-reduced across the 8 cores.
  All pre-exp matmuls (H, A, V^T) run in fp16 (fp32 PSUM accumulate):
  full PE rate, FWL fast weight loads, no fp32r ISA restrictions.

Sharding: data-parallel over batch N=16 -> 2 batches per core x 8 cores.
"""

import os
import numpy as np

N_CORES = 8
N, C, T, V = 16, 64, 64, 25
TV = T * V            # 1600
IC = 32
NB = N // N_CORES     # batches per core
EPS = 1e-5
NSB = 13              # 12 full 128-row s-blocks + one 64-row tail
SB = [(j * 128, 128) for j in range(12)] + [(1536, 64)]
# phase1 A-psum half-tiles [128, 800] (2 banks); chunks bank-aligned inside
HALVES = [(0, 800), (800, 1600)]
CH_H = [(0, 512), (512, 800)]
# H-matmul chunks (PSUM-bank aligned, one bank per matmul)
CH_A = [(0, 512), (512, 1024), (1024, 1536), (1536, 1600)]
# phase2 chunks: two 800-wide accumulators (f16 matmuls may move up to 1024)
CH_P = [(0, 800), (800, 1600)]
# fp32r sub-chunks within an 800-wide psum tile (fp32r moving max is 512)
CH_R = [(0, 512), (512, 800)]


def _build(nc, debug=False):
    import concourse.tile as tile
    from concourse import mybir
    from contextlib import ExitStack

    f32 = mybir.dt.float32
    f32r = mybir.dt.float32r
    f16 = mybir.dt.float16
    AF = mybir.ActivationFunctionType
    ALU = mybir.AluOpType
    AX = mybir.AxisListType

    def r32(ap):
        return ap.bitcast(f32r)

    # ---------------- DRAM I/O ----------------
    x_in = nc.dram_tensor("x_in", [NB, C, TV], f32, kind="ExternalInput").ap()
    wq_d = nc.dram_tensor("wq", [IC, C], f32, kind="ExternalInput").ap()
    wk_d = nc.dram_tensor("wk", [IC, C], f32, kind="ExternalInput").ap()
    bk_d = nc.dram_tensor("bk", [IC, 1], f32, kind="ExternalInput").ap()
    wv_d = nc.dram_tensor("wv", [IC, C], f32, kind="ExternalInput").ap()
    bv_d = nc.dram_tensor("bv", [1, IC], f32, kind="ExternalInput").ap()
    wt_d = nc.dram_tensor("wt", [C, IC], f32, kind="ExternalInput").ap()
    bt_d = nc.dram_tensor("bt", [1, C], f32, kind="ExternalInput").ap()
    gm_d = nc.dram_tensor("gamma", [C, 1], f32, kind="ExternalInput").ap()
    bt2_d = nc.dram_tensor("beta", [C, 1], f32, kind="ExternalInput").ap()
    w1_d = nc.dram_tensor("w1", [C // 16, C], f32, kind="ExternalInput").ap()
    b1_d = nc.dram_tensor("b1", [C // 16, 1], f32, kind="ExternalInput").ap()
    w2_d = nc.dram_tensor("w2", [C, C // 16], f32, kind="ExternalInput").ap()
    b2_d = nc.dram_tensor("b2", [C, 1], f32, kind="ExternalInput").ap()
    out_d = nc.dram_tensor("out", [NB, C, TV], f32, kind="ExternalOutput").ap()
    if debug:
        dbg_ha = nc.dram_tensor("dbg_ha", [C + 1, TV], f32, kind="ExternalOutput").ap()
        dbg_vt = nc.dram_tensor("dbg_vt", [128, NSB, IC + 1], mybir.dt.float16, kind="ExternalOutput").ap()
        dbg_eb = nc.dram_tensor("dbg_eb", [128, NSB, TV], mybir.dt.float16, kind="ExternalOutput").ap()
        dbg_p2 = nc.dram_tensor("dbg_p2", [NB, C, TV], f32, kind="ExternalOutput").ap()
        dbg_gate = nc.dram_tensor("dbg_gate", [C, NB], f32, kind="ExternalOutput").ap()
        dbg_sums = nc.dram_tensor("dbg_sums", [C, 2], f32, kind="ExternalOutput").ap()
        dbg_gs = nc.dram_tensor("dbg_gs", [C, 2], f32, kind="ExternalOutput").ap()
        dbg_scsh = nc.dram_tensor("dbg_scsh", [C, 2], f32, kind="ExternalOutput").ap()

    R = C // 16  # 4

    with tile.TileContext(nc) as tc, ExitStack() as ctx:
        consts = ctx.enter_context(tc.tile_pool(name="consts", bufs=1))
        xpool = ctx.enter_context(tc.tile_pool(name="xpool", bufs=2))
        workp = ctx.enter_context(tc.tile_pool(name="workp", bufs=2))
        statp = ctx.enter_context(tc.tile_pool(name="statp", bufs=1))
        psA = ctx.enter_context(tc.tile_pool(name="psA", bufs=2, space="PSUM"))
        psW = ctx.enter_context(tc.tile_pool(name="psW", bufs=2, space="PSUM"))
        dramp = ctx.enter_context(tc.tile_pool(name="dramp", bufs=1, space="DRAM"))

        # warmup collective: spins up the CC engine early so the real BN
        # all-reduce at the end dispatches without the cold-start gap
        ccw_in = dramp.tile([C, 2], f32, name="ccw_in")
        ccw_out = dramp.tile([C, 2], f32, name="ccw_out", addr_space="Shared")
        nc.gpsimd.collective_compute(
            "AllReduce", ALU.add, ins=[ccw_in.opt()], outs=[ccw_out.opt()],
            replica_groups=[list(range(N_CORES))])

        # ---------------- input DMAs first (sync queue is the x path) ----------
        xa = [None] * NB      # [65, TV] f32 : [X; 1]
        for b in range(NB):
            t = xpool.tile([C + 1, TV], f32, name="xa", tag="xa")
            xa[b] = t
            nc.sync.dma_start(out=t[0:C, :], in_=x_in[b])
            nc.gpsimd.memset(t[C:C + 1, :], 1.0)

        # ---------------- constants / weights (gpsimd DMA queue) --------------
        ones1f = consts.tile([3 * IC + 1, C], f32)
        nc.vector.memset(ones1f, 1.0)
        ones1 = consts.tile([3 * IC + 1, C], f32r)
        nc.vector.tensor_copy(ones1, ones1f)
        # warm up the ACT table: Ln first so the ln+exp set loads once
        warmz = consts.tile([1, 1], f32)
        nc.vector.memset(warmz, 1.0)
        warmo = consts.tile([1, 1], f32)
        nc.scalar.activation(warmo, warmz, AF.Exp)

        wq_sb = consts.tile([IC, C], f32)
        nc.gpsimd.dma_start(out=wq_sb, in_=wq_d)
        wkbk = consts.tile([IC, C + 2], f32)
        nc.vector.memset(wkbk[:, C + 1:C + 2], 0.0)
        nc.gpsimd.dma_start(out=wkbk[:, 0:C], in_=wk_d)
        nc.gpsimd.dma_start(out=wkbk[:, C:C + 1], in_=bk_d)

        # padded to 34 cols: fp32r matmuls need an even moving size
        wvt_aug = consts.tile([C + 1, IC + 2], f32)
        nc.vector.memset(wvt_aug, 0.0)
        nc.gpsimd.dma_start(out=wvt_aug[0:C, 0:IC], in_=wv_d.rearrange("i c -> c i"))
        nc.gpsimd.dma_start(out=wvt_aug[C:C + 1, 0:IC], in_=bv_d)
        nc.vector.memset(wvt_aug[C:C + 1, IC:IC + 1], 1.0)
        wvt_r = consts.tile([C + 1, IC + 2], f16)
        nc.vector.tensor_copy(wvt_r, wvt_aug)

        # Wt^T replicated at partition 0 and 64 for the col-packed PV halves.
        # (bt drops out entirely: BN subtracts the batch mean, which absorbs
        # any per-channel constant added before normalization.)
        wt_rep = consts.tile([IC, C], f32)
        nc.gpsimd.dma_start(out=wt_rep, in_=wt_d.rearrange("c i -> i c"))
        wt_rep_r = consts.tile([IC, C], f32r)
        nc.vector.tensor_copy(wt_rep_r, wt_rep)

        w1t = consts.tile([C, R], f32)
        nc.gpsimd.dma_start(out=w1t, in_=w1_d.rearrange("j c -> c j"))
        w2t = consts.tile([R, C], f32)
        nc.gpsimd.dma_start(out=w2t, in_=w2_d.rearrange("c j -> j c"))
        b1_sb = consts.tile([R, 1], f32)
        nc.gpsimd.dma_start(out=b1_sb, in_=b1_d)
        b2_sb = consts.tile([C, 1], f32)
        nc.gpsimd.dma_start(out=b2_sb, in_=b2_d)
        b2n = consts.tile([C, 1], f32)
        nc.vector.tensor_scalar_mul(b2n, b2_sb, -1.0)
        gamma_sb = consts.tile([C, 1], f32)
        nc.gpsimd.dma_start(out=gamma_sb, in_=gm_d)
        beta_sb = consts.tile([C, 1], f32)
        nc.gpsimd.dma_start(out=beta_sb, in_=bt2_d)

        # G^T | r  =  Wq^T @ [Wk | bk]   -> lhsT for the H matmul
        psg = psW.tile([C, C + 2], f32, name="psg", tag="w")
        nc.tensor.matmul(psg, lhsT=wq_sb, rhs=wkbk, start=True, stop=True)
        gr = consts.tile([C, C + 1], f16)
        nc.vector.tensor_copy(gr, psg[:, 0:C + 1])

        # ---------------- per-batch state ----------------
        xr = [None] * NB      # [65, TV] f32r copy for matmul operands
        ha = [None] * NB      # [65, TV] f32r : [G X; r^T X]
        vt1 = [None] * NB     # [128, 13, 33] f16 : [V^T | 1] per s-block
        eb = [None] * NB      # [128, 13, TV] f16 : exp(A^T)
        p2 = [None] * NB      # [64, TV] f32 : p2 = (Wt p + bt)  (pre-BN)
        avgs = statp.tile([C, NB], f32)
        stats = statp.tile([C, NB * len(CH_P) * 2, 6], f32)

        def prologue(b):
            t = xa[b]
            tr = xpool.tile([C + 1, TV], f16, name="xr", tag="xr")
            xr[b] = tr
            nc.vector.tensor_copy(tr, t)
            h = xpool.tile([C + 1, TV], f16, name="ha", tag="ha")
            ha[b] = h
            for (t0, t1) in CH_P:
                hps = psW.tile([C + 1, 800], f32, name="hps", tag="w")
                for (c0, c1) in CH_H:
                    nc.tensor.matmul(hps[:, c0:c1], lhsT=gr,
                                     rhs=tr[0:C, t0 + c0:t0 + c1],
                                     start=True, stop=True)
                nc.vector.tensor_copy(h[:, t0:t1], hps[:, 0:t1 - t0])
            nc.vector.reduce_sum(avgs[:, b:b + 1], t[0:C, :], axis=AX.X)
            vt1[b] = xpool.tile([128, NSB, IC + 1], f16, name="vt1", tag="vt1")
            eb[b] = xpool.tile([128, NSB, TV], f16, name="eb", tag="eb")
            p2[b] = xpool.tile([C, TV], f32, name="p2", tag="p2")
            # all V^T blocks up front (keeps phase1 PE-dense)
            for j, (off, p) in enumerate(SB):
                vps = psW.tile([128, IC + 2], f32, name="vps", tag="w")
                nc.tensor.matmul(vps[0:p, :], lhsT=tr[:, off:off + p],
                                 rhs=wvt_r, start=True, stop=True)
                nc.vector.tensor_copy(vt1[b][0:p, j, :], vps[0:p, 0:IC + 1])

        def phase1(b):
            """A^T block -> exp -> PV accumulation, pipelined via two
            half-width A psum tiles (exp of one half overlaps matmuls of
            the next)."""
            paccs = []
            for ti, (t0, t1) in enumerate(CH_P):
                paccs.append(psW.tile([IC + 1, 800], f32, name=f"pacc{ti}", tag="w"))
            for j, (off, p) in enumerate(SB):
                for (h0, h1) in HALVES:
                    aps = psA.tile([128, 800], f32, name="aps", tag="aps")
                    for (c0, c1) in CH_H:
                        nc.tensor.matmul(aps[0:p, c0:c1],
                                         lhsT=ha[b][:, off:off + p],
                                         rhs=xr[b][:, h0 + c0:h0 + c1],
                                         start=True, stop=True)
                    nc.scalar.activation(eb[b][0:p, j, h0:h1], aps[0:p, :], AF.Exp)
                for ti, (t0, t1) in enumerate(CH_P):
                    for (c0, c1) in CH_H:
                        nc.tensor.matmul(paccs[ti][0:IC + 1, c0:c1],
                                         lhsT=vt1[b][0:p, j, :],
                                         rhs=eb[b][0:p, j, t0 + c0:t0 + c1],
                                         start=(j == 0), stop=(j == NSB - 1))
            return paccs

        def remainder(b, paccs):
            """PD -> Wt conv -> /D -> bn_stats per chunk."""
            pds = []
            for ti, (t0, t1) in enumerate(CH_P):
                pd = workp.tile([IC + 1, 800], f32r, name="pd", tag="pd")
                pds.append(pd)
                nc.vector.tensor_copy(pd[0:IC + 1, :], paccs[ti][0:IC + 1, :])
            for ti, (t0, t1) in enumerate(CH_P):
                w = t1 - t0
                pd = pds[ti]
                p2ps = psW.tile([C, 800], f32, name="p2ps", tag="w")
                dps = psW.tile([C, 800], f32, name="dps", tag="w")
                for (c0, c1) in CH_R:
                    nc.tensor.matmul(p2ps[:, c0:c1], lhsT=wt_rep_r[0:IC, :],
                                     rhs=pd[0:IC, c0:c1], start=True, stop=True)
                    nc.tensor.matmul(dps[:, c0:c1], lhsT=ones1[IC:IC + 1, :],
                                     rhs=pd[IC:IC + 1, c0:c1], start=True, stop=True)
                rrep = workp.tile([C, 800], f32, name="rrep", tag="rrep")
                nc.vector.reciprocal_approx_fast(out=rrep[:, 0:w], in_=dps[:, 0:w])
                nc.vector.tensor_mul(p2[b][:, t0:t1], p2ps[:, 0:w], rrep[:, 0:w])
                nc.vector.bn_stats(stats[:, 2 * (b * len(CH_P) + ti), :],
                                   p2[b][:, t0:t0 + 512])
                nc.vector.bn_stats(stats[:, 2 * (b * len(CH_P) + ti) + 1, :],
                                   p2[b][:, t0 + 512:t1])

        prologue(0)
        pa0 = phase1(0)
        remainder(0, pa0)
        prologue(1)

        # ---------------- channel gate (hidden under phase1(1)) ----------------
        hps2 = psW.tile([R, NB], f32, name="hps2", tag="w")
        nc.tensor.matmul(hps2, lhsT=w1t, rhs=avgs, start=True, stop=True)
        h_pre = statp.tile([R, NB], f32)
        nc.vector.tensor_scalar(h_pre, hps2, 1.0 / TV, b1_sb,
                                op0=ALU.mult, op1=ALU.add)
        h_sb = statp.tile([R, NB], f32)
        nc.vector.tensor_scalar_max(h_sb, h_pre, 0.0)
        zps = psW.tile([C, NB], f32, name="zps", tag="w")
        nc.tensor.matmul(zps, lhsT=w2t, rhs=h_sb, start=True, stop=True)
        eg = statp.tile([C, NB], f32)
        nc.scalar.activation(eg, zps, AF.Exp, bias=b2n, scale=-1.0)
        gp1 = statp.tile([C, NB], f32)
        nc.vector.tensor_scalar_add(gp1, eg, 1.0)
        gate = statp.tile([C, NB], f32)
        nc.vector.reciprocal(gate, gp1)

        # w_b = gate (.) p2_b can be computed before the stats collective
        wts = [None] * NB

        def w_precompute(b):
            u = workp.tile([C, TV], f32, name="u", tag="u")
            wts[b] = u
            nc.vector.tensor_scalar_mul(u, p2[b], gate[:, b:b + 1])

        w_precompute(0)
        pa1 = phase1(1)
        remainder(1, pa1)
        w_precompute(1)

        # ---------------- BN stats: local -> allgather -> global ----------------
        mv = statp.tile([C, 2], f32)
        nc.vector.bn_aggr(out=mv, in_=stats)
        m2 = statp.tile([C, 1], f32)
        nc.vector.tensor_mul(m2, mv[:, 0:1], mv[:, 0:1])
        ex2 = statp.tile([C, 1], f32)
        nc.vector.tensor_add(ex2, mv[:, 1:2], m2)
        sums = statp.tile([C, 2], f32)
        cnt_local = float(NB * TV)
        nc.vector.tensor_scalar_mul(sums[:, 0:1], mv[:, 0:1], cnt_local)
        nc.vector.tensor_scalar_mul(sums[:, 1:2], ex2, cnt_local)

        cc_in = dramp.tile([C, 2], f32, name="cc_in")
        cc_out = dramp.tile([C, 2], f32, name="cc_out", addr_space="Shared")
        nc.sync.dma_start(out=cc_in, in_=sums)
        nc.gpsimd.collective_compute(
            "AllReduce",
            ALU.add,
            ins=[cc_in.opt()],
            outs=[cc_out.opt()],
            replica_groups=[list(range(N_CORES))],
        )
        gs = statp.tile([C, 2], f32)
        nc.sync.dma_start(out=gs, in_=cc_out)

        inv_cnt = 1.0 / (N * TV)
        mean_g = statp.tile([C, 1], f32)
        nc.vector.tensor_scalar_mul(mean_g, gs[:, 0:1], inv_cnt)
        q_g = statp.tile([C, 1], f32)
        nc.vector.tensor_scalar_mul(q_g, gs[:, 1:2], inv_cnt)
        mg2 = statp.tile([C, 1], f32)
        nc.vector.tensor_mul(mg2, mean_g, mean_g)
        var_g = statp.tile([C, 1], f32)
        nc.vector.tensor_sub(var_g, q_g, mg2)
        ve = statp.tile([C, 1], f32)
        nc.vector.tensor_scalar_add(ve, var_g, EPS)
        sq = statp.tile([C, 1], f32)
        nc.scalar.activation(sq, ve, AF.Sqrt)
        rstd = statp.tile([C, 1], f32)
        nc.vector.reciprocal(rstd, sq)
        sc = statp.tile([C, 1], f32)
        nc.vector.tensor_mul(sc, gamma_sb, rstd)
        msc = statp.tile([C, 1], f32)
        nc.vector.tensor_mul(msc, mean_g, sc)
        sh = statp.tile([C, 1], f32)
        nc.vector.tensor_sub(sh, beta_sb, msc)

        if debug:
            nc.sync.dma_start(out=dbg_ha, in_=ha[0].bitcast(f32))
            nc.sync.dma_start(out=dbg_vt, in_=vt1[0])
            nc.sync.dma_start(out=dbg_eb, in_=eb[0])
            for _b in range(NB):
                nc.sync.dma_start(out=dbg_p2[_b], in_=p2[_b])
            nc.sync.dma_start(out=dbg_gate, in_=gate)
            nc.sync.dma_start(out=dbg_sums, in_=sums)
            nc.sync.dma_start(out=dbg_gs, in_=gs)
            nc.sync.dma_start(out=dbg_scsh[:, 0:1], in_=sc)
            nc.sync.dma_start(out=dbg_scsh[:, 1:2], in_=sh)

        # ------------- finalize: out = sc*(gate*p2) + (x + gate*sh) ------------
        for b in range(NB):
            d_b = statp.tile([C, 1], f32, name=f"d_{b}")
            nc.vector.tensor_mul(d_b, gate[:, b:b + 1], sh)
            x3 = workp.tile([C, TV], f32, name="x3", tag="x3")
            nc.scalar.activation(x3, xa[b][0:C, :], AF.Identity, bias=d_b)
            osb = workp.tile([C, TV], f32, name="osb", tag="osb")
            nc.vector.scalar_tensor_tensor(out=osb, in0=wts[b], scalar=sc,
                                           in1=x3, op0=ALU.mult, op1=ALU.add)
            nc.sync.dma_start(out=out_d[b], in_=osb)


_CACHE = {}


def _get_compiled(debug=False):
    key = ("nc", debug)
    if key in _CACHE:
        return _CACHE[key]
    import concourse.bacc as bacc

    nc = bacc.Bacc("TRN2", target_bir_lowering=False, debug=False,
                   enable_asserts=False, num_devices=N_CORES)
    _build(nc, debug=debug)
    nc.compile()
    _CACHE[key] = nc
    return nc


def _run(inputs, trace=False, debug=False, **kw):
    from concourse import bass_utils

    nc = _get_compiled(debug=debug)
    x = np.ascontiguousarray(np.asarray(inputs["x"], dtype=np.float32))
    x = x.reshape(N, C, TV)
    f = lambda a: np.ascontiguousarray(np.asarray(a, dtype=np.float32))
    common = {
        "wq": f(inputs["Wq"]),
        "wk": f(inputs["Wk"]),
        "bk": f(inputs["bk"]).reshape(IC, 1),
        "wv": f(inputs["Wv"]),
        "bv": f(inputs["bv"]).reshape(1, IC),
        "wt": f(inputs["Wt"]),
        "bt": f(inputs["bt"]).reshape(1, C),
        "gamma": f(inputs["gamma"]).reshape(C, 1),
        "beta": f(inputs["beta"]).reshape(C, 1),
        "w1": f(inputs["W1"]),
        "b1": f(inputs["b1"]).reshape(C // 16, 1),
        "w2": f(inputs["W2"]),
        "b2": f(inputs["b2"]).reshape(C, 1),
    }
    in_maps = []
    for c in range(N_CORES):
        m = dict(common)
        m["x_in"] = np.ascontiguousarray(x[c * NB:(c + 1) * NB])
        in_maps.append(m)
    try:
        res = bass_utils.run_bass_kernel_spmd(
            nc, in_maps, core_ids=list(range(N_CORES)), trace=trace, **kw)
    except Exception:
        import time as _time
        _time.sleep(5)
        res = bass_utils.run_bass_kernel_spmd(
            nc, in_maps, core_ids=list(range(N_CORES)), trace=False, **kw)
    out = np.concatenate([res.results[c]["out"] for c in range(N_CORES)], axis=0)
    return out.reshape(N, C, T, V).astype(np.float32), res


def kernel(**inputs):
    return _run(inputs, trace=False)[0]



# revision 31
# speedup vs baseline: 1.8070x; 1.0296x over previous
"""Trainium2 Bass kernel for nn_FEM_35072702939287 (attention + BN + channel gate).

Math restructuring (validated vs reference to ~1e-6):
  A[t,s] = (Wk x + bk)[:,t] . (Wq x + bq)[:,s]
         = [X_aug^T @ H_aug](t,s) + row-const(t) + const
  where X_aug = [X; 1] (65 x TV), H_aug = [G X ; r^T X], G = Wk^T Wq,
  r = Wq^T bk.  Row-constant terms drop under softmax over s.
  We compute A^T tiles [s_block=128, t] so softmax's denominator
  D[t] = sum_s exp(A^T[s,t]) falls out of the PV matmul by augmenting
  V^T with a ones column.  The division by D is folded past the Wt conv:
  P2 = (Wt^T @ P~) * (1/D broadcast); the conv bias bt cancels under BN.
  BatchNorm batch stats are all-reduced across the 8 cores.
  All pre-exp matmuls (H, A, V^T) run in fp16 (fp32 PSUM accumulate):
  full PE rate, FWL fast weight loads, no fp32r ISA restrictions.

Sharding: data-parallel over batch N=16 -> 2 batches per core x 8 cores.
"""

import os
import numpy as np

N_CORES = 8
N, C, T, V = 16, 64, 64, 25
TV = T * V            # 1600
IC = 32
NB = N // N_CORES     # batches per core
EPS = 1e-5
NSB = 13              # 12 full 128-row s-blocks + one 64-row tail
SB = [(j * 128, 128) for j in range(12)] + [(1536, 64)]
# phase1 A-psum half-tiles [128, 800] (2 banks); chunks bank-aligned inside
HALVES = [(0, 800), (800, 1600)]
CH_H = [(0, 512), (512, 800)]
# H-matmul chunks (PSUM-bank aligned, one bank per matmul)
CH_A = [(0, 512), (512, 1024), (1024, 1536), (1536, 1600)]
# phase2 chunks: two 800-wide accumulators (f16 matmuls may move up to 1024)
CH_P = [(0, 800), (800, 1600)]
# fp32r sub-chunks within an 800-wide psum tile (fp32r moving max is 512)
CH_R = [(0, 512), (512, 800)]


def _build(nc, debug=False):
    import concourse.tile as tile
    from concourse import mybir
    from contextlib import ExitStack

    f32 = mybir.dt.float32
    f32r = mybir.dt.float32r
    f16 = mybir.dt.float16
    AF = mybir.ActivationFunctionType
    ALU = mybir.AluOpType
    AX = mybir.AxisListType

    def r32(ap):
        return ap.bitcast(f32r)

    # ---------------- DRAM I/O ----------------
    x_in = nc.dram_tensor("x_in", [NB, C, TV], f32, kind="ExternalInput").ap()
    wq_d = nc.dram_tensor("wq", [IC, C], f32, kind="ExternalInput").ap()
    wk_d = nc.dram_tensor("wk", [IC, C], f32, kind="ExternalInput").ap()
    bk_d = nc.dram_tensor("bk", [IC, 1], f32, kind="ExternalInput").ap()
    wv_d = nc.dram_tensor("wv", [IC, C], f32, kind="ExternalInput").ap()
    bv_d = nc.dram_tensor("bv", [1, IC], f32, kind="ExternalInput").ap()
    wt_d = nc.dram_tensor("wt", [C, IC], f32, kind="ExternalInput").ap()
    bt_d = nc.dram_tensor("bt", [1, C], f32, kind="ExternalInput").ap()
    gm_d = nc.dram_tensor("gamma", [C, 1], f32, kind="ExternalInput").ap()
    bt2_d = nc.dram_tensor("beta", [C, 1], f32, kind="ExternalInput").ap()
    w1_d = nc.dram_tensor("w1", [C // 16, C], f32, kind="ExternalInput").ap()
    b1_d = nc.dram_tensor("b1", [C // 16, 1], f32, kind="ExternalInput").ap()
    w2_d = nc.dram_tensor("w2", [C, C // 16], f32, kind="ExternalInput").ap()
    b2_d = nc.dram_tensor("b2", [C, 1], f32, kind="ExternalInput").ap()
    out_d = nc.dram_tensor("out", [NB, C, TV], f32, kind="ExternalOutput").ap()
    if debug:
        dbg_ha = nc.dram_tensor("dbg_ha", [C + 1, TV], f32, kind="ExternalOutput").ap()
        dbg_vt = nc.dram_tensor("dbg_vt", [128, NSB, IC + 1], mybir.dt.float16, kind="ExternalOutput").ap()
        dbg_eb = nc.dram_tensor("dbg_eb", [128, NSB, TV], mybir.dt.float16, kind="ExternalOutput").ap()
        dbg_p2 = nc.dram_tensor("dbg_p2", [NB, C, TV], f32, kind="ExternalOutput").ap()
        dbg_gate = nc.dram_tensor("dbg_gate", [C, NB], f32, kind="ExternalOutput").ap()
        dbg_sums = nc.dram_tensor("dbg_sums", [C, 2], f32, kind="ExternalOutput").ap()
        dbg_gs = nc.dram_tensor("dbg_gs", [C, 2], f32, kind="ExternalOutput").ap()
        dbg_scsh = nc.dram_tensor("dbg_scsh", [C, 2], f32, kind="ExternalOutput").ap()

    R = C // 16  # 4

    with tile.TileContext(nc) as tc, ExitStack() as ctx:
        consts = ctx.enter_context(tc.tile_pool(name="consts", bufs=1))
        xpool = ctx.enter_context(tc.tile_pool(name="xpool", bufs=2))
        workp = ctx.enter_context(tc.tile_pool(name="workp", bufs=2))
        statp = ctx.enter_context(tc.tile_pool(name="statp", bufs=1))
        psA = ctx.enter_context(tc.tile_pool(name="psA", bufs=2, space="PSUM"))
        psW = ctx.enter_context(tc.tile_pool(name="psW", bufs=2, space="PSUM"))
        dramp = ctx.enter_context(tc.tile_pool(name="dramp", bufs=1, space="DRAM"))

        # warmup collective: spins up the CC engine early so the real BN
        # all-reduce at the end dispatches without the cold-start gap
        ccw_in = dramp.tile([C, 2], f32, name="ccw_in")
        ccw_out = dramp.tile([C, 2], f32, name="ccw_out", addr_space="Shared")
        nc.gpsimd.collective_compute(
            "AllReduce", ALU.add, ins=[ccw_in.opt()], outs=[ccw_out.opt()],
            replica_groups=[list(range(N_CORES))])

        # ---------------- input DMAs first (sync queue is the x path) ----------
        xa = [None] * NB      # [65, TV] f32 : [X; 1]
        for b in range(NB):
            t = xpool.tile([C + 1, TV], f32, name="xa", tag="xa")
            xa[b] = t
            nc.sync.dma_start(out=t[0:C, :], in_=x_in[b])
            nc.gpsimd.memset(t[C:C + 1, :], 1.0)

        # ---------------- constants / weights (gpsimd DMA queue) --------------
        ones1f = consts.tile([3 * IC + 1, C], f32)
        nc.vector.memset(ones1f, 1.0)
        ones1 = consts.tile([3 * IC + 1, C], f32r)
        nc.vector.tensor_copy(ones1, ones1f)
        # warm up the ACT table: Ln first so the ln+exp set loads once
        warmz = consts.tile([1, 1], f32)
        nc.vector.memset(warmz, 1.0)
        warmo = consts.tile([1, 1], f32)
        nc.scalar.activation(warmo, warmz, AF.Exp)

        wq_sb = consts.tile([IC, C], f32)
        nc.gpsimd.dma_start(out=wq_sb, in_=wq_d)
        wkbk = consts.tile([IC, C + 2], f32)
        nc.vector.memset(wkbk[:, C + 1:C + 2], 0.0)
        nc.gpsimd.dma_start(out=wkbk[:, 0:C], in_=wk_d)
        nc.gpsimd.dma_start(out=wkbk[:, C:C + 1], in_=bk_d)

        # padded to 34 cols: fp32r matmuls need an even moving size
        wvt_aug = consts.tile([C + 1, IC + 2], f32)
        nc.vector.memset(wvt_aug, 0.0)
        nc.gpsimd.dma_start(out=wvt_aug[0:C, 0:IC], in_=wv_d.rearrange("i c -> c i"))
        nc.gpsimd.dma_start(out=wvt_aug[C:C + 1, 0:IC], in_=bv_d)
        nc.vector.memset(wvt_aug[C:C + 1, IC:IC + 1], 1.0)
        wvt_r = consts.tile([C + 1, IC + 2], f16)
        nc.vector.tensor_copy(wvt_r, wvt_aug)

        # Wt^T replicated at partition 0 and 64 for the col-packed PV halves.
        # (bt drops out entirely: BN subtracts the batch mean, which absorbs
        # any per-channel constant added before normalization.)
        wt_rep = consts.tile([IC, C], f32)
        nc.gpsimd.dma_start(out=wt_rep, in_=wt_d.rearrange("c i -> i c"))
        wt_rep_r = consts.tile([IC, C], f32r)
        nc.vector.tensor_copy(wt_rep_r, wt_rep)

        w1t = consts.tile([C, R], f32)
        nc.gpsimd.dma_start(out=w1t, in_=w1_d.rearrange("j c -> c j"))
        w2t = consts.tile([R, C], f32)
        nc.gpsimd.dma_start(out=w2t, in_=w2_d.rearrange("c j -> j c"))
        b1_sb = consts.tile([R, 1], f32)
        nc.gpsimd.dma_start(out=b1_sb, in_=b1_d)
        b2_sb = consts.tile([C, 1], f32)
        nc.gpsimd.dma_start(out=b2_sb, in_=b2_d)
        b2n = consts.tile([C, 1], f32)
        nc.vector.tensor_scalar_mul(b2n, b2_sb, -1.0)
        gamma_sb = consts.tile([C, 1], f32)
        nc.gpsimd.dma_start(out=gamma_sb, in_=gm_d)
        beta_sb = consts.tile([C, 1], f32)
        nc.gpsimd.dma_start(out=beta_sb, in_=bt2_d)

        # G^T | r  =  Wq^T @ [Wk | bk]   -> lhsT for the H matmul
        psg = psW.tile([C, C + 2], f32, name="psg", tag="w")
        nc.tensor.matmul(psg, lhsT=wq_sb, rhs=wkbk, start=True, stop=True)
        gr = consts.tile([C, C + 1], f16)
        nc.vector.tensor_copy(gr, psg[:, 0:C + 1])

        # ---------------- per-batch state ----------------
        xr = [None] * NB      # [65, TV] f32r copy for matmul operands
        ha = [None] * NB      # [65, TV] f32r : [G X; r^T X]
        vt1 = [None] * NB     # [128, 13, 33] f16 : [V^T | 1] per s-block
        eb = [None] * NB      # [128, 13, TV] f16 : exp(A^T)
        p2 = [None] * NB      # [64, TV] f32 : p2 = (Wt p + bt)  (pre-BN)
        avgs = statp.tile([C, NB], f32)
        stats = statp.tile([C, NB * len(CH_P) * 2, 6], f32)

        def prologue(b):
            t = xa[b]
            tr = xpool.tile([C + 1, TV], f16, name="xr", tag="xr")
            xr[b] = tr
            nc.vector.tensor_copy(tr, t)
            h = xpool.tile([C + 1, TV], f16, name="ha", tag="ha")
            ha[b] = h
            for (t0, t1) in CH_P:
                hps = psW.tile([C + 1, 800], f32, name="hps", tag="w")
                for (c0, c1) in CH_H:
                    nc.tensor.matmul(hps[:, c0:c1], lhsT=gr,
                                     rhs=tr[0:C, t0 + c0:t0 + c1],
                                     start=True, stop=True)
                nc.vector.tensor_copy(h[:, t0:t1], hps[:, 0:t1 - t0])
            nc.vector.reduce_sum(avgs[:, b:b + 1], t[0:C, :], axis=AX.X)
            vt1[b] = xpool.tile([128, NSB, IC + 1], f16, name="vt1", tag="vt1")
            eb[b] = xpool.tile([128, NSB, TV], f16, name="eb", tag="eb")
            p2[b] = xpool.tile([C, TV], f32, name="p2", tag="p2")
            # all V^T blocks up front (keeps phase1 PE-dense)
            for j, (off, p) in enumerate(SB):
                vps = psW.tile([128, IC + 2], f32, name="vps", tag="w")
                nc.tensor.matmul(vps[0:p, :], lhsT=tr[:, off:off + p],
                                 rhs=wvt_r, start=True, stop=True)
                nc.vector.tensor_copy(vt1[b][0:p, j, :], vps[0:p, 0:IC + 1])

        def phase1(b):
            """A^T block -> exp -> PV accumulation, pipelined via two
            half-width A psum tiles (exp of one half overlaps matmuls of
            the next)."""
            paccs = []
            for ti, (t0, t1) in enumerate(CH_P):
                paccs.append(psW.tile([IC + 1, 800], f32, name=f"pacc{ti}", tag="w"))
            for j, (off, p) in enumerate(SB):
                for (h0, h1) in HALVES:
                    aps = psA.tile([128, 800], f32, name="aps", tag="aps")
                    for (c0, c1) in CH_H:
                        nc.tensor.matmul(aps[0:p, c0:c1],
                                         lhsT=ha[b][:, off:off + p],
                                         rhs=xr[b][:, h0 + c0:h0 + c1],
                                         start=True, stop=True)
                    if (j * 2 + (h0 > 0)) % 4 == 3:
                        # Schraudolph fast exp on DVE (i16=rne(1477.32*A+15316)
                        # bitcast f16, ~3% max err) -- offloads 1/4 of the exp
                        # stream from the Scalar engine
                        nc.vector.tensor_scalar(
                            eb[b][0:p, j, h0:h1].bitcast(mybir.dt.int16),
                            aps[0:p, :], 1477.319757644609, 15316.0,
                            op0=ALU.mult, op1=ALU.add)
                    else:
                        nc.scalar.activation(eb[b][0:p, j, h0:h1], aps[0:p, :], AF.Exp)
                for ti, (t0, t1) in enumerate(CH_P):
                    for (c0, c1) in CH_H:
                        nc.tensor.matmul(paccs[ti][0:IC + 1, c0:c1],
                                         lhsT=vt1[b][0:p, j, :],
                                         rhs=eb[b][0:p, j, t0 + c0:t0 + c1],
                                         start=(j == 0), stop=(j == NSB - 1))
            return paccs

        def remainder(b, paccs):
            """PD -> Wt conv -> /D -> bn_stats per chunk."""
            pds = []
            for ti, (t0, t1) in enumerate(CH_P):
                pd = workp.tile([IC + 1, 800], f32r, name="pd", tag="pd")
                pds.append(pd)
                nc.vector.tensor_copy(pd[0:IC + 1, :], paccs[ti][0:IC + 1, :])
            for ti, (t0, t1) in enumerate(CH_P):
                w = t1 - t0
                pd = pds[ti]
                p2ps = psW.tile([C, 800], f32, name="p2ps", tag="w")
                dps = psW.tile([C, 800], f32, name="dps", tag="w")
                for (c0, c1) in CH_R:
                    nc.tensor.matmul(p2ps[:, c0:c1], lhsT=wt_rep_r[0:IC, :],
                                     rhs=pd[0:IC, c0:c1], start=True, stop=True)
                    nc.tensor.matmul(dps[:, c0:c1], lhsT=ones1[IC:IC + 1, :],
                                     rhs=pd[IC:IC + 1, c0:c1], start=True, stop=True)
                rrep = workp.tile([C, 800], f32, name="rrep", tag="rrep")
                nc.vector.reciprocal_approx_fast(out=rrep[:, 0:w], in_=dps[:, 0:w])
                nc.vector.tensor_mul(p2[b][:, t0:t1], p2ps[:, 0:w], rrep[:, 0:w])
                nc.vector.bn_stats(stats[:, 2 * (b * len(CH_P) + ti), :],
                                   p2[b][:, t0:t0 + 512])
                nc.vector.bn_stats(stats[:, 2 * (b * len(CH_P) + ti) + 1, :],
                                   p2[b][:, t0 + 512:t1])

        prologue(0)
        pa0 = phase1(0)
        remainder(0, pa0)
        prologue(1)

        # ---------------- channel gate (hidden under phase1(1)) ----------------
        hps2 = psW.tile([R, NB], f32, name="hps2", tag="w")
        nc.tensor.matmul(hps2, lhsT=w1t, rhs=avgs, start=True, stop=True)
        h_pre = statp.tile([R, NB], f32)
        nc.vector.tensor_scalar(h_pre, hps2, 1.0 / TV, b1_sb,
                                op0=ALU.mult, op1=ALU.add)
        h_sb = statp.tile([R, NB], f32)
        nc.vector.tensor_scalar_max(h_sb, h_pre, 0.0)
        zps = psW.tile([C, NB], f32, name="zps", tag="w")
        nc.tensor.matmul(zps, lhsT=w2t, rhs=h_sb, start=True, stop=True)
        eg = statp.tile([C, NB], f32)
        nc.scalar.activation(eg, zps, AF.Exp, bias=b2n, scale=-1.0)
        gp1 = statp.tile([C, NB], f32)
        nc.vector.tensor_scalar_add(gp1, eg, 1.0)
        gate = statp.tile([C, NB], f32)
        nc.vector.reciprocal(gate, gp1)

        # w_b = gate (.) p2_b can be computed before the stats collective
        wts = [None] * NB

        def w_precompute(b):
            u = workp.tile([C, TV], f32, name="u", tag="u")
            wts[b] = u
            nc.vector.tensor_scalar_mul(u, p2[b], gate[:, b:b + 1])

        w_precompute(0)
        pa1 = phase1(1)
        remainder(1, pa1)
        w_precompute(1)

        # ---------------- BN stats: local -> allgather -> global ----------------
        mv = statp.tile([C, 2], f32)
        nc.vector.bn_aggr(out=mv, in_=stats)
        m2 = statp.tile([C, 1], f32)
        nc.vector.tensor_mul(m2, mv[:, 0:1], mv[:, 0:1])
        ex2 = statp.tile([C, 1], f32)
        nc.vector.tensor_add(ex2, mv[:, 1:2], m2)
        sums = statp.tile([C, 2], f32)
        cnt_local = float(NB * TV)
        nc.vector.tensor_scalar_mul(sums[:, 0:1], mv[:, 0:1], cnt_local)
        nc.vector.tensor_scalar_mul(sums[:, 1:2], ex2, cnt_local)

        cc_in = dramp.tile([C, 2], f32, name="cc_in")
        cc_out = dramp.tile([C, 2], f32, name="cc_out", addr_space="Shared")
        nc.sync.dma_start(out=cc_in, in_=sums)
        nc.gpsimd.collective_compute(
            "AllReduce",
            ALU.add,
            ins=[cc_in.opt()],
            outs=[cc_out.opt()],
            replica_groups=[list(range(N_CORES))],
        )
        gs = statp.tile([C, 2], f32)
        nc.sync.dma_start(out=gs, in_=cc_out)

        inv_cnt = 1.0 / (N * TV)
        mean_g = statp.tile([C, 1], f32)
        nc.vector.tensor_scalar_mul(mean_g, gs[:, 0:1], inv_cnt)
        q_g = statp.tile([C, 1], f32)
        nc.vector.tensor_scalar_mul(q_g, gs[:, 1:2], inv_cnt)
        mg2 = statp.tile([C, 1], f32)
        nc.vector.tensor_mul(mg2, mean_g, mean_g)
        var_g = statp.tile([C, 1], f32)
        nc.vector.tensor_sub(var_g, q_g, mg2)
        ve = statp.tile([C, 1], f32)
        nc.vector.tensor_scalar_add(ve, var_g, EPS)
        sq = statp.tile([C, 1], f32)
        nc.scalar.activation(sq, ve, AF.Sqrt)
        rstd = statp.tile([C, 1], f32)
        nc.vector.reciprocal(rstd, sq)
        sc = statp.tile([C, 1], f32)
        nc.vector.tensor_mul(sc, gamma_sb, rstd)
        msc = statp.tile([C, 1], f32)
        nc.vector.tensor_mul(msc, mean_g, sc)
        sh = statp.tile([C, 1], f32)
        nc.vector.tensor_sub(sh, beta_sb, msc)

        if debug:
            nc.sync.dma_start(out=dbg_ha, in_=ha[0].bitcast(f32))
            nc.sync.dma_start(out=dbg_vt, in_=vt1[0])
            nc.sync.dma_start(out=dbg_eb, in_=eb[0])
            for _b in range(NB):
                nc.sync.dma_start(out=dbg_p2[_b], in_=p2[_b])
            nc.sync.dma_start(out=dbg_gate, in_=gate)
            nc.sync.dma_start(out=dbg_sums, in_=sums)
            nc.sync.dma_start(out=dbg_gs, in_=gs)
            nc.sync.dma_start(out=dbg_scsh[:, 0:1], in_=sc)
            nc.sync.dma_start(out=dbg_scsh[:, 1:2], in_=sh)

        # ------------- finalize: out = sc*(gate*p2) + (x + gate*sh) ------------
        for b in range(NB):
            d_b = statp.tile([C, 1], f32, name=f"d_{b}")
            nc.vector.tensor_mul(d_b, gate[:, b:b + 1], sh)
            x3 = workp.tile([C, TV], f32, name="x3", tag="x3")
            nc.scalar.activation(x3, xa[b][0:C, :], AF.Identity, bias=d_b)
            osb = workp.tile([C, TV], f32, name="osb", tag="osb")
            nc.vector.scalar_tensor_tensor(out=osb, in0=wts[b], scalar=sc,
                                           in1=x3, op0=ALU.mult, op1=ALU.add)
            nc.sync.dma_start(out=out_d[b], in_=osb)


_CACHE = {}


def _get_compiled(debug=False):
    key = ("nc", debug)
    if key in _CACHE:
        return _CACHE[key]
    import concourse.bacc as bacc

    nc = bacc.Bacc("TRN2", target_bir_lowering=False, debug=False,
                   enable_asserts=False, num_devices=N_CORES)
    _build(nc, debug=debug)
    nc.compile()
    _CACHE[key] = nc
    return nc


def _run(inputs, trace=False, debug=False, **kw):
    from concourse import bass_utils

    nc = _get_compiled(debug=debug)
    x = np.ascontiguousarray(np.asarray(inputs["x"], dtype=np.float32))
    x = x.reshape(N, C, TV)
    f = lambda a: np.ascontiguousarray(np.asarray(a, dtype=np.float32))
    common = {
        "wq": f(inputs["Wq"]),
        "wk": f(inputs["Wk"]),
        "bk": f(inputs["bk"]).reshape(IC, 1),
        "wv": f(inputs["Wv"]),
        "bv": f(inputs["bv"]).reshape(1, IC),
        "wt": f(inputs["Wt"]),
        "bt": f(inputs["bt"]).reshape(1, C),
        "gamma": f(inputs["gamma"]).reshape(C, 1),
        "beta": f(inputs["beta"]).reshape(C, 1),
        "w1": f(inputs["W1"]),
        "b1": f(inputs["b1"]).reshape(C // 16, 1),
        "w2": f(inputs["W2"]),
        "b2": f(inputs["b2"]).reshape(C, 1),
    }
    in_maps = []
    for c in range(N_CORES):
        m = dict(common)
        m["x_in"] = np.ascontiguousarray(x[c * NB:(c + 1) * NB])
        in_maps.append(m)
    try:
        res = bass_utils.run_bass_kernel_spmd(
            nc, in_maps, core_ids=list(range(N_CORES)), trace=trace, **kw)
    except Exception:
        import time as _time
        _time.sleep(5)
        res = bass_utils.run_bass_kernel_spmd(
            nc, in_maps, core_ids=list(range(N_CORES)), trace=False, **kw)
    out = np.concatenate([res.results[c]["out"] for c in range(N_CORES)], axis=0)
    return out.reshape(N, C, T, V).astype(np.float32), res


def kernel(**inputs):
    return _run(inputs, trace=False)[0]

